# revision 1
# baseline (speedup 1.0000x reference)
"""MLA (DeepSeek-style multi-head latent attention) forward pass on 8 trn2 cores.

Sharding: tensor-parallel over heads (16 heads -> 2 per core). LoRA-A
projections are replicated; o_proj is input-split on the head dim and the
partial outputs are reduced on the host (the unshard step for this TP layout).

On-device layout: activations are kept transposed [feature, seq] so that every
matmul chains without transposes (contraction dim = partition dim). The V
projection swaps matmul operand roles to produce v in natural [seq, vdim]
orientation. Softmax runs over the partition (key) axis: exp via ScalarE, the
denominator via a ones-row matmul, and the broadcast of per-column scalars
across partitions via a K=1 matmul with a ones column. RoPE's rotate-half is a
PE matmul against a constant signed-permutation matrix (engines cannot move
data across partitions). All matmul operands are float32r (FP22 truncated
multiplies at full PE speed for free dims >= 256).
"""
import numpy as np

import concourse.bass as bass
import concourse.tile as tile
from concourse import bacc, mybir
from concourse.bass_utils import run_bass_kernel_spmd

F32 = mybir.dt.float32
F32R = mybir.dt.float32r

HIDDEN = 2048
S = 2048
NUM_HEADS = 16
Q_LORA = 1536
KV_LORA = 512
NOPE = 128
ROPE = 64
VD = 128
QD = NOPE + ROPE            # 192
SCALE = QD ** -0.5
EPS = 1e-6
ROPE_THETA = 10000.0

NCORES = 8
HPC = NUM_HEADS // NCORES   # heads per core = 2
SB = 512                    # seq block
NSB = S // SB               # 4
KT = HIDDEN // 128          # 16 k-tiles of hidden
QLT = Q_LORA // 128         # 12 tiles of q_latent
CT = KV_LORA // 128         # 4 tiles of compressed kv

_CACHE = {}
LAST_RESULT = None


def _build_program():
    nc = bacc.Bacc("TRN2", target_bir_lowering=False, debug=False,
                   num_devices=NCORES)
    d_xt = nc.dram_tensor("xt", [HIDDEN, S], F32R, kind="ExternalInput").ap()
    d_wqa = nc.dram_tensor("wqa_t", [HIDDEN, Q_LORA], F32R, kind="ExternalInput").ap()
    d_wkva = nc.dram_tensor("wkva_t", [HIDDEN, KV_LORA + ROPE], F32R, kind="ExternalInput").ap()
    d_wqb = nc.dram_tensor("wqb_t", [Q_LORA, HPC * QD], F32R, kind="ExternalInput").ap()
    d_wk = nc.dram_tensor("wk_t", [KV_LORA, HPC * NOPE], F32R, kind="ExternalInput").ap()
    d_wv = nc.dram_tensor("wv_t", [KV_LORA, HPC * VD], F32R, kind="ExternalInput").ap()
    d_wo = nc.dram_tensor("wo_t", [HPC * VD, HIDDEN], F32R, kind="ExternalInput").ap()
    d_cos = nc.dram_tensor("cosd", [128, S], F32R, kind="ExternalInput").ap()
    d_sin = nc.dram_tensor("sind", [128, S], F32R, kind="ExternalInput").ap()
    d_msk = nc.dram_tensor("mask", [128, 4, SB], F32R, kind="ExternalInput").ap()
    d_ones = nc.dram_tensor("ones", [128, 128], F32R, kind="ExternalInput").ap()
    d_rotq = nc.dram_tensor("rotq", [128, 128], F32R, kind="ExternalInput").ap()
    d_dupx = nc.dram_tensor("dupx", [64, 128], F32R, kind="ExternalInput").ap()
    d_duprot = nc.dram_tensor("duprot", [64, 128], F32R, kind="ExternalInput").ap()
    d_out = nc.dram_tensor("out", [S, HIDDEN], F32, kind="ExternalOutput").ap()

    with tile.TileContext(nc) as tc:
        _mla(tc, d_xt, d_wqa, d_wkva, d_wqb, d_wk, d_wv, d_wo, d_cos, d_sin,
             d_msk, d_ones, d_rotq, d_dupx, d_duprot, d_out)
    nc.compile()
    return nc


def _mla(tc, d_xt, d_wqa, d_wkva, d_wqb, d_wk, d_wv, d_wo, d_cos, d_sin,
         d_msk, d_ones, d_rotq, d_dupx, d_duprot, d_out):
    nc = tc.nc
    Exp = mybir.ActivationFunctionType.Exp
    Sqrt = mybir.ActivationFunctionType.Sqrt

    with nc.allow_low_precision(reason="fp32r pipeline: matmul operands are "
                                "deliberately rounded to fp22"), \
         tc.tile_pool(name="pconst", bufs=1) as pc, \
         tc.tile_pool(name="pqkv", bufs=1) as pqkv:
        # constants / small weights, resident for the whole kernel
        ones = pc.tile([128, 128], F32R)
        nc.sync.dma_start(out=ones, in_=d_ones)
        wqb = pc.tile([128, QLT, HPC * QD], F32R)
        wk = pc.tile([128, CT, HPC * NOPE], F32R)
        wv = pc.tile([128, CT, HPC * VD], F32R)
        rotq = pc.tile([128, 128], F32R)
        nc.sync.dma_start(out=rotq, in_=d_rotq)
        dupx = pc.tile([64, 128], F32R)
        nc.sync.dma_start(out=dupx, in_=d_dupx)
        duprot = pc.tile([64, 128], F32R)
        nc.sync.dma_start(out=duprot, in_=d_duprot)
        eps1 = pc.tile([1, 1], F32)
        nc.vector.memset(eps1, EPS)

        # persistent per-head tensors (feature-on-partition, full S)
        qn = [pqkv.tile([128, S], F32R, tag=f"qn{h}", name=f"qn{h}") for h in range(HPC)]
        kn = [pqkv.tile([128, S], F32R, tag=f"kn{h}", name=f"kn{h}") for h in range(HPC)]
        qpe = pqkv.tile([128, S], F32R, tag="qpe")    # rows 0-63 h0, 64-127 h1
        kpd = pqkv.tile([128, S], F32R, tag="kpd")    # rope(k_pe) duplicated
        vst = pqkv.tile([128, S // 128, HPC * VD], F32R, tag="vst")
        ao = [pqkv.tile([128, S], F32R, tag=f"ao{h}", name=f"ao{h}")
              for h in range(HPC)]

        # ---------------- stage A: projections, per seq block ----------------
        with tc.tile_pool(name="pcs", bufs=1) as pcs, \
             tc.tile_pool(name="pxt", bufs=19) as pxt, \
             tc.tile_pool(name="pwstr", bufs=3) as pwstr, \
             tc.tile_pool(name="pql", bufs=3) as pql, \
             tc.tile_pool(name="pckv", bufs=5) as pckv, \
             tc.tile_pool(name="psq", bufs=2) as psq, \
             tc.tile_pool(name="pmisc", bufs=2) as pmisc, \
             tc.tile_pool(name="pstatS", bufs=1) as pstatS, \
             tc.tile_pool(name="ppacc", bufs=3, space="PSUM") as ppacc, \
             tc.tile_pool(name="ppstat", bufs=2, space="PSUM") as ppstat, \
             tc.tile_pool(name="ppmt", bufs=3, space="PSUM") as ppmt:
            for b in range(NSB):
                cols = bass.ts(b, SB)
                cosd = pcs.tile([128, SB], F32R, tag="cos", bufs=2)
                nc.sync.dma_start(out=cosd, in_=d_cos[:, cols])
                sind = pcs.tile([128, SB], F32R, tag="sin", bufs=2)
                nc.sync.dma_start(out=sind, in_=d_sin[:, cols])
                xt = []
                for k in range(KT):
                    t = pxt.tile([128, SB], F32R, tag="xt")
                    nc.sync.dma_start(out=t, in_=d_xt[k * 128:(k + 1) * 128, cols])
                    xt.append(t)

                if b == 0:
                    # deferred resident-weight loads: issued after block 0's
                    # xt/lora-weight DMAs so they don't delay the first matmuls
                    nc.sync.dma_start(out=wk, in_=d_wk.rearrange("(t p) f -> p t f", p=128))
                    nc.sync.dma_start(out=wv, in_=d_wv.rearrange("(t p) f -> p t f", p=128))
                    nc.sync.dma_start(out=wqb, in_=d_wqb.rearrange("(t p) f -> p t f", p=128))
                # --- KV LoRA-A: ckv rows [512 c | 64 pe] ---
                ckv = []
                p_cs = ppstat.tile([1, SB], F32, tag="stat")
                for m in range(CT + 1):
                    mw = 128 if m < CT else ROPE
                    wstr = [pwstr.tile([128, KT // 2, 128], F32R, tag="wstr",
                                       name=f"wkva_{m}_{hh}") for hh in range(2)]
                    for hh in range(2):
                        nc.sync.dma_start(
                            out=wstr[hh][:, :, :mw],
                            in_=d_wkva[hh * 1024:(hh + 1) * 1024,
                                       m * 128:m * 128 + mw].rearrange(
                                "(t p) f -> p t f", p=128))
                    p_a = ppacc.tile([128, SB], F32, tag="acc")
                    for k in range(KT):
                        nc.tensor.matmul(p_a[:mw, :], wstr[k // 8][:, k % 8, :mw],
                                         xt[k], start=(k == 0), stop=(k == KT - 1))
                    t = pckv.tile([mw, SB], F32R, tag="ckv")
                    nc.any.tensor_copy(t[:], p_a[:mw, :])
                    ckv.append(t)
                    if m < CT:
                        sq = psq.tile([128, SB], F32R, tag="sq")
                        nc.vector.tensor_mul(sq[:], t[:], t[:])
                        nc.tensor.matmul(p_cs[:], ones[:, 0:1], sq[:],
                                         start=(m == 0), stop=(m == CT - 1))
                # inv rms of compressed kv, replicated across partitions
                cs_s = pstatS.tile([1, SB], F32R, tag="s1")
                nc.scalar.activation(cs_s[:], p_cs[:], Sqrt,
                                     scale=1.0 / KV_LORA, bias=eps1[:])
                p_bc = ppacc.tile([128, SB], F32, tag="acc")
                nc.tensor.matmul(p_bc[:], ones[0:1, :], cs_s[:],
                                 start=True, stop=True)
                invc = pmisc.tile([128, SB], F32R, tag="invc")
                nc.vector.reciprocal(invc[:], p_bc[:])
                for m in range(CT):
                    nc.vector.tensor_mul(ckv[m][:], ckv[m][:], invc[:])

                # --- kv_b: k_nope per head ---
                for h in range(HPC):
                    p_kv = ppmt.tile([128, SB], F32, tag="mt")
                    for k in range(CT):
                        nc.tensor.matmul(p_kv[:], wk[:, k, h * NOPE:(h + 1) * NOPE],
                                         ckv[k][:], start=(k == 0), stop=(k == CT - 1))
                    nc.any.tensor_copy(kn[h][:, cols], p_kv[:])
                # --- v in natural [s, vdim] orientation (swapped operands) ---
                for t4 in range(SB // 128):
                    p_v = ppmt.tile([128, SB], F32, tag="mt")
                    for k in range(CT):
                        nc.tensor.matmul(p_v[:, :HPC * VD],
                                         ckv[k][:, t4 * 128:(t4 + 1) * 128],
                                         wv[:, k, :], start=(k == 0), stop=(k == CT - 1))
                    nc.any.tensor_copy(vst[:, b * (SB // 128) + t4, :],
                                       p_v[:, :HPC * VD])

                # --- k_pe rope + duplicate to both 64-row halves ---
                kpe_raw = ckv[CT]                       # [64, SB]
                p_x = ppacc.tile([128, SB], F32, tag="acc")
                nc.tensor.matmul(p_x[:], dupx[:], kpe_raw[:], start=True, stop=True)
                p_r = ppmt.tile([128, SB], F32, tag="mt")
                nc.tensor.matmul(p_r[:], duprot[:], kpe_raw[:], start=True, stop=True)
                t1 = pmisc.tile([128, SB], F32R, tag="t1")
                nc.vector.tensor_mul(t1[:], p_x[:], cosd[:])
                t2 = pmisc.tile([128, SB], F32R, tag="t2")
                nc.vector.tensor_mul(t2[:], p_r[:], sind[:])
                nc.vector.tensor_add(kpd[:, cols], t1[:], t2[:])

                # --- Q LoRA-A + q_b fused over latent tiles ---
                p_q = [ppmt.tile([128, SB], F32, tag="mt", name=f"p_q{_i}") for _i in range(3)]
                p_qs = ppstat.tile([1, SB], F32, tag="stat")
                for k in range(QLT):
                    wstr = [pwstr.tile([128, KT // 2, 128], F32R, tag="wstr",
                                       name=f"wqa_{k}_{hh}") for hh in range(2)]
                    for hh in range(2):
                        nc.sync.dma_start(
                            out=wstr[hh],
                            in_=d_wqa[hh * 1024:(hh + 1) * 1024,
                                      k * 128:(k + 1) * 128].rearrange(
                                "(t p) f -> p t f", p=128))
                    p_a = ppacc.tile([128, SB], F32, tag="acc")
                    for kk in range(KT):
                        nc.tensor.matmul(p_a[:], wstr[kk // 8][:, kk % 8, :],
                                         xt[kk], start=(kk == 0), stop=(kk == KT - 1))
                    ql = pql.tile([128, SB], F32R, tag="ql")
                    nc.any.tensor_copy(ql[:], p_a[:])
                    sq = psq.tile([128, SB], F32R, tag="sq")
                    nc.vector.tensor_mul(sq[:], ql[:], ql[:])
                    nc.tensor.matmul(p_qs[:], ones[:, 0:1], sq[:],
                                     start=(k == 0), stop=(k == QLT - 1))
                    for mt in range(3):
                        nc.tensor.matmul(p_q[mt][:], wqb[:, k, mt * 128:(mt + 1) * 128],
                                         ql[:], start=(k == 0), stop=(k == QLT - 1))
                qs_s = pstatS.tile([1, SB], F32R, tag="s2")
                nc.scalar.activation(qs_s[:], p_qs[:], Sqrt,
                                     scale=1.0 / Q_LORA, bias=eps1[:])
                p_bc2 = ppacc.tile([128, SB], F32, tag="acc")
                nc.tensor.matmul(p_bc2[:], ones[0:1, :], qs_s[:],
                                 start=True, stop=True)
                invq = pmisc.tile([128, SB], F32R, tag="invq")
                nc.vector.reciprocal(invq[:], p_bc2[:])
                nc.vector.tensor_mul(qn[0][:, cols], p_q[0][:], invq[:])
                nc.vector.tensor_mul(qn[1][:, cols], p_q[1][:], invq[:])
                qpe_raw = psq.tile([128, SB], F32R, tag="sq", name="qpe_raw")
                nc.vector.tensor_mul(qpe_raw[:], p_q[2][:], invq[:])
                # q_pe rope (both heads packed in 64-row halves)
                p_rq = ppacc.tile([128, SB], F32, tag="acc")
                nc.tensor.matmul(p_rq[:], rotq[:], qpe_raw[:], start=True, stop=True)
                t1q = pmisc.tile([128, SB], F32R, tag="t1")
                nc.vector.tensor_mul(t1q[:], qpe_raw[:], cosd[:])
                t2q = pmisc.tile([128, SB], F32R, tag="t2")
                nc.vector.tensor_mul(t2q[:], p_rq[:], sind[:])
                nc.vector.tensor_add(qpe[:, cols], t1q[:], t2q[:])

        # ------- stage B+C: attention per (block, head) + fused o_proj -------
        # qb-outer so each block's o-projection (output-DMA-bound) overlaps
        # the next block's attention compute.
        with tc.tile_pool(name="pbm", bufs=1) as pbm, \
             tc.tile_pool(name="pexp", bufs=3) as pexp, \
             tc.tile_pool(name="pbn", bufs=2) as pbn, \
             tc.tile_pool(name="pout", bufs=3) as pout, \
             tc.tile_pool(name="ppS", bufs=2, space="PSUM") as ppS, \
             tc.tile_pool(name="ppO", bufs=2, space="PSUM") as ppO, \
             tc.tile_pool(name="ppD", bufs=1, space="PSUM") as ppD, \
             tc.tile_pool(name="ppB", bufs=1, space="PSUM") as ppB, \
             tc.tile_pool(name="ppC", bufs=2, space="PSUM") as ppC:
            msk = pbm.tile([128, 4, SB], F32R)
            nc.sync.dma_start(out=msk, in_=d_msk)
            wo = pbm.tile([128, HPC, HIDDEN], F32R)
            nc.sync.dma_start(out=wo, in_=d_wo.rearrange("(t p) f -> p t f", p=128))
            for qb in range(NSB):
                qcols = bass.ts(qb, SB)
                nk = 4 * (qb + 1)
                for h in range(HPC):
                    hp = slice(64 * h, 64 * h + 64)
                    p_o = ppO.tile([128, SB], F32, tag="o")
                    p_d = ppD.tile([1, SB], F32, tag="d")
                    for ik in range(nk):
                        kc = slice(ik * 128, (ik + 1) * 128)
                        p_s = ppS.tile([128, SB], F32, tag="s")
                        nc.tensor.matmul(p_s[:], kn[h][:, kc], qn[h][:, qcols],
                                         start=True, stop=False)
                        nc.tensor.matmul(p_s[:], kpd[hp, kc], qpe[hp, qcols],
                                         start=False, stop=True)
                        e = pexp.tile([128, SB], F32R, tag="e")
                        nc.scalar.activation(e[:], p_s[:], Exp, scale=SCALE)
                        r = ik - 4 * qb
                        if r >= 0:
                            nc.vector.tensor_mul(e[:], e[:], msk[:, r, :])
                        nc.tensor.matmul(p_o[:], vst[:, ik, h * VD:(h + 1) * VD],
                                         e[:], start=(ik == 0), stop=(ik == nk - 1))
                        nc.tensor.matmul(p_d[:], ones[:, 0:1], e[:],
                                         start=(ik == 0), stop=(ik == nk - 1))
                    den = pbn.tile([1, SB], F32R, tag="den")
                    nc.any.tensor_copy(den[:], p_d[:])
                    p_bc = ppB.tile([128, SB], F32, tag="bc")
                    nc.tensor.matmul(p_bc[:], ones[0:1, :], den[:],
                                     start=True, stop=True)
                    rec = pbn.tile([128, SB], F32, tag="rec")
                    nc.vector.reciprocal(rec[:], p_bc[:])
                    nc.vector.tensor_mul(ao[h][:, qcols], p_o[:], rec[:])
                # o-projection for this block's 4 row-tiles (both heads ready)
                for st in range(qb * (SB // 128), (qb + 1) * (SB // 128)):
                    sc = slice(st * 128, (st + 1) * 128)
                    for nb in range(HIDDEN // SB):
                        ncols = bass.ts(nb, SB)
                        p_c = ppC.tile([128, SB], F32, tag="c")
                        for h in range(HPC):
                            nc.tensor.matmul(p_c[:], ao[h][:, sc], wo[:, h, ncols],
                                             start=(h == 0), stop=(h == HPC - 1))
                        ot = pout.tile([128, SB], F32, tag="ot")
                        nc.any.tensor_copy(ot[:], p_c[:])
                        nc.sync.dma_start(out=d_out[sc, ncols], in_=ot[:])


def _host_constants():
    inv_freq = 1.0 / (ROPE_THETA ** (np.arange(0, ROPE, 2, dtype=np.float32) / ROPE))
    t = np.arange(S, dtype=np.float32)
    freqs = np.outer(t, inv_freq)
    emb = np.concatenate([freqs, freqs], -1)          # [S, 64]
    cos, sin = np.cos(emb), np.sin(emb)
    cosd = np.concatenate([cos.T, cos.T], 0).astype(np.float32)   # [128, S]
    sind = np.concatenate([sin.T, sin.T], 0).astype(np.float32)

    msk = np.zeros((128, 4, SB), np.float32)
    for r in range(4):
        for p in range(128):
            k_idx = p + 128 * r
            if k_idx < SB:
                msk[p, r, k_idx:] = 1.0               # keep where k <= q
    onesm = np.ones((128, 128), np.float32)

    # rotate-half as matrices: rot = P @ x, per 64-row block
    Q = np.zeros((64, 64), np.float32)
    for i in range(32):
        Q[i, i + 32] = -1.0
        Q[i + 32, i] = 1.0
    P = np.zeros((128, 128), np.float32)
    P[:64, :64] = Q
    P[64:, 64:] = Q
    rotq = P.T.copy()                                  # lhsT
    D = np.concatenate([np.eye(64, dtype=np.float32)] * 2, 0)   # [128, 64]
    dupx = D.T.copy()                                  # [64, 128]
    duprot = np.concatenate([Q, Q], 0).T.copy()        # [64, 128]
    return cosd, sind, msk, onesm, rotq, dupx, duprot


def kernel(hidden_states, w_q_a, q_a_weight, w_q_b, w_kv_a, kv_a_weight,
           w_kv_b, w_o):
    global LAST_RESULT
    if "nc" not in _CACHE:
        _CACHE["nc"] = _build_program()
    nc = _CACHE["nc"]

    x = np.asarray(hidden_states, np.float32)[0]       # [S, 2048]
    xt = np.ascontiguousarray(x.T)
    wqa_t = np.ascontiguousarray(np.asarray(w_q_a, np.float32).T)
    wkva_t = np.ascontiguousarray(np.asarray(w_kv_a, np.float32).T)
    wqb_eff = np.asarray(w_q_b, np.float32) * np.asarray(q_a_weight, np.float32)[None, :]
    wkvb_eff = np.asarray(w_kv_b, np.float32) * np.asarray(kv_a_weight, np.float32)[None, :]
    won = np.asarray(w_o, np.float32)

    cosd, sind, msk, onesm, rotq, dupx, duprot = _host_constants()
    shared = {"xt": xt, "wqa_t": wqa_t, "wkva_t": wkva_t, "cosd": cosd,
              "sind": sind, "mask": msk, "ones": onesm, "rotq": rotq,
              "dupx": dupx, "duprot": duprot}

    in_maps = []
    for c in range(NCORES):
        h0, h1 = HPC * c, HPC * c + 1
        # wqb_t cols: [h0 nope | h1 nope | h0 pe | h1 pe]
        cols = []
        for h in (h0, h1):
            cols.append(wqb_eff[h * QD:h * QD + NOPE])         # [128, 1536]
        for h in (h0, h1):
            cols.append(wqb_eff[h * QD + NOPE:(h + 1) * QD])   # [64, 1536]
        wqb_t = np.ascontiguousarray(np.concatenate(cols, 0).T)  # [1536, 384]
        wk_t = np.ascontiguousarray(np.concatenate(
            [wkvb_eff[h * (NOPE + VD):h * (NOPE + VD) + NOPE] for h in (h0, h1)],
            0).T)                                               # [512, 256]
        wv_t = np.ascontiguousarray(np.concatenate(
            [wkvb_eff[h * (NOPE + VD) + NOPE:(h + 1) * (NOPE + VD)] for h in (h0, h1)],
            0).T)                                               # [512, 256]
        wo_t = np.ascontiguousarray(np.concatenate(
            [won[:, h * VD:(h + 1) * VD] for h in (h0, h1)], 1).T)  # [256, 2048]
        im = dict(shared)
        im.update({"wqb_t": wqb_t, "wk_t": wk_t, "wv_t": wv_t, "wo_t": wo_t})
        in_maps.append(im)

    res = run_bass_kernel_spmd(nc, in_maps, list(range(NCORES)))
    LAST_RESULT = res
    out = np.zeros((S, HIDDEN), np.float32)
    for c in range(NCORES):
        out += res.results[c]["out"]
    return out.reshape(1, S, HIDDEN)



# revision 7
# speedup vs baseline: 1.4918x; 1.4918x over previous
"""MLA (DeepSeek-style multi-head latent attention) forward on 8 trn2 cores.

Layout v2: sequence-sharded LoRA-A + device collectives + bf16 matmuls.

Each core computes the LoRA-A projections (q_latent, compressed-kv latent,
k_pe) only for its 256-column sequence shard (8x less replicated work than
pure head-TP). The normalized kv latent + rope'd k_pe are AllGathered
(shared by every head); the per-head q vectors are redistributed with two
AllToAlls (one per head of each core's head pair) so attention runs fully
head-local: core c owns heads 2c, 2c+1 over the full sequence. kv_b expands
kn/v from the gathered latent per head; o_proj is input-split on heads and
the partial products are summed on the host (the unshard step).

All matmuls run in bf16 (1 PE cycle/row regardless of free-dim size, half
the DMA/communication bytes of fp32; final accuracy ~4e-3 vs the 2e-2
gate). Softmax runs over the key (partition) axis: exp on the scalar
engine, denominator via a ones-column matmul, broadcast of per-column
scalars via a K=1 matmul. RoPE rotate-half is a matmul against a constant
signed permutation. o_proj results are DMA'd directly from PSUM.
"""
import numpy as np
import ml_dtypes

import concourse.bass as bass
import concourse.tile as tile
from concourse import bacc, mybir
from concourse.bass_utils import run_bass_kernel_spmd

F32 = mybir.dt.float32
BF16 = mybir.dt.bfloat16
NPBF = ml_dtypes.bfloat16

HID = 2048
S = 2048
H = 16
QL = 1536
KVL = 512
NOPE = 128
RP = 64
VD = 128
QD = NOPE + RP              # 192
SCALE = QD ** -0.5
EPS = 1e-6
ROPE_THETA = 10000.0

NC = 8
HPC = 2                     # heads per core
SSH = S // NC               # 256-seq shard
KT = HID // 128             # 16
QLT = QL // 128             # 12
CT = KVL // 128             # 4
SB = 512                    # attention query block
NSB = S // SB               # 4
NEG = -30000.0

_CACHE = {}
LAST_RESULT = None


def _build_program():
    nc = bacc.Bacc("TRN2", target_bir_lowering=False, debug=False,
                   num_devices=NC)
    d_xt = nc.dram_tensor("xt16", [128, KT, SSH], BF16, kind="ExternalInput").ap()
    d_wqa = nc.dram_tensor("wqa16", [128, KT, QL], BF16, kind="ExternalInput").ap()
    d_wkva = nc.dram_tensor("wkva16", [128, KT, KVL + RP], BF16, kind="ExternalInput").ap()
    d_wqb = nc.dram_tensor("wqb16", [128, QLT, H * QD], BF16, kind="ExternalInput").ap()
    d_wk = nc.dram_tensor("wk16", [128, CT, HPC * NOPE], BF16, kind="ExternalInput").ap()
    d_wv = nc.dram_tensor("wv16", [128, CT, HPC * VD], BF16, kind="ExternalInput").ap()
    d_wo = nc.dram_tensor("wo16", [128, HPC, HID], BF16, kind="ExternalInput").ap()
    d_cos = nc.dram_tensor("cosd", [128, SSH], BF16, kind="ExternalInput").ap()
    d_sin = nc.dram_tensor("sind", [128, SSH], BF16, kind="ExternalInput").ap()
    d_msk = nc.dram_tensor("maskadd", [128, 4, SB], F32, kind="ExternalInput").ap()
    d_rotq = nc.dram_tensor("rotq16", [128, 128], BF16, kind="ExternalInput").ap()
    d_out = nc.dram_tensor("out", [S, HID], F32, kind="ExternalOutput").ap()

    with tile.TileContext(nc) as tc:
        _mla(tc, d_xt, d_wqa, d_wkva, d_wqb, d_wk, d_wv, d_wo, d_cos, d_sin,
             d_msk, d_rotq, d_out)
    nc.compile()
    return nc


def _mla(tc, d_xt, d_wqa, d_wkva, d_wqb, d_wk, d_wv, d_wo, d_cos, d_sin,
         d_msk, d_rotq, d_out):
    nc = tc.nc
    Exp = mybir.ActivationFunctionType.Exp
    Sqrt = mybir.ActivationFunctionType.Sqrt
    groups = [list(range(NC))]

    with nc.allow_low_precision(reason="bf16 pipeline"), \
         tc.tile_pool(name="pdram", bufs=1, space="DRAM") as pdram, \
         tc.tile_pool(name="pconst", bufs=1) as pc, \
         tc.tile_pool(name="pglob", bufs=1) as pg:
        # ---- DRAM bounce buffers for collectives ----
        ag_in = pdram.tile([KVL + RP, SSH], BF16)
        ag_out = pdram.tile([NC, KVL + RP, SSH], BF16)
        aa_in = [pdram.tile([NC, QD, SSH], BF16, name=f"aain{i}") for i in range(HPC)]
        aa_out = [pdram.tile([NC, QD, SSH], BF16, name=f"aaout{i}") for i in range(HPC)]

        # ---- small constants ----
        ones_c = pc.tile([128, 1], BF16)
        nc.vector.memset(ones_c, 1.0)
        ones_r = pc.tile([1, 128], BF16)
        nc.vector.memset(ones_r, 1.0)
        eps1 = pc.tile([1, 1], F32)
        nc.vector.memset(eps1, EPS)
        rotq = pc.tile([128, 128], BF16)
        nc.sync.dma_start(out=rotq, in_=d_rotq)
        cosd = pc.tile([128, SSH], BF16)
        nc.sync.dma_start(out=cosd, in_=d_cos)
        sind = pc.tile([128, SSH], BF16)
        nc.sync.dma_start(out=sind, in_=d_sin)

        # =============== stage A: shard projections ===============
        with tc.tile_pool(name="pw", bufs=1) as pw, \
             tc.tile_pool(name="pA", bufs=1) as pa, \
             tc.tile_pool(name="pAq", bufs=3) as paq, \
             tc.tile_pool(name="pAs", bufs=3) as pas, \
             tc.tile_pool(name="ppA", bufs=3, space="PSUM") as ppa, \
             tc.tile_pool(name="ppSt", bufs=2, space="PSUM") as ppst, \
             tc.tile_pool(name="ppM", bufs=2, space="PSUM") as ppm:
            # ---- stage-A weights: per-k-slice tiles so matmuls start early ----
            xt = pw.tile([128, KT, SSH], BF16)
            wkva = pw.tile([128, KT, KVL + RP], BF16)
            for k in range(KT):
                nc.sync.dma_start(out=wkva[:, k, :], in_=d_wkva[:, k, :])
            nc.sync.dma_start(out=xt, in_=d_xt)
            wqa = pw.tile([128, KT, QL], BF16)
            for k in range(KT):
                nc.sync.dma_start(out=wqa[:, k, :], in_=d_wqa[:, k, :])
            wqb = pw.tile([128, QLT, H * QD], BF16)
            for k in range(QLT):
                nc.sync.dma_start(out=wqb[:, k, :], in_=d_wqb[:, k, :])
            # stage-B weights (small; loaded during stage A)
            wk = pg.tile([128, CT, HPC * NOPE], BF16)
            nc.sync.dma_start(out=wk, in_=d_wk)
            wv = pg.tile([128, CT, HPC * VD], BF16)
            nc.sync.dma_start(out=wv, in_=d_wv)
            wo = pg.tile([128, HPC, HID], BF16)
            nc.sync.dma_start(out=wo, in_=d_wo)
            msk = pg.tile([128, 4, SB], F32)
            nc.sync.dma_start(out=msk, in_=d_msk)

            # --- kv LoRA-A ---
            ckvu = pa.tile([128, CT, SSH], BF16)
            kpe = pa.tile([RP, SSH], BF16)
            p_st = ppst.tile([1, SSH], F32, tag="st", name="cstat")
            for m in range(CT + 1):
                mw = 128 if m < CT else RP
                p_a = ppa.tile([128, SSH], F32, tag="a")
                for k in range(KT):
                    nc.tensor.matmul(p_a[:mw, :], wkva[:, k, m * 128:m * 128 + mw],
                                     xt[:, k, :], start=(k == 0), stop=(k == KT - 1))
                if m < CT:
                    nc.any.tensor_copy(ckvu[:, m, :], p_a[:])
                    sq = pas.tile([128, SSH], BF16, tag="sq")
                    nc.vector.tensor_mul(sq[:], ckvu[:, m, :], ckvu[:, m, :])
                    nc.tensor.matmul(p_st[:], ones_c[:], sq[:],
                                     start=(m == 0), stop=(m == CT - 1))
                else:
                    nc.any.tensor_copy(kpe[:], p_a[:mw, :])
            rms_c = pa.tile([1, SSH], BF16)
            nc.scalar.activation(rms_c[:], p_st[:], Sqrt, scale=1.0 / KVL,
                                 bias=eps1[:])
            p_bc = ppm.tile([128, SSH], F32, tag="m")
            nc.tensor.matmul(p_bc[:], ones_r[:], rms_c[:], start=True, stop=True)
            invc = pa.tile([128, SSH], BF16)
            nc.vector.reciprocal(invc[:], p_bc[:])
            ckv = pa.tile([128, CT, SSH], BF16)
            for m in range(CT):
                nc.vector.tensor_mul(ckv[:, m, :], ckvu[:, m, :], invc[:])
                nc.sync.dma_start(out=ag_in[m * 128:(m + 1) * 128, :],
                                  in_=ckv[:, m, :])
            # --- k_pe rope (scale folded: none needed in bf16) ---
            p_rk = ppm.tile([128, SSH], F32, tag="m", name="rotk")
            nc.tensor.matmul(p_rk[:RP, :], rotq[0:RP, 0:RP], kpe[:],
                             start=True, stop=True)
            rk16 = pas.tile([RP, SSH], BF16, tag="rk")
            nc.any.tensor_copy(rk16[:], p_rk[:RP, :])
            t1 = pas.tile([RP, SSH], BF16, tag="t1")
            nc.vector.tensor_mul(t1[:], kpe[:], cosd[0:RP, :])
            t2 = pas.tile([RP, SSH], BF16, tag="t2")
            nc.vector.tensor_mul(t2[:], rk16[:], sind[0:RP, :])
            kpd = pa.tile([RP, SSH], BF16)
            nc.vector.tensor_add(kpd[:], t1[:], t2[:])
            nc.sync.dma_start(out=ag_in[KVL:KVL + RP, :], in_=kpd[:])
            # --- collective #1: AllGather latent+kpe ---
            nc.gpsimd.collective_compute(
                "AllGather", mybir.AluOpType.bypass, replica_groups=groups,
                ins=[ag_in[:].opt()], outs=[ag_out[:].opt()])

            # --- q LoRA-A ---
            qlu = pa.tile([128, QLT, SSH], BF16)
            p_qst = ppst.tile([1, SSH], F32, tag="st", name="qstat")
            for k in range(QLT):
                p_a = ppa.tile([128, SSH], F32, tag="a")
                for kk in range(KT):
                    nc.tensor.matmul(p_a[:], wqa[:, kk, k * 128:(k + 1) * 128],
                                     xt[:, kk, :], start=(kk == 0), stop=(kk == KT - 1))
                nc.any.tensor_copy(qlu[:, k, :], p_a[:])
                sq = pas.tile([128, SSH], BF16, tag="sq")
                nc.vector.tensor_mul(sq[:], qlu[:, k, :], qlu[:, k, :])
                nc.tensor.matmul(p_qst[:], ones_c[:], sq[:],
                                 start=(k == 0), stop=(k == QLT - 1))
            rms_q = pa.tile([1, SSH], BF16)
            nc.scalar.activation(rms_q[:], p_qst[:], Sqrt, scale=1.0 / QL,
                                 bias=eps1[:])
            p_bq = ppm.tile([128, SSH], F32, tag="m")
            nc.tensor.matmul(p_bq[:], ones_r[:], rms_q[:], start=True, stop=True)
            invq = pa.tile([128, SSH], F32)
            nc.vector.reciprocal(invq[:], p_bq[:])

            # --- q_b for all heads; nope tiles 0..15, rope tiles 16..23 ---
            for mt in range(H + NC):
                p_q = ppa.tile([128, SSH], F32, tag="a")
                for k in range(QLT):
                    nc.tensor.matmul(p_q[:], wqb[:, k, mt * 128:(mt + 1) * 128],
                                     qlu[:, k, :], start=(k == 0), stop=(k == QLT - 1))
                if mt < H:
                    qn = paq.tile([128, SSH], BF16, tag="qn")
                    nc.vector.tensor_mul(qn[:], p_q[:], invq[:])
                    nc.sync.dma_start(out=aa_in[mt % 2][mt // 2, 0:NOPE, :], in_=qn[:])
                else:
                    d = mt - H
                    qpe = paq.tile([128, SSH], BF16, tag="qpe")
                    nc.vector.tensor_mul(qpe[:], p_q[:], invq[:])
                    p_rq = ppm.tile([128, SSH], F32, tag="m")
                    nc.tensor.matmul(p_rq[:], rotq[:], qpe[:], start=True, stop=True)
                    rq16 = pas.tile([128, SSH], BF16, tag="rk", name="rq16")
                    nc.any.tensor_copy(rq16[:], p_rq[:])
                    t1q = pas.tile([128, SSH], BF16, tag="t1")
                    nc.vector.tensor_mul(t1q[:], qpe[:], cosd[:])
                    t2q = pas.tile([128, SSH], BF16, tag="t2")
                    nc.vector.tensor_mul(t2q[:], rq16[:], sind[:])
                    qpr = paq.tile([128, SSH], BF16, tag="qpr")
                    nc.vector.tensor_add(qpr[:], t1q[:], t2q[:])
                    nc.sync.dma_start(out=aa_in[0][d, NOPE:QD, :], in_=qpr[0:RP, :])
                    nc.sync.dma_start(out=aa_in[1][d, NOPE:QD, :], in_=qpr[RP:128, :])
            # --- collectives #2/#3: AllToAll q per head of the pair ---
            for i in range(HPC):
                nc.gpsimd.collective_compute(
                    "AllToAll", mybir.AluOpType.bypass, replica_groups=groups,
                    ins=[aa_in[i][:].opt()], outs=[aa_out[i][:].opt()])

        # =============== stage B: head-local attention ===============
        with tc.tile_pool(name="pB", bufs=1) as pb, \
             tc.tile_pool(name="pBe", bufs=4) as pbe, \
             tc.tile_pool(name="pBo", bufs=3) as pbo, \
             tc.tile_pool(name="pBn", bufs=2) as pbn, \
             tc.tile_pool(name="ppS", bufs=2, space="PSUM") as pps, \
             tc.tile_pool(name="ppO", bufs=2, space="PSUM") as ppo, \
             tc.tile_pool(name="ppD", bufs=2, space="PSUM") as ppd, \
             tc.tile_pool(name="ppC", bufs=2, space="PSUM") as ppc:
            ckvg = pb.tile([128, CT, S], BF16)
            for t in range(CT):
                nc.sync.dma_start(
                    out=ckvg[:, t, :].rearrange("p (j c) -> p j c", j=NC),
                    in_=ag_out[:, t * 128:(t + 1) * 128, :].rearrange(
                        "j p c -> p j c"))
            kpdg = pb.tile([RP, S], BF16)
            nc.sync.dma_start(out=kpdg[:].rearrange("p (j c) -> p j c", j=NC),
                              in_=ag_out[:, KVL:KVL + RP, :].rearrange(
                                  "j p c -> p j c"))
            qt = []
            qpt = []
            for h in range(HPC):
                qn_t = pb.tile([128, S], BF16, name=f"qt{h}")
                nc.sync.dma_start(out=qn_t[:].rearrange("p (j c) -> p j c", j=NC),
                                  in_=aa_out[h][:, 0:NOPE, :].rearrange(
                                      "j p c -> p j c"))
                qp_t = pb.tile([RP, S], BF16, name=f"qpt{h}")
                nc.sync.dma_start(out=qp_t[:].rearrange("p (j c) -> p j c", j=NC),
                                  in_=aa_out[h][:, NOPE:QD, :].rearrange(
                                      "j p c -> p j c"))
                qt.append(qn_t)
                qpt.append(qp_t)

            # --- kv_b: kn per head, v (both heads) keys-on-partitions ---
            kn = [pb.tile([128, S], BF16, name=f"kn{h}") for h in range(HPC)]
            for h in range(HPC):
                for cb in range(S // SSH):
                    p_k = ppc.tile([128, SSH], F32, tag="c")
                    for t in range(CT):
                        nc.tensor.matmul(p_k[:], wk[:, t, h * NOPE:(h + 1) * NOPE],
                                         ckvg[:, t, cb * SSH:(cb + 1) * SSH],
                                         start=(t == 0), stop=(t == CT - 1))
                    nc.any.tensor_copy(kn[h][:, cb * SSH:(cb + 1) * SSH], p_k[:])
            vst = pb.tile([128, S // 128, HPC * VD], BF16)
            for sb in range(S // 128):
                p_v = ppc.tile([128, HPC * VD], F32, tag="c")
                for t in range(CT):
                    nc.tensor.matmul(p_v[:], ckvg[:, t, sb * 128:(sb + 1) * 128],
                                     wv[:, t, :], start=(t == 0), stop=(t == CT - 1))
                nc.any.tensor_copy(vst[:, sb, :], p_v[:])

            # --- attention: heads outer (matches AllToAll arrival) ---
            ao = pb.tile([128, NSB, HPC, SB], BF16)
            for h in range(HPC):
                for qb in range(NSB):
                    qcols = bass.ts(qb, SB)
                    nk = 4 * (qb + 1)
                    p_o = ppo.tile([128, SB], F32, tag="o")
                    p_d = ppd.tile([1, SB], F32, tag="d")
                    for ik in range(nk):
                        kc = slice(ik * 128, (ik + 1) * 128)
                        p_s = pps.tile([128, SB], F32, tag="s")
                        nc.tensor.matmul(p_s[:], kn[h][:, kc], qt[h][:, qcols],
                                         start=True, stop=False)
                        nc.tensor.matmul(p_s[:], kpdg[:, kc], qpt[h][:, qcols],
                                         start=False, stop=True)
                        r = ik - 4 * qb
                        if r >= 0:
                            nc.any.tensor_add(p_s[:], p_s[:], msk[:, r, :])
                        e = pbe.tile([128, SB], BF16, tag="e")
                        nc.scalar.activation(e[:], p_s[:], Exp, scale=SCALE)
                        nc.tensor.matmul(p_o[:], vst[:, ik, h * VD:(h + 1) * VD],
                                         e[:], start=(ik == 0), stop=(ik == nk - 1))
                        nc.tensor.matmul(p_d[:], ones_c[:], e[:],
                                         start=(ik == 0), stop=(ik == nk - 1))
                    den = pbn.tile([1, SB], BF16, tag="den")
                    nc.any.tensor_copy(den[:], p_d[:])
                    p_b = ppc.tile([128, SB], F32, tag="c", name="bcast")
                    nc.tensor.matmul(p_b[:], ones_r[:], den[:], start=True, stop=True)
                    rec = pbn.tile([128, SB], F32, tag="rec")
                    nc.vector.reciprocal(rec[:], p_b[:])
                    nc.vector.tensor_mul(ao[:, qb, h, :], p_o[:], rec[:])
                    # o_proj as soon as both heads of this block are done
                    if h == HPC - 1:
                        for st in range(SB // 128):
                            sc = slice(qb * SB + st * 128, qb * SB + (st + 1) * 128)
                            for nb in range(HID // SB):
                                ncols = bass.ts(nb, SB)
                                p_c = ppc.tile([128, SB], F32, tag="c")
                                for hh in range(HPC):
                                    nc.tensor.matmul(
                                        p_c[:], ao[:, qb, hh, st * 128:(st + 1) * 128],
                                        wo[:, hh, ncols],
                                        start=(hh == 0), stop=(hh == HPC - 1))
                                ot = pbo.tile([128, SB], F32, tag="ot")
                                nc.any.tensor_copy(ot[:], p_c[:])
                                nc.sync.dma_start(out=d_out[sc, ncols], in_=ot[:])


def _host_constants():
    inv_freq = 1.0 / (ROPE_THETA ** (np.arange(0, RP, 2, dtype=np.float32) / RP))
    t = np.arange(S, dtype=np.float32)
    freqs = np.outer(t, inv_freq)
    emb = np.concatenate([freqs, freqs], -1)          # [S, 64]
    cos, sin = np.cos(emb), np.sin(emb)
    cosd = np.concatenate([cos.T, cos.T], 0).astype(np.float32)   # [128, S]
    sind = np.concatenate([sin.T, sin.T], 0).astype(np.float32)

    # additive causal mask for diagonal 128-key blocks: [128, 4, 512]
    mska = np.zeros((128, 4, SB), np.float32)
    for r in range(4):
        for p in range(128):
            mska[p, r, :p + 128 * r] = NEG
    # rotate-half as matmul lhsT: same as baseline
    Q = np.zeros((RP, RP), np.float32)
    for i in range(RP // 2):
        Q[i, i + RP // 2] = -1.0
        Q[i + RP // 2, i] = 1.0
    P = np.zeros((128, 128), np.float32)
    P[:RP, :RP] = Q
    P[RP:, RP:] = Q
    rotq = P.T.copy()
    return cosd, sind, mska, rotq


def _tile3(w, kt):
    """[kt*128, F] -> [128, kt, F]"""
    return np.ascontiguousarray(
        w.reshape(kt, 128, w.shape[1]).transpose(1, 0, 2))


def kernel(hidden_states, w_q_a, q_a_weight, w_q_b, w_kv_a, kv_a_weight,
           w_kv_b, w_o):
    global LAST_RESULT
    if "nc" not in _CACHE:
        _CACHE["nc"] = _build_program()
    nc = _CACHE["nc"]

    x = np.asarray(hidden_states, np.float32)[0]       # [S, 2048]
    xt = np.ascontiguousarray(x.T)                     # [2048, S]
    wqa_t = np.asarray(w_q_a, np.float32).T            # [HID, QL]
    wkva_t = np.asarray(w_kv_a, np.float32).T          # [HID, KVL+RP]
    wqb_eff = np.asarray(w_q_b, np.float32) * np.asarray(q_a_weight, np.float32)[None, :]
    wkvb_eff = np.asarray(w_kv_b, np.float32) * np.asarray(kv_a_weight, np.float32)[None, :]
    won = np.asarray(w_o, np.float32)                  # [HID, H*VD]

    # q_b output feature permutation: nope head-major, then rope packed 2/tile
    perm = np.zeros(H * QD, np.int64)
    for h in range(H):
        perm[h * NOPE:(h + 1) * NOPE] = h * QD + np.arange(NOPE)
    base = H * NOPE
    for d in range(NC):
        for j in range(HPC):
            hh = 2 * d + j
            perm[base + d * 128 + j * RP: base + d * 128 + (j + 1) * RP] = \
                hh * QD + NOPE + np.arange(RP)
    wqb_p = wqb_eff[perm, :]                           # [3072, QL]

    cosd, sind, mska, rotq = _host_constants()

    wqa16 = _tile3(wqa_t, KT).astype(NPBF)
    wkva16 = _tile3(wkva_t, KT).astype(NPBF)
    wqb16 = _tile3(np.ascontiguousarray(wqb_p.T), QLT).astype(NPBF)
    rotq16 = rotq.astype(NPBF)

    shared = {"wqa16": wqa16, "wkva16": wkva16, "wqb16": wqb16,
              "maskadd": mska, "rotq16": rotq16}

    in_maps = []
    for c in range(NC):
        h0, h1 = HPC * c, HPC * c + 1
        wk_t = np.concatenate(
            [wkvb_eff[h * (NOPE + VD):h * (NOPE + VD) + NOPE] for h in (h0, h1)],
            0).T                                        # [KVL, 256]
        wv_t = np.concatenate(
            [wkvb_eff[h * (NOPE + VD) + NOPE:(h + 1) * (NOPE + VD)] for h in (h0, h1)],
            0).T                                        # [KVL, 256]
        wo_t = np.stack(
            [np.ascontiguousarray(won[:, h * VD:(h + 1) * VD].T) for h in (h0, h1)],
            1)                                          # [128, 2, HID]
        cols = slice(c * SSH, (c + 1) * SSH)
        im = dict(shared)
        im.update({
            "xt16": _tile3(np.ascontiguousarray(xt[:, cols]), KT).astype(NPBF),
            "wk16": _tile3(wk_t, CT).astype(NPBF),
            "wv16": _tile3(wv_t, CT).astype(NPBF),
            "wo16": np.ascontiguousarray(wo_t).astype(NPBF),
            "cosd": np.ascontiguousarray(cosd[:, cols]).astype(NPBF),
            "sind": np.ascontiguousarray(sind[:, cols]).astype(NPBF),
        })
        in_maps.append(im)

    res = run_bass_kernel_spmd(nc, in_maps, list(range(NC)))
    LAST_RESULT = res
    out = np.zeros((S, HID), np.float32)
    for c in range(NC):
        out += np.asarray(res.results[c]["out"], np.float32)
    return out.reshape(1, S, HID)


# revision 19
# speedup vs baseline: 1.7047x; 1.1427x over previous
"""MLA (DeepSeek-style multi-head latent attention) forward on 8 trn2 cores.

Layout v2: sequence-sharded LoRA-A + device collectives + bf16 matmuls.

Each core computes the LoRA-A projections (q_latent, compressed-kv latent,
k_pe) only for its 256-column sequence shard (8x less replicated work than
pure head-TP). The normalized kv latent + rope'd k_pe are AllGathered
(shared by every head); the per-head q vectors are redistributed with two
AllToAlls (one per head of each core's head pair) so attention runs fully
head-local: core c owns heads 2c, 2c+1 over the full sequence. kv_b expands
kn/v from the gathered latent per head; o_proj is input-split on heads and
the partial products are summed on the host (the unshard step).

All matmuls run in bf16 (1 PE cycle/row regardless of free-dim size, half
the DMA/communication bytes of fp32; final accuracy ~4e-3 vs the 2e-2
gate). Softmax runs over the key (partition) axis: exp on the scalar
engine, denominator via a ones-column matmul, broadcast of per-column
scalars via a K=1 matmul. RoPE rotate-half is a matmul against a constant
signed permutation. o_proj results are DMA'd directly from PSUM.
"""
import numpy as np
import ml_dtypes

import concourse.bass as bass
import concourse.tile as tile
from concourse import bacc, mybir
from concourse.bass_utils import run_bass_kernel_spmd

F32 = mybir.dt.float32
BF16 = mybir.dt.bfloat16
NPBF = ml_dtypes.bfloat16

HID = 2048
S = 2048
H = 16
QL = 1536
KVL = 512
NOPE = 128
RP = 64
VD = 128
QD = NOPE + RP              # 192
SCALE = QD ** -0.5
EPS = 1e-6
ROPE_THETA = 10000.0

NC = 8
HPC = 2                     # heads per core
SSH = S // NC               # 256-seq shard
KT = HID // 128             # 16
QLT = QL // 128             # 12
CT = KVL // 128             # 4
SB = 512                    # attention query block
NSB = S // SB               # 4
NEG = -30000.0

_CACHE = {}
LAST_RESULT = None


def _build_program():
    nc = bacc.Bacc("TRN2", target_bir_lowering=False, debug=False,
                   num_devices=NC)
    d_xt = nc.dram_tensor("xt16", [128, KT, SSH], BF16, kind="ExternalInput").ap()
    d_wqa = nc.dram_tensor("wqa16", [128, KT, QL], BF16, kind="ExternalInput").ap()
    d_wkva = nc.dram_tensor("wkva16", [128, KT, KVL + RP], BF16, kind="ExternalInput").ap()
    d_wqb = nc.dram_tensor("wqb16", [128, QLT, H * QD], BF16, kind="ExternalInput").ap()
    d_wk = nc.dram_tensor("wk16", [128, CT, HPC * NOPE], BF16, kind="ExternalInput").ap()
    d_wv = nc.dram_tensor("wv16", [128, CT, HPC * VD], BF16, kind="ExternalInput").ap()
    d_wo = nc.dram_tensor("wo16", [128, HPC, HID], BF16, kind="ExternalInput").ap()
    d_cos = nc.dram_tensor("cosd", [128, SSH], BF16, kind="ExternalInput").ap()
    d_sin = nc.dram_tensor("sind", [128, SSH], BF16, kind="ExternalInput").ap()
    d_msk = nc.dram_tensor("maskadd", [128, 4, SB], F32, kind="ExternalInput").ap()
    d_rotq = nc.dram_tensor("rotq16", [128, 128], BF16, kind="ExternalInput").ap()
    d_out = nc.dram_tensor("out", [S, HID], F32, kind="ExternalOutput").ap()

    with tile.TileContext(nc) as tc:
        _mla(tc, d_xt, d_wqa, d_wkva, d_wqb, d_wk, d_wv, d_wo, d_cos, d_sin,
             d_msk, d_rotq, d_out)
    nc.compile()
    return nc


def _mla(tc, d_xt, d_wqa, d_wkva, d_wqb, d_wk, d_wv, d_wo, d_cos, d_sin,
         d_msk, d_rotq, d_out):
    nc = tc.nc
    Exp = mybir.ActivationFunctionType.Exp
    Sqrt = mybir.ActivationFunctionType.Sqrt
    groups = [list(range(NC))]

    with nc.allow_low_precision(reason="bf16 pipeline"), \
         tc.tile_pool(name="pdram", bufs=1, space="DRAM") as pdram, \
         tc.tile_pool(name="pconst", bufs=1) as pc, \
         tc.tile_pool(name="pglob", bufs=1) as pg:
        # ---- DRAM bounce buffers for collectives ----
        ag_in = pdram.tile([KVL + RP, SSH], BF16)
        ag_out = pdram.tile([NC, KVL + RP, SSH], BF16)
        aa_in = [pdram.tile([NC, QD, SSH], BF16, name=f"aain{i}") for i in range(HPC)]
        aa_out = [pdram.tile([NC, QD, SSH], BF16, name=f"aaout{i}") for i in range(HPC)]

        # ---- small constants ----
        ones_c = pc.tile([128, 1], BF16)
        nc.vector.memset(ones_c, 1.0)
        ones_r = pc.tile([1, 128], BF16)
        nc.vector.memset(ones_r, 1.0)
        eps1 = pc.tile([1, 1], F32)
        nc.vector.memset(eps1, EPS)
        rotq = pc.tile([128, 128], BF16)
        nc.sync.dma_start(out=rotq, in_=d_rotq)
        cosd = pc.tile([128, SSH], BF16)
        nc.sync.dma_start(out=cosd, in_=d_cos)
        sind = pc.tile([128, SSH], BF16)
        nc.sync.dma_start(out=sind, in_=d_sin)

        # =============== stage A: shard projections ===============
        with tc.tile_pool(name="pw", bufs=1) as pw, \
             tc.tile_pool(name="pA", bufs=1) as pa, \
             tc.tile_pool(name="pAq", bufs=3) as paq, \
             tc.tile_pool(name="pAs", bufs=3) as pas, \
             tc.tile_pool(name="ppA", bufs=3, space="PSUM") as ppa, \
             tc.tile_pool(name="ppSt", bufs=2, space="PSUM") as ppst, \
             tc.tile_pool(name="ppM", bufs=2, space="PSUM") as ppm:
            # ---- stage-A weights: per-k-slice tiles so matmuls start early ----
            xt = pw.tile([128, KT, SSH], BF16)
            # Early (eager) loads on the SP HW queue: only what the first
            # ~20us of compute needs. Everything else is loaded via gpsimd
            # SWDGE triggers placed AFTER the AllGather in program order, so
            # those transfers enter the global DMA FIFO behind the
            # collective staging instead of ahead of it.
            wkva = pw.tile([128, KT, KVL + RP], BF16)
            nc.sync.dma_start(out=xt, in_=d_xt)
            for half in range(2):
                hk = slice(half * KT // 2, (half + 1) * KT // 2)
                nc.sync.dma_start(out=wkva[:, hk, :], in_=d_wkva[:, hk, :])
            wqa = pw.tile([128, KT, QL], BF16)
            # gate wqa-colA behind wkva via a write-after-read dep: the
            # reader consumes both the colA region and the wkva tail, so the
            # colA DMA (a writer of that region) must wait for wkva.
            gate = pc.tile([1, 2], BF16, name="gate")
            nc.vector.tensor_tensor(gate[0:1, 0:1], wqa[0:1, 0, 0:1],
                                    wkva[0:1, KT - 1, 0:1], mybir.AluOpType.mult)
            nc.scalar.dma_start(out=wqa[:, :, 0:QL // 2], in_=d_wqa[:, :, 0:QL // 2])
            wqb = pw.tile([128, QLT, H * QD], BF16)
            wk = pg.tile([128, CT, HPC * NOPE], BF16)
            wv = pg.tile([128, CT, HPC * VD], BF16)
            wo = pg.tile([128, HPC, HID], BF16)
            msk = pg.tile([128, 4, SB], F32)

            # --- kv LoRA-A ---
            ckvu = pa.tile([128, CT, SSH], BF16)
            kpe = pa.tile([RP, SSH], BF16)
            p_st = ppst.tile([1, SSH], F32, tag="st", name="cstat")
            sqc = pa.tile([128, CT, SSH], BF16, name="sqc")
            for m in range(CT + 1):
                mw = 128 if m < CT else RP
                p_a = ppa.tile([128, SSH], F32, tag="a")
                for k in range(KT):
                    nc.tensor.matmul(p_a[:mw, :], wkva[:, k, m * 128:m * 128 + mw],
                                     xt[:, k, :], start=(k == 0), stop=(k == KT - 1))
                if m < CT:
                    nc.vector.tensor_copy(ckvu[:, m, :], p_a[:])
                    nc.vector.tensor_mul(sqc[:, m, :], ckvu[:, m, :], ckvu[:, m, :])
                else:
                    nc.vector.tensor_copy(kpe[:], p_a[:mw, :])
            for m in range(CT):
                nc.tensor.matmul(p_st[:], ones_c[:], sqc[:, m, :],
                                 start=(m == 0), stop=(m == CT - 1))
            rms_c = pa.tile([1, SSH], BF16)
            nc.scalar.activation(rms_c[:], p_st[:], Sqrt, scale=1.0 / KVL,
                                 bias=eps1[:])
            p_bc = ppm.tile([128, SSH], F32, tag="m")
            nc.tensor.matmul(p_bc[:], ones_r[:], rms_c[:], start=True, stop=True)
            invc = pa.tile([128, SSH], BF16)
            nc.vector.reciprocal(invc[:], p_bc[:])
            ckv = pa.tile([128, CT, SSH], BF16)
            for m in range(CT):
                nc.vector.tensor_mul(ckv[:, m, :], ckvu[:, m, :], invc[:])
            nc.gpsimd.dma_start(
                out=ag_in[0:KVL, :].rearrange("(t p) c -> p t c", p=128),
                in_=ckv[:])
            # --- k_pe rope (scale folded: none needed in bf16) ---
            p_rk = ppm.tile([128, SSH], F32, tag="m", name="rotk")
            nc.tensor.matmul(p_rk[:RP, :], rotq[0:RP, 0:RP], kpe[:],
                             start=True, stop=True)
            rk16 = pas.tile([RP, SSH], BF16, tag="rk")
            nc.vector.tensor_copy(rk16[:], p_rk[:RP, :])
            t1 = pas.tile([RP, SSH], BF16, tag="t1")
            nc.vector.tensor_mul(t1[:], kpe[:], cosd[0:RP, :])
            t2 = pas.tile([RP, SSH], BF16, tag="t2")
            nc.vector.tensor_mul(t2[:], rk16[:], sind[0:RP, :])
            kpd = pa.tile([RP, SSH], BF16)
            nc.vector.tensor_add(kpd[:], t1[:], t2[:])
            nc.gpsimd.dma_start(out=ag_in[KVL:KVL + RP, :], in_=kpd[:])
            # --- collective #1: AllGather latent+kpe ---
            nc.gpsimd.collective_compute(
                "AllGather", mybir.AluOpType.bypass, replica_groups=groups,
                ins=[ag_in[:].opt()], outs=[ag_out[:].opt()])
            # deferred bulk weight loads, chained with write-after-read
            # gates so each transfer enters the exclusive DMA FIFO after the
            # AllGather staging and after the previous weight transfer.
            agmark = pc.tile([1, 2], BF16, name="agmark")
            nc.gpsimd.dma_start(out=agmark[0:1, 0:2],
                                in_=ag_in[KVL + RP - 1:KVL + RP, 0:2])
            Mul = mybir.AluOpType.mult

            def gate_read(region, token):
                g = pas.tile([1, 1], BF16, tag="g8")
                nc.vector.tensor_tensor(g[:], region, token, Mul)

            gate_read(wqa[0:1, 0, QL - 1:QL], agmark[0:1, 0:1])
            nc.gpsimd.dma_start(out=wqa[:, :, QL // 2:QL],
                                in_=d_wqa[:, :, QL // 2:QL])
            gate_read(wqb[0:1, 0, H * QD - 1:H * QD], wqa[0:1, 0, QL - 1:QL])
            nc.gpsimd.dma_start(out=wqb[:, :, H * NOPE:H * QD],
                                in_=d_wqb[:, :, H * NOPE:H * QD])
            gate_read(wqb[0:1, 0, 0:1], wqb[0:1, 0, H * QD - 1:H * QD])
            nc.gpsimd.dma_start(out=wqb[:, :, 0:H * NOPE],
                                in_=d_wqb[:, :, 0:H * NOPE])
            for wtile, dsrc in ((wk, d_wk), (wv, d_wv), (wo, d_wo), (msk, d_msk)):
                gate_read(wtile[0:1, 0, 0:1], wqb[0:1, 0, 0:1])
                nc.gpsimd.dma_start(out=wtile, in_=dsrc)

            # --- q LoRA-A ---
            qlu = pa.tile([128, QLT, SSH], BF16)
            p_qst = ppst.tile([1, SSH], F32, tag="st", name="qstat")
            sqq = pa.tile([128, QLT, SSH], BF16, name="sqq")
            for k in range(QLT):
                p_a = ppa.tile([128, SSH], F32, tag="a")
                for kk in range(KT):
                    nc.tensor.matmul(p_a[:], wqa[:, kk, k * 128:(k + 1) * 128],
                                     xt[:, kk, :], start=(kk == 0), stop=(kk == KT - 1))
                nc.vector.tensor_copy(qlu[:, k, :], p_a[:])
                nc.vector.tensor_mul(sqq[:, k, :], qlu[:, k, :], qlu[:, k, :])
            for k in range(QLT):
                nc.tensor.matmul(p_qst[:], ones_c[:], sqq[:, k, :],
                                 start=(k == 0), stop=(k == QLT - 1))
            rms_q = pa.tile([1, SSH], BF16)
            nc.scalar.activation(rms_q[:], p_qst[:], Sqrt, scale=1.0 / QL,
                                 bias=eps1[:])
            p_bq = ppm.tile([128, SSH], F32, tag="m")
            nc.tensor.matmul(p_bq[:], ones_r[:], rms_q[:], start=True, stop=True)
            invq = pa.tile([128, SSH], F32)
            nc.vector.reciprocal(invq[:], p_bq[:])

            # --- q_b for all heads: rope tiles (16..23) first so the rope
            # chain and the AllToAll staging DMAs start as early as possible;
            # nope tiles follow in parity order (A2A#1's inputs first).
            q16 = pa.tile([128, H + NC, SSH], BF16, name="q16")

            def qb_group(mt):
                p_q = ppa.tile([128, SSH], F32, tag="a")
                for k in range(QLT):
                    nc.tensor.matmul(p_q[:], wqb[:, k, mt * 128:(mt + 1) * 128],
                                     qlu[:, k, :], start=(k == 0), stop=(k == QLT - 1))
                nc.vector.tensor_mul(q16[:, mt, :], p_q[:], invq[:])

            for mt in range(H, H + NC):
                qb_group(mt)
            # rope rotate-half + cos/sin (inputs ready; no PE stalls)
            for d in range(NC):
                p_rq = ppm.tile([128, SSH], F32, tag="m")
                nc.tensor.matmul(p_rq[:], rotq[:], q16[:, H + d, :],
                                 start=True, stop=True)
                rq16 = pas.tile([128, SSH], BF16, tag="rk", name="rq16")
                nc.vector.tensor_copy(rq16[:], p_rq[:])
                t1q = pas.tile([128, SSH], BF16, tag="t1")
                nc.vector.tensor_mul(t1q[:], q16[:, H + d, :], cosd[:])
                t2q = pas.tile([128, SSH], BF16, tag="t2")
                nc.vector.tensor_mul(t2q[:], rq16[:], sind[:])
                nc.vector.tensor_add(q16[:, H + d, :], t1q[:], t2q[:])
            for mt in range(0, H, 2):
                qb_group(mt)
            nc.gpsimd.dma_start(
                out=aa_in[0][:, 0:NOPE, :].rearrange("j p c -> p j c"),
                in_=q16[:, 0:H:2, :].rearrange("p j c -> p j c"))
            nc.gpsimd.dma_start(
                out=aa_in[0][:, NOPE:QD, :].rearrange("j p c -> p j c"),
                in_=q16[0:RP, H:H + NC, :])
            nc.gpsimd.collective_compute(
                "AllToAll", mybir.AluOpType.bypass, replica_groups=groups,
                ins=[aa_in[0][:].opt()], outs=[aa_out[0][:].opt()])
            for mt in range(1, H, 2):
                qb_group(mt)
            nc.gpsimd.dma_start(
                out=aa_in[1][:, 0:NOPE, :].rearrange("j p c -> p j c"),
                in_=q16[:, 1:H:2, :].rearrange("p j c -> p j c"))
            nc.gpsimd.dma_start(
                out=aa_in[1][:, NOPE:QD, :].rearrange("j p c -> p j c"),
                in_=q16[RP:128, H:H + NC, :])
            nc.gpsimd.collective_compute(
                "AllToAll", mybir.AluOpType.bypass, replica_groups=groups,
                ins=[aa_in[1][:].opt()], outs=[aa_out[1][:].opt()])

        # =============== stage B: head-local attention ===============
        with tc.tile_pool(name="pB", bufs=1) as pb, \
             tc.tile_pool(name="pBe", bufs=6) as pbe, \
             tc.tile_pool(name="pBo", bufs=3) as pbo, \
             tc.tile_pool(name="pBn", bufs=2) as pbn, \
             tc.tile_pool(name="ppS", bufs=3, space="PSUM") as pps, \
             tc.tile_pool(name="ppO", bufs=2, space="PSUM") as ppo, \
             tc.tile_pool(name="ppD", bufs=1, space="PSUM") as ppd, \
             tc.tile_pool(name="ppC", bufs=2, space="PSUM") as ppc:
            ckvg = pb.tile([128, CT, S], BF16)
            for t in range(CT):
                nc.gpsimd.dma_start(
                    out=ckvg[:, t, :].rearrange("p (j c) -> p j c", j=NC),
                    in_=ag_out[:, t * 128:(t + 1) * 128, :].rearrange(
                        "j p c -> p j c"))
            kpdg = pb.tile([RP, S], BF16)
            nc.gpsimd.dma_start(out=kpdg[:].rearrange("p (j c) -> p j c", j=NC),
                              in_=ag_out[:, KVL:KVL + RP, :].rearrange(
                                  "j p c -> p j c"))
            qt = [pb.tile([128, S], BF16, name=f"qt{h}") for h in range(HPC)]
            qpt = [pb.tile([RP, S], BF16, name=f"qpt{h}") for h in range(HPC)]

            def unpack_q(h):
                nc.gpsimd.dma_start(
                    out=qt[h][:].rearrange("p (j c) -> p j c", j=NC),
                    in_=aa_out[h][:, 0:NOPE, :].rearrange("j p c -> p j c"))
                nc.gpsimd.dma_start(
                    out=qpt[h][:].rearrange("p (j c) -> p j c", j=NC),
                    in_=aa_out[h][:, NOPE:QD, :].rearrange("j p c -> p j c"))

            # --- kv_b: kn per head, v (both heads) keys-on-partitions ---
            kn = [pb.tile([128, S], BF16, name=f"kn{h}") for h in range(HPC)]
            for h in range(HPC):
                for cb in range(S // SSH):
                    p_k = ppc.tile([128, SSH], F32, tag="c")
                    for t in range(CT):
                        nc.tensor.matmul(p_k[:], wk[:, t, h * NOPE:(h + 1) * NOPE],
                                         ckvg[:, t, cb * SSH:(cb + 1) * SSH],
                                         start=(t == 0), stop=(t == CT - 1))
                    nc.any.tensor_copy(kn[h][:, cb * SSH:(cb + 1) * SSH], p_k[:])
            vst = pb.tile([128, S // 128, HPC * VD], BF16)
            for sb in range(S // 128):
                p_v = ppc.tile([128, HPC * VD], F32, tag="c")
                for t in range(CT):
                    nc.tensor.matmul(p_v[:], ckvg[:, t, sb * 128:(sb + 1) * 128],
                                     wv[:, t, :], start=(t == 0), stop=(t == CT - 1))
                nc.any.tensor_copy(vst[:, sb, :], p_v[:])

            # --- attention: heads outer (matches AllToAll arrival).
            # Software-pipelined: AV/den for ik are issued after the scores
            # of ik+1 so the PE never stalls on the exp; the per-(qb,h)
            # normalization finisher is deferred into the next iteration's
            # matmul stream.
            ao = pb.tile([128, NSB, HPC, SB], BF16)
            pending = None

            def finisher(fin):
                h, qb, p_o, p_d = fin
                den = pbn.tile([1, SB], BF16, tag="den")
                nc.vector.tensor_copy(den[:], p_d[:])
                p_b = ppc.tile([128, SB], F32, tag="c", name="bcast")
                nc.tensor.matmul(p_b[:], ones_r[:], den[:], start=True, stop=True)
                rec = pbn.tile([128, SB], F32, tag="rec")
                nc.vector.reciprocal(rec[:], p_b[:])
                nc.vector.tensor_mul(ao[:, qb, h, :], p_o[:], rec[:])

            def oproj(qb):
                for st in range(SB // 128):
                    sc = slice(qb * SB + st * 128, qb * SB + (st + 1) * 128)
                    for nb in range(HID // SB):
                        ncols = bass.ts(nb, SB)
                        p_c = ppc.tile([128, SB], F32, tag="c")
                        for hh in range(HPC):
                            nc.tensor.matmul(
                                p_c[:], ao[:, qb, hh, st * 128:(st + 1) * 128],
                                wo[:, hh, ncols],
                                start=(hh == 0), stop=(hh == HPC - 1))
                        ot = pbo.tile([128, SB], F32, tag="ot")
                        eng = nc.vector if (st + nb) % 2 == 0 else nc.any
                        eng.tensor_copy(ot[:], p_c[:])
                        nc.sync.dma_start(out=d_out[sc, ncols], in_=ot[:])

            for h in range(HPC):
                unpack_q(h)
                for qb in range(NSB):
                    qcols = bass.ts(qb, SB)
                    nk = 4 * (qb + 1)
                    p_o = ppo.tile([128, SB], F32, tag="o")
                    p_d = ppd.tile([1, SB], F32, tag="d")
                    prev_e = None
                    for ik in range(nk):
                        kc = slice(ik * 128, (ik + 1) * 128)
                        p_s = pps.tile([128, SB], F32, tag="s")
                        nc.tensor.matmul(p_s[:], kn[h][:, kc], qt[h][:, qcols],
                                         start=True, stop=False)
                        nc.tensor.matmul(p_s[:], kpdg[:, kc], qpt[h][:, qcols],
                                         start=False, stop=True)
                        if ik == 1 and pending is not None:
                            fin, oqb = pending
                            finisher(fin)
                            pending = None
                            if oqb is not None:
                                oproj(oqb)
                        if prev_e is not None:
                            pik = ik - 1
                            nc.tensor.matmul(p_o[:], vst[:, pik, h * VD:(h + 1) * VD],
                                             prev_e[:], start=(pik == 0),
                                             stop=(pik == nk - 1))
                            nc.tensor.matmul(p_d[:], ones_c[:], prev_e[:],
                                             start=(pik == 0), stop=(pik == nk - 1))
                        r = ik - 4 * qb
                        if r >= 0:
                            nc.any.tensor_add(p_s[:], p_s[:], msk[:, r, :])
                        e = pbe.tile([128, SB], BF16, tag="e")
                        nc.scalar.activation(e[:], p_s[:], Exp, scale=SCALE)
                        prev_e = e
                    nc.tensor.matmul(p_o[:], vst[:, nk - 1, h * VD:(h + 1) * VD],
                                     prev_e[:], start=(nk == 1), stop=True)
                    nc.tensor.matmul(p_d[:], ones_c[:], prev_e[:],
                                     start=(nk == 1), stop=True)
                    pending = ((h, qb, p_o, p_d),
                               qb if h == HPC - 1 else None)
            fin, oqb = pending
            finisher(fin)
            if oqb is not None:
                oproj(oqb)


def _host_constants():
    inv_freq = 1.0 / (ROPE_THETA ** (np.arange(0, RP, 2, dtype=np.float32) / RP))
    t = np.arange(S, dtype=np.float32)
    freqs = np.outer(t, inv_freq)
    emb = np.concatenate([freqs, freqs], -1)          # [S, 64]
    cos, sin = np.cos(emb), np.sin(emb)
    cosd = np.concatenate([cos.T, cos.T], 0).astype(np.float32)   # [128, S]
    sind = np.concatenate([sin.T, sin.T], 0).astype(np.float32)

    # additive causal mask for diagonal 128-key blocks: [128, 4, 512]
    mska = np.zeros((128, 4, SB), np.float32)
    for r in range(4):
        for p in range(128):
            mska[p, r, :p + 128 * r] = NEG
    # rotate-half as matmul lhsT: same as baseline
    Q = np.zeros((RP, RP), np.float32)
    for i in range(RP // 2):
        Q[i, i + RP // 2] = -1.0
        Q[i + RP // 2, i] = 1.0
    P = np.zeros((128, 128), np.float32)
    P[:RP, :RP] = Q
    P[RP:, RP:] = Q
    rotq = P.T.copy()
    return cosd, sind, mska, rotq


def _tile3(w, kt):
    """[kt*128, F] -> [128, kt, F]"""
    return np.ascontiguousarray(
        w.reshape(kt, 128, w.shape[1]).transpose(1, 0, 2))


def kernel(hidden_states, w_q_a, q_a_weight, w_q_b, w_kv_a, kv_a_weight,
           w_kv_b, w_o):
    global LAST_RESULT
    if "nc" not in _CACHE:
        _CACHE["nc"] = _build_program()
    nc = _CACHE["nc"]

    x = np.asarray(hidden_states, np.float32)[0]       # [S, 2048]
    xt = np.ascontiguousarray(x.T)                     # [2048, S]
    wqa_t = np.asarray(w_q_a, np.float32).T            # [HID, QL]
    wkva_t = np.asarray(w_kv_a, np.float32).T          # [HID, KVL+RP]
    wqb_eff = np.asarray(w_q_b, np.float32) * np.asarray(q_a_weight, np.float32)[None, :]
    wkvb_eff = np.asarray(w_kv_b, np.float32) * np.asarray(kv_a_weight, np.float32)[None, :]
    won = np.asarray(w_o, np.float32)                  # [HID, H*VD]

    # q_b output feature permutation: nope head-major, then rope packed 2/tile
    perm = np.zeros(H * QD, np.int64)
    for h in range(H):
        perm[h * NOPE:(h + 1) * NOPE] = h * QD + np.arange(NOPE)
    base = H * NOPE
    for d in range(NC):
        for j in range(HPC):
            hh = 2 * d + j
            perm[base + d * 128 + j * RP: base + d * 128 + (j + 1) * RP] = \
                hh * QD + NOPE + np.arange(RP)
    wqb_p = wqb_eff[perm, :]                           # [3072, QL]

    cosd, sind, mska, rotq = _host_constants()

    wqa16 = _tile3(wqa_t, KT).astype(NPBF)
    wkva16 = _tile3(wkva_t, KT).astype(NPBF)
    wqb16 = _tile3(np.ascontiguousarray(wqb_p.T), QLT).astype(NPBF)
    rotq16 = rotq.astype(NPBF)

    shared = {"wqa16": wqa16, "wkva16": wkva16, "wqb16": wqb16,
              "maskadd": mska, "rotq16": rotq16}

    in_maps = []
    for c in range(NC):
        h0, h1 = HPC * c, HPC * c + 1
        wk_t = np.concatenate(
            [wkvb_eff[h * (NOPE + VD):h * (NOPE + VD) + NOPE] for h in (h0, h1)],
            0).T                                        # [KVL, 256]
        wv_t = np.concatenate(
            [wkvb_eff[h * (NOPE + VD) + NOPE:(h + 1) * (NOPE + VD)] for h in (h0, h1)],
            0).T                                        # [KVL, 256]
        wo_t = np.stack(
            [np.ascontiguousarray(won[:, h * VD:(h + 1) * VD].T) for h in (h0, h1)],
            1)                                          # [128, 2, HID]
        cols = slice(c * SSH, (c + 1) * SSH)
        im = dict(shared)
        im.update({
            "xt16": _tile3(np.ascontiguousarray(xt[:, cols]), KT).astype(NPBF),
            "wk16": _tile3(wk_t, CT).astype(NPBF),
            "wv16": _tile3(wv_t, CT).astype(NPBF),
            "wo16": np.ascontiguousarray(wo_t).astype(NPBF),
            "cosd": np.ascontiguousarray(cosd[:, cols]).astype(NPBF),
            "sind": np.ascontiguousarray(sind[:, cols]).astype(NPBF),
        })
        in_maps.append(im)

    res = run_bass_kernel_spmd(nc, in_maps, list(range(NC)))
    LAST_RESULT = res
    out = np.zeros((S, HID), np.float32)
    for c in range(NC):
        out += np.asarray(res.results[c]["out"], np.float32)
    return out.reshape(1, S, HID)


# revision 22
# speedup vs baseline: 1.7120x; 1.0043x over previous
"""MLA (DeepSeek-style multi-head latent attention) forward on 8 trn2 cores.

Layout v2: sequence-sharded LoRA-A + device collectives + bf16 matmuls.

Each core computes the LoRA-A projections (q_latent, compressed-kv latent,
k_pe) only for its 256-column sequence shard (8x less replicated work than
pure head-TP). The normalized kv latent + rope'd k_pe are AllGathered
(shared by every head); the per-head q vectors are redistributed with two
AllToAlls (one per head of each core's head pair) so attention runs fully
head-local: core c owns heads 2c, 2c+1 over the full sequence. kv_b expands
kn/v from the gathered latent per head; o_proj is input-split on heads and
the partial products are summed on the host (the unshard step).

All matmuls run in bf16 (1 PE cycle/row regardless of free-dim size, half
the DMA/communication bytes of fp32; final accuracy ~4e-3 vs the 2e-2
gate). Softmax runs over the key (partition) axis: exp on the scalar
engine, denominator via a ones-column matmul, broadcast of per-column
scalars via a K=1 matmul. RoPE rotate-half is a matmul against a constant
signed permutation. o_proj results are DMA'd directly from PSUM.
"""
import numpy as np
import ml_dtypes

import concourse.bass as bass
import concourse.tile as tile
from concourse import bacc, mybir
from concourse.bass_utils import run_bass_kernel_spmd

F32 = mybir.dt.float32
BF16 = mybir.dt.bfloat16
NPBF = ml_dtypes.bfloat16

HID = 2048
S = 2048
H = 16
QL = 1536
KVL = 512
NOPE = 128
RP = 64
VD = 128
QD = NOPE + RP              # 192
SCALE = QD ** -0.5
EPS = 1e-6
ROPE_THETA = 10000.0

NC = 8
HPC = 2                     # heads per core
SSH = S // NC               # 256-seq shard
KT = HID // 128             # 16
QLT = QL // 128             # 12
CT = KVL // 128             # 4
SB = 512                    # attention query block
NSB = S // SB               # 4
NEG = -30000.0

_CACHE = {}
LAST_RESULT = None


def _build_program():
    nc = bacc.Bacc("TRN2", target_bir_lowering=False, debug=False,
                   num_devices=NC)
    d_xt = nc.dram_tensor("xt16", [128, KT, SSH], BF16, kind="ExternalInput").ap()
    d_wqa = nc.dram_tensor("wqa16", [128, KT, QL], BF16, kind="ExternalInput").ap()
    d_wkva = nc.dram_tensor("wkva16", [128, KT, KVL + RP], BF16, kind="ExternalInput").ap()
    d_wqb = nc.dram_tensor("wqb16", [128, QLT, H * QD], BF16, kind="ExternalInput").ap()
    d_wk = nc.dram_tensor("wk16", [128, CT, HPC * NOPE], BF16, kind="ExternalInput").ap()
    d_wv = nc.dram_tensor("wv16", [128, CT, HPC * VD], BF16, kind="ExternalInput").ap()
    d_wo = nc.dram_tensor("wo16", [128, HPC, HID], BF16, kind="ExternalInput").ap()
    d_cos = nc.dram_tensor("cosd", [128, SSH], BF16, kind="ExternalInput").ap()
    d_sin = nc.dram_tensor("sind", [128, SSH], BF16, kind="ExternalInput").ap()
    d_msk = nc.dram_tensor("maskadd", [128, 4, SB], F32, kind="ExternalInput").ap()
    d_rotq = nc.dram_tensor("rotq16", [128, 128], BF16, kind="ExternalInput").ap()
    d_out = nc.dram_tensor("out", [S, HID], F32, kind="ExternalOutput").ap()

    with tile.TileContext(nc) as tc:
        _mla(tc, d_xt, d_wqa, d_wkva, d_wqb, d_wk, d_wv, d_wo, d_cos, d_sin,
             d_msk, d_rotq, d_out)
    nc.compile()
    return nc


def _mla(tc, d_xt, d_wqa, d_wkva, d_wqb, d_wk, d_wv, d_wo, d_cos, d_sin,
         d_msk, d_rotq, d_out):
    nc = tc.nc
    Exp = mybir.ActivationFunctionType.Exp
    Sqrt = mybir.ActivationFunctionType.Sqrt
    groups = [list(range(NC))]

    with nc.allow_low_precision(reason="bf16 pipeline"), \
         tc.tile_pool(name="pdram", bufs=1, space="DRAM") as pdram, \
         tc.tile_pool(name="pconst", bufs=1) as pc, \
         tc.tile_pool(name="pglob", bufs=1) as pg:
        # ---- DRAM bounce buffers for collectives ----
        ag_in = pdram.tile([KVL + RP, SSH], BF16)
        ag_out = pdram.tile([NC, KVL + RP, SSH], BF16)
        aa_in = [pdram.tile([NC, QD, SSH], BF16, name=f"aain{i}") for i in range(HPC)]
        aa_out = [pdram.tile([NC, QD, SSH], BF16, name=f"aaout{i}") for i in range(HPC)]

        # ---- small constants ----
        ones_c = pc.tile([128, 1], BF16)
        nc.vector.memset(ones_c, 1.0)
        ones_r = pc.tile([1, 128], BF16)
        nc.vector.memset(ones_r, 1.0)
        eps1 = pc.tile([1, 1], F32)
        nc.vector.memset(eps1, EPS)
        rotq = pc.tile([128, 128], BF16)
        nc.sync.dma_start(out=rotq, in_=d_rotq)
        cosd = pc.tile([128, SSH], BF16)
        nc.sync.dma_start(out=cosd, in_=d_cos)
        sind = pc.tile([128, SSH], BF16)
        nc.sync.dma_start(out=sind, in_=d_sin)

        # =============== stage A: shard projections ===============
        with tc.tile_pool(name="pw", bufs=1) as pw, \
             tc.tile_pool(name="pA", bufs=1) as pa, \
             tc.tile_pool(name="pAq", bufs=3) as paq, \
             tc.tile_pool(name="pAs", bufs=3) as pas, \
             tc.tile_pool(name="ppA", bufs=3, space="PSUM") as ppa, \
             tc.tile_pool(name="ppSt", bufs=2, space="PSUM") as ppst, \
             tc.tile_pool(name="ppM", bufs=2, space="PSUM") as ppm:
            # ---- stage-A weights: per-k-slice tiles so matmuls start early ----
            xt = pw.tile([128, KT, SSH], BF16)
            # Early (eager) loads on the SP HW queue: only what the first
            # ~20us of compute needs. Everything else is loaded via gpsimd
            # SWDGE triggers placed AFTER the AllGather in program order, so
            # those transfers enter the global DMA FIFO behind the
            # collective staging instead of ahead of it.
            wkva = pw.tile([128, KT, KVL + RP], BF16)
            nc.sync.dma_start(out=xt, in_=d_xt)
            for half in range(2):
                hk = slice(half * KT // 2, (half + 1) * KT // 2)
                nc.sync.dma_start(out=wkva[:, hk, :], in_=d_wkva[:, hk, :])
            wqa = pw.tile([128, KT, QL], BF16)
            # gate wqa-colA behind wkva via a write-after-read dep: the
            # reader consumes both the colA region and the wkva tail, so the
            # colA DMA (a writer of that region) must wait for wkva.
            gate = pc.tile([1, 2], BF16, name="gate")
            nc.vector.tensor_tensor(gate[0:1, 0:1], wqa[0:1, 0, 0:1],
                                    wkva[0:1, KT - 1, 0:1], mybir.AluOpType.mult)
            nc.scalar.dma_start(out=wqa[:, :, 0:QL // 2], in_=d_wqa[:, :, 0:QL // 2])
            wqb = pw.tile([128, QLT, H * QD], BF16)
            wk = pg.tile([128, CT, HPC * NOPE], BF16)
            wv = pg.tile([128, CT, HPC * VD], BF16)
            wo = pg.tile([128, HPC, HID], BF16)
            msk = pg.tile([128, 4, SB], F32)

            # --- kv LoRA-A ---
            ckvu = pa.tile([128, CT, SSH], BF16)
            kpe = pa.tile([RP, SSH], BF16)
            p_st = ppst.tile([1, SSH], F32, tag="st", name="cstat")
            sqc = pa.tile([128, CT, SSH], BF16, name="sqc")
            for m in range(CT + 1):
                mw = 128 if m < CT else RP
                p_a = ppa.tile([128, SSH], F32, tag="a")
                for k in range(KT):
                    nc.tensor.matmul(p_a[:mw, :], wkva[:, k, m * 128:m * 128 + mw],
                                     xt[:, k, :], start=(k == 0), stop=(k == KT - 1))
                if m < CT:
                    nc.vector.tensor_copy(ckvu[:, m, :], p_a[:])
                    nc.vector.tensor_mul(sqc[:, m, :], ckvu[:, m, :], ckvu[:, m, :])
                else:
                    nc.vector.tensor_copy(kpe[:], p_a[:mw, :])
            for m in range(CT):
                nc.tensor.matmul(p_st[:], ones_c[:], sqc[:, m, :],
                                 start=(m == 0), stop=(m == CT - 1))
            rms_c = pa.tile([1, SSH], BF16)
            nc.scalar.activation(rms_c[:], p_st[:], Sqrt, scale=1.0 / KVL,
                                 bias=eps1[:])
            p_bc = ppm.tile([128, SSH], F32, tag="m")
            nc.tensor.matmul(p_bc[:], ones_r[:], rms_c[:], start=True, stop=True)
            invc = pa.tile([128, SSH], BF16)
            nc.vector.reciprocal(invc[:], p_bc[:])
            ckv = pa.tile([128, CT, SSH], BF16)
            for m in range(CT):
                nc.vector.tensor_mul(ckv[:, m, :], ckvu[:, m, :], invc[:])
            nc.gpsimd.dma_start(
                out=ag_in[0:KVL, :].rearrange("(t p) c -> p t c", p=128),
                in_=ckv[:])
            # --- k_pe rope (scale folded: none needed in bf16) ---
            p_rk = ppm.tile([128, SSH], F32, tag="m", name="rotk")
            nc.tensor.matmul(p_rk[:RP, :], rotq[0:RP, 0:RP], kpe[:],
                             start=True, stop=True)
            rk16 = pas.tile([RP, SSH], BF16, tag="rk")
            nc.vector.tensor_copy(rk16[:], p_rk[:RP, :])
            t1 = pas.tile([RP, SSH], BF16, tag="t1")
            nc.vector.tensor_mul(t1[:], kpe[:], cosd[0:RP, :])
            t2 = pas.tile([RP, SSH], BF16, tag="t2")
            nc.vector.tensor_mul(t2[:], rk16[:], sind[0:RP, :])
            kpd = pa.tile([RP, SSH], BF16)
            nc.vector.tensor_add(kpd[:], t1[:], t2[:])
            nc.gpsimd.dma_start(out=ag_in[KVL:KVL + RP, :], in_=kpd[:])
            # --- collective #1: AllGather latent+kpe ---
            nc.gpsimd.collective_compute(
                "AllGather", mybir.AluOpType.bypass, replica_groups=groups,
                ins=[ag_in[:].opt()], outs=[ag_out[:].opt()])
            # deferred bulk weight loads, chained with write-after-read
            # gates so each transfer enters the exclusive DMA FIFO after the
            # AllGather staging and after the previous weight transfer.
            agmark = pc.tile([1, 2], BF16, name="agmark")
            nc.gpsimd.dma_start(out=agmark[0:1, 0:2],
                                in_=ag_in[KVL + RP - 1:KVL + RP, 0:2])
            Mul = mybir.AluOpType.mult

            def gate_read(region, token):
                g = pas.tile([1, 1], BF16, tag="g8")
                nc.vector.tensor_tensor(g[:], region, token, Mul)

            gate_read(wqa[0:1, 0, QL - 1:QL], agmark[0:1, 0:1])
            nc.gpsimd.dma_start(out=wqa[:, :, QL // 2:QL],
                                in_=d_wqa[:, :, QL // 2:QL])
            gate_read(wqb[0:1, 0, H * QD - 1:H * QD], wqa[0:1, 0, QL - 1:QL])
            nc.gpsimd.dma_start(out=wqb[:, :, H * NOPE:H * QD],
                                in_=d_wqb[:, :, H * NOPE:H * QD])
            gate_read(wqb[0:1, 0, 0:1], wqb[0:1, 0, H * QD - 1:H * QD])
            nc.gpsimd.dma_start(out=wqb[:, :, 0:H * NOPE],
                                in_=d_wqb[:, :, 0:H * NOPE])
            for wtile, dsrc in ((wk, d_wk), (wv, d_wv), (wo, d_wo), (msk, d_msk)):
                gate_read(wtile[0:1, 0, 0:1], wqb[0:1, 0, 0:1])
                nc.gpsimd.dma_start(out=wtile, in_=dsrc)

            # --- q LoRA-A ---
            qlu = pa.tile([128, QLT, SSH], BF16)
            p_qst = ppst.tile([1, SSH], F32, tag="st", name="qstat")
            sqq = pa.tile([128, QLT, SSH], BF16, name="sqq")
            for k in range(QLT):
                p_a = ppa.tile([128, SSH], F32, tag="a")
                for kk in range(KT):
                    nc.tensor.matmul(p_a[:], wqa[:, kk, k * 128:(k + 1) * 128],
                                     xt[:, kk, :], start=(kk == 0), stop=(kk == KT - 1))
                nc.vector.tensor_copy(qlu[:, k, :], p_a[:])
                nc.vector.tensor_mul(sqq[:, k, :], qlu[:, k, :], qlu[:, k, :])
            for k in range(QLT):
                nc.tensor.matmul(p_qst[:], ones_c[:], sqq[:, k, :],
                                 start=(k == 0), stop=(k == QLT - 1))
            rms_q = pa.tile([1, SSH], BF16)
            nc.scalar.activation(rms_q[:], p_qst[:], Sqrt, scale=1.0 / QL,
                                 bias=eps1[:])
            p_bq = ppm.tile([128, SSH], F32, tag="m")
            nc.tensor.matmul(p_bq[:], ones_r[:], rms_q[:], start=True, stop=True)
            invq = pa.tile([128, SSH], F32)
            nc.vector.reciprocal(invq[:], p_bq[:])

            # --- q_b for all heads: rope tiles (16..23) first so the rope
            # chain and the AllToAll staging DMAs start as early as possible;
            # nope tiles follow in parity order (A2A#1's inputs first).
            q16 = pa.tile([128, H + NC, SSH], BF16, name="q16")

            def qb_group(mt):
                p_q = ppa.tile([128, SSH], F32, tag="a")
                for k in range(QLT):
                    nc.tensor.matmul(p_q[:], wqb[:, k, mt * 128:(mt + 1) * 128],
                                     qlu[:, k, :], start=(k == 0), stop=(k == QLT - 1))
                nc.vector.tensor_mul(q16[:, mt, :], p_q[:], invq[:])

            for mt in range(H, H + NC):
                qb_group(mt)
            # rope rotate-half + cos/sin (inputs ready; no PE stalls)
            for d in range(NC):
                p_rq = ppm.tile([128, SSH], F32, tag="m")
                nc.tensor.matmul(p_rq[:], rotq[:], q16[:, H + d, :],
                                 start=True, stop=True)
                rq16 = pas.tile([128, SSH], BF16, tag="rk", name="rq16")
                nc.vector.tensor_copy(rq16[:], p_rq[:])
                t1q = pas.tile([128, SSH], BF16, tag="t1")
                nc.vector.tensor_mul(t1q[:], q16[:, H + d, :], cosd[:])
                t2q = pas.tile([128, SSH], BF16, tag="t2")
                nc.vector.tensor_mul(t2q[:], rq16[:], sind[:])
                nc.vector.tensor_add(q16[:, H + d, :], t1q[:], t2q[:])
            for mt in range(0, H, 2):
                qb_group(mt)
            nc.gpsimd.dma_start(
                out=aa_in[0][:, 0:NOPE, :].rearrange("j p c -> p j c"),
                in_=q16[:, 0:H:2, :].rearrange("p j c -> p j c"))
            nc.gpsimd.dma_start(
                out=aa_in[0][:, NOPE:QD, :].rearrange("j p c -> p j c"),
                in_=q16[0:RP, H:H + NC, :])
            nc.gpsimd.collective_compute(
                "AllToAll", mybir.AluOpType.bypass, replica_groups=groups,
                ins=[aa_in[0][:].opt()], outs=[aa_out[0][:].opt()])
            for mt in range(1, H, 2):
                qb_group(mt)
            nc.gpsimd.dma_start(
                out=aa_in[1][:, 0:NOPE, :].rearrange("j p c -> p j c"),
                in_=q16[:, 1:H:2, :].rearrange("p j c -> p j c"))
            nc.gpsimd.dma_start(
                out=aa_in[1][:, NOPE:QD, :].rearrange("j p c -> p j c"),
                in_=q16[RP:128, H:H + NC, :])
            nc.gpsimd.collective_compute(
                "AllToAll", mybir.AluOpType.bypass, replica_groups=groups,
                ins=[aa_in[1][:].opt()], outs=[aa_out[1][:].opt()])

        # =============== stage B: head-local attention ===============
        with tc.tile_pool(name="pB", bufs=1) as pb, \
             tc.tile_pool(name="pBe", bufs=6) as pbe, \
             tc.tile_pool(name="pBo", bufs=3) as pbo, \
             tc.tile_pool(name="pBn", bufs=2) as pbn, \
             tc.tile_pool(name="ppS", bufs=3, space="PSUM") as pps, \
             tc.tile_pool(name="ppO", bufs=2, space="PSUM") as ppo, \
             tc.tile_pool(name="ppD", bufs=1, space="PSUM") as ppd, \
             tc.tile_pool(name="ppC", bufs=2, space="PSUM") as ppc:
            ckvg = pb.tile([128, CT, S], BF16)
            for t in range(CT):
                nc.gpsimd.dma_start(
                    out=ckvg[:, t, :].rearrange("p (j c) -> p j c", j=NC),
                    in_=ag_out[:, t * 128:(t + 1) * 128, :].rearrange(
                        "j p c -> p j c"))
            kpdg = pb.tile([RP, S], BF16)
            nc.gpsimd.dma_start(out=kpdg[:].rearrange("p (j c) -> p j c", j=NC),
                              in_=ag_out[:, KVL:KVL + RP, :].rearrange(
                                  "j p c -> p j c"))
            qt = [pb.tile([128, S], BF16, name=f"qt{h}") for h in range(HPC)]
            qpt = [pb.tile([RP, S], BF16, name=f"qpt{h}") for h in range(HPC)]

            def unpack_q(h):
                nc.gpsimd.dma_start(
                    out=qt[h][:].rearrange("p (j c) -> p j c", j=NC),
                    in_=aa_out[h][:, 0:NOPE, :].rearrange("j p c -> p j c"))
                nc.gpsimd.dma_start(
                    out=qpt[h][:].rearrange("p (j c) -> p j c", j=NC),
                    in_=aa_out[h][:, NOPE:QD, :].rearrange("j p c -> p j c"))

            # --- kv_b: kn per head, v (both heads) keys-on-partitions ---
            kn = [pb.tile([128, S], BF16, name=f"kn{h}") for h in range(HPC)]
            for h in range(HPC):
                for cb in range(S // SSH):
                    p_k = ppc.tile([128, SSH], F32, tag="c")
                    for t in range(CT):
                        nc.tensor.matmul(p_k[:], wk[:, t, h * NOPE:(h + 1) * NOPE],
                                         ckvg[:, t, cb * SSH:(cb + 1) * SSH],
                                         start=(t == 0), stop=(t == CT - 1))
                    nc.any.tensor_copy(kn[h][:, cb * SSH:(cb + 1) * SSH], p_k[:])
            vst = pb.tile([128, S // 128, HPC * VD], BF16)
            for sb in range(S // 128):
                p_v = ppc.tile([128, HPC * VD], F32, tag="c")
                for t in range(CT):
                    nc.tensor.matmul(p_v[:], ckvg[:, t, sb * 128:(sb + 1) * 128],
                                     wv[:, t, :], start=(t == 0), stop=(t == CT - 1))
                nc.any.tensor_copy(vst[:, sb, :], p_v[:])

            # --- attention: heads outer (matches AllToAll arrival).
            # Software-pipelined: AV/den for ik are issued after the scores
            # of ik+1 so the PE never stalls on the exp; the per-(qb,h)
            # normalization finisher is deferred into the next iteration's
            # matmul stream.
            ao = pb.tile([128, NSB, HPC, SB], BF16)
            pending = None

            def finisher(fin):
                h, qb, p_o, p_d = fin
                den = pbn.tile([1, SB], BF16, tag="den")
                nc.vector.tensor_copy(den[:], p_d[:])
                p_b = ppc.tile([128, SB], F32, tag="c", name="bcast")
                nc.tensor.matmul(p_b[:], ones_r[:], den[:], start=True, stop=True)
                rec = pbn.tile([128, SB], F32, tag="rec")
                nc.vector.reciprocal(rec[:], p_b[:])
                nc.vector.tensor_mul(ao[:, qb, h, :], p_o[:], rec[:])

            def oproj(qb):
                for st in range(SB // 128):
                    sc = slice(qb * SB + st * 128, qb * SB + (st + 1) * 128)
                    for nb in range(HID // SB):
                        ncols = bass.ts(nb, SB)
                        p_c = ppc.tile([128, SB], F32, tag="c")
                        for hh in range(HPC):
                            nc.tensor.matmul(
                                p_c[:], ao[:, qb, hh, st * 128:(st + 1) * 128],
                                wo[:, hh, ncols],
                                start=(hh == 0), stop=(hh == HPC - 1))
                        ot = pbo.tile([128, SB], F32, tag="ot")
                        if (st + nb) % 2 == 0:
                            nc.vector.tensor_copy(ot[:], p_c[:])
                        else:
                            nc.scalar.activation(
                                ot[:], p_c[:], mybir.ActivationFunctionType.Copy)
                        eng = None
                        nc.sync.dma_start(out=d_out[sc, ncols], in_=ot[:])

            for h in range(HPC):
                unpack_q(h)
                for qb in range(NSB):
                    qcols = bass.ts(qb, SB)
                    nk = 4 * (qb + 1)
                    p_o = ppo.tile([128, SB], F32, tag="o")
                    p_d = ppd.tile([1, SB], F32, tag="d")
                    prev_e = None
                    for ik in range(nk):
                        kc = slice(ik * 128, (ik + 1) * 128)
                        p_s = pps.tile([128, SB], F32, tag="s")
                        nc.tensor.matmul(p_s[:], kn[h][:, kc], qt[h][:, qcols],
                                         start=True, stop=False)
                        nc.tensor.matmul(p_s[:], kpdg[:, kc], qpt[h][:, qcols],
                                         start=False, stop=True)
                        if ik == 1 and pending is not None:
                            fin, oqb = pending
                            finisher(fin)
                            pending = None
                            if oqb is not None:
                                oproj(oqb)
                        if prev_e is not None:
                            pik = ik - 1
                            nc.tensor.matmul(p_o[:], vst[:, pik, h * VD:(h + 1) * VD],
                                             prev_e[:], start=(pik == 0),
                                             stop=(pik == nk - 1))
                            nc.tensor.matmul(p_d[:], ones_c[:], prev_e[:],
                                             start=(pik == 0), stop=(pik == nk - 1))
                        r = ik - 4 * qb
                        if r >= 0:
                            nc.any.tensor_add(p_s[:], p_s[:], msk[:, r, :])
                        e = pbe.tile([128, SB], BF16, tag="e")
                        nc.scalar.activation(e[:], p_s[:], Exp, scale=SCALE)
                        prev_e = e
                    nc.tensor.matmul(p_o[:], vst[:, nk - 1, h * VD:(h + 1) * VD],
                                     prev_e[:], start=(nk == 1), stop=True)
                    nc.tensor.matmul(p_d[:], ones_c[:], prev_e[:],
                                     start=(nk == 1), stop=True)
                    pending = ((h, qb, p_o, p_d),
                               qb if h == HPC - 1 else None)
            fin, oqb = pending
            finisher(fin)
            if oqb is not None:
                oproj(oqb)


def _host_constants():
    inv_freq = 1.0 / (ROPE_THETA ** (np.arange(0, RP, 2, dtype=np.float32) / RP))
    t = np.arange(S, dtype=np.float32)
    freqs = np.outer(t, inv_freq)
    emb = np.concatenate([freqs, freqs], -1)          # [S, 64]
    cos, sin = np.cos(emb), np.sin(emb)
    cosd = np.concatenate([cos.T, cos.T], 0).astype(np.float32)   # [128, S]
    sind = np.concatenate([sin.T, sin.T], 0).astype(np.float32)

    # additive causal mask for diagonal 128-key blocks: [128, 4, 512]
    mska = np.zeros((128, 4, SB), np.float32)
    for r in range(4):
        for p in range(128):
            mska[p, r, :p + 128 * r] = NEG
    # rotate-half as matmul lhsT: same as baseline
    Q = np.zeros((RP, RP), np.float32)
    for i in range(RP // 2):
        Q[i, i + RP // 2] = -1.0
        Q[i + RP // 2, i] = 1.0
    P = np.zeros((128, 128), np.float32)
    P[:RP, :RP] = Q
    P[RP:, RP:] = Q
    rotq = P.T.copy()
    return cosd, sind, mska, rotq


def _tile3(w, kt):
    """[kt*128, F] -> [128, kt, F]"""
    return np.ascontiguousarray(
        w.reshape(kt, 128, w.shape[1]).transpose(1, 0, 2))


def kernel(hidden_states, w_q_a, q_a_weight, w_q_b, w_kv_a, kv_a_weight,
           w_kv_b, w_o):
    global LAST_RESULT
    if "nc" not in _CACHE:
        _CACHE["nc"] = _build_program()
    nc = _CACHE["nc"]

    x = np.asarray(hidden_states, np.float32)[0]       # [S, 2048]
    xt = np.ascontiguousarray(x.T)                     # [2048, S]
    wqa_t = np.asarray(w_q_a, np.float32).T            # [HID, QL]
    wkva_t = np.asarray(w_kv_a, np.float32).T          # [HID, KVL+RP]
    wqb_eff = np.asarray(w_q_b, np.float32) * np.asarray(q_a_weight, np.float32)[None, :]
    wkvb_eff = np.asarray(w_kv_b, np.float32) * np.asarray(kv_a_weight, np.float32)[None, :]
    won = np.asarray(w_o, np.float32)                  # [HID, H*VD]

    # q_b output feature permutation: nope head-major, then rope packed 2/tile
    perm = np.zeros(H * QD, np.int64)
    for h in range(H):
        perm[h * NOPE:(h + 1) * NOPE] = h * QD + np.arange(NOPE)
    base = H * NOPE
    for d in range(NC):
        for j in range(HPC):
            hh = 2 * d + j
            perm[base + d * 128 + j * RP: base + d * 128 + (j + 1) * RP] = \
                hh * QD + NOPE + np.arange(RP)
    wqb_p = wqb_eff[perm, :]                           # [3072, QL]

    cosd, sind, mska, rotq = _host_constants()

    wqa16 = _tile3(wqa_t, KT).astype(NPBF)
    wkva16 = _tile3(wkva_t, KT).astype(NPBF)
    wqb16 = _tile3(np.ascontiguousarray(wqb_p.T), QLT).astype(NPBF)
    rotq16 = rotq.astype(NPBF)

    shared = {"wqa16": wqa16, "wkva16": wkva16, "wqb16": wqb16,
              "maskadd": mska, "rotq16": rotq16}

    in_maps = []
    for c in range(NC):
        h0, h1 = HPC * c, HPC * c + 1
        wk_t = np.concatenate(
            [wkvb_eff[h * (NOPE + VD):h * (NOPE + VD) + NOPE] for h in (h0, h1)],
            0).T                                        # [KVL, 256]
        wv_t = np.concatenate(
            [wkvb_eff[h * (NOPE + VD) + NOPE:(h + 1) * (NOPE + VD)] for h in (h0, h1)],
            0).T                                        # [KVL, 256]
        wo_t = np.stack(
            [np.ascontiguousarray(won[:, h * VD:(h + 1) * VD].T) for h in (h0, h1)],
            1)                                          # [128, 2, HID]
        cols = slice(c * SSH, (c + 1) * SSH)
        im = dict(shared)
        im.update({
            "xt16": _tile3(np.ascontiguousarray(xt[:, cols]), KT).astype(NPBF),
            "wk16": _tile3(wk_t, CT).astype(NPBF),
            "wv16": _tile3(wv_t, CT).astype(NPBF),
            "wo16": np.ascontiguousarray(wo_t).astype(NPBF),
            "cosd": np.ascontiguousarray(cosd[:, cols]).astype(NPBF),
            "sind": np.ascontiguousarray(sind[:, cols]).astype(NPBF),
        })
        in_maps.append(im)

    res = run_bass_kernel_spmd(nc, in_maps, list(range(NC)))
    LAST_RESULT = res
    out = np.zeros((S, HID), np.float32)
    for c in range(NC):
        out += np.asarray(res.results[c]["out"], np.float32)
    return out.reshape(1, S, HID)


# revision 24
# speedup vs baseline: 1.8044x; 1.0540x over previous
"""MLA (DeepSeek-style multi-head latent attention) forward on 8 trn2 cores.

Layout v2: sequence-sharded LoRA-A + device collectives + bf16 matmuls.

Each core computes the LoRA-A projections (q_latent, compressed-kv latent,
k_pe) only for its 256-column sequence shard (8x less replicated work than
pure head-TP). The normalized kv latent + rope'd k_pe are AllGathered
(shared by every head); the per-head q vectors are redistributed with two
AllToAlls (one per head of each core's head pair) so attention runs fully
head-local: core c owns heads 2c, 2c+1 over the full sequence. kv_b expands
kn/v from the gathered latent per head; o_proj is input-split on heads and
the partial products are summed on the host (the unshard step).

All matmuls run in bf16 (1 PE cycle/row regardless of free-dim size, half
the DMA/communication bytes of fp32; final accuracy ~4e-3 vs the 2e-2
gate). Softmax runs over the key (partition) axis: exp on the scalar
engine, denominator via a ones-column matmul, broadcast of per-column
scalars via a K=1 matmul. RoPE rotate-half is a matmul against a constant
signed permutation. o_proj results are DMA'd directly from PSUM.
"""
import numpy as np
import ml_dtypes

import concourse.bass as bass
import concourse.tile as tile
from concourse import bacc, mybir
from concourse.bass_utils import run_bass_kernel_spmd

F32 = mybir.dt.float32
BF16 = mybir.dt.bfloat16
NPBF = ml_dtypes.bfloat16

HID = 2048
S = 2048
H = 16
QL = 1536
KVL = 512
NOPE = 128
RP = 64
VD = 128
QD = NOPE + RP              # 192
SCALE = QD ** -0.5
EPS = 1e-6
ROPE_THETA = 10000.0

NC = 8
HPC = 2                     # heads per core
SSH = S // NC               # 256-seq shard
KT = HID // 128             # 16
QLT = QL // 128             # 12
CT = KVL // 128             # 4
SB = 512                    # attention query block
NSB = S // SB               # 4
NEG = -30000.0

_CACHE = {}
LAST_RESULT = None


def _build_program():
    nc = bacc.Bacc("TRN2", target_bir_lowering=False, debug=False,
                   num_devices=NC)
    d_xt = nc.dram_tensor("xt16", [128, KT, SSH], BF16, kind="ExternalInput").ap()
    d_wqa = nc.dram_tensor("wqa16", [128, KT, QL], BF16, kind="ExternalInput").ap()
    d_wkva = nc.dram_tensor("wkva16", [128, KT, KVL + RP], BF16, kind="ExternalInput").ap()
    d_wqb = nc.dram_tensor("wqb16", [128, QLT, H * QD], BF16, kind="ExternalInput").ap()
    d_wk = nc.dram_tensor("wk16", [128, CT, HPC * NOPE], BF16, kind="ExternalInput").ap()
    d_wv = nc.dram_tensor("wv16", [128, CT, HPC * VD], BF16, kind="ExternalInput").ap()
    d_wo = nc.dram_tensor("wo16", [128, HPC, HID], BF16, kind="ExternalInput").ap()
    d_cos = nc.dram_tensor("cosd", [128, SSH], BF16, kind="ExternalInput").ap()
    d_sin = nc.dram_tensor("sind", [128, SSH], BF16, kind="ExternalInput").ap()
    d_msk = nc.dram_tensor("maskadd", [128, 4, SB], F32, kind="ExternalInput").ap()
    d_rotq = nc.dram_tensor("rotq16", [128, 128], BF16, kind="ExternalInput").ap()
    d_out = nc.dram_tensor("out", [S, HID], BF16, kind="ExternalOutput").ap()

    with tile.TileContext(nc) as tc:
        _mla(tc, d_xt, d_wqa, d_wkva, d_wqb, d_wk, d_wv, d_wo, d_cos, d_sin,
             d_msk, d_rotq, d_out)
    nc.compile()
    return nc


def _mla(tc, d_xt, d_wqa, d_wkva, d_wqb, d_wk, d_wv, d_wo, d_cos, d_sin,
         d_msk, d_rotq, d_out):
    nc = tc.nc
    Exp = mybir.ActivationFunctionType.Exp
    Sqrt = mybir.ActivationFunctionType.Sqrt
    groups = [list(range(NC))]

    with nc.allow_low_precision(reason="bf16 pipeline"), \
         tc.tile_pool(name="pdram", bufs=1, space="DRAM") as pdram, \
         tc.tile_pool(name="pconst", bufs=1) as pc, \
         tc.tile_pool(name="pglob", bufs=1) as pg:
        # ---- DRAM bounce buffers for collectives ----
        ag_in = pdram.tile([KVL + RP, SSH], BF16)
        ag_out = pdram.tile([NC, KVL + RP, SSH], BF16)
        aa_in = [pdram.tile([NC, QD, SSH], BF16, name=f"aain{i}") for i in range(HPC)]
        aa_out = [pdram.tile([NC, QD, SSH], BF16, name=f"aaout{i}") for i in range(HPC)]

        # ---- small constants ----
        ones_c = pc.tile([128, 1], BF16)
        nc.vector.memset(ones_c, 1.0)
        ones_r = pc.tile([1, 128], BF16)
        nc.vector.memset(ones_r, 1.0)
        eps1 = pc.tile([1, 1], F32)
        nc.vector.memset(eps1, EPS)
        rotq = pc.tile([128, 128], BF16)
        nc.sync.dma_start(out=rotq, in_=d_rotq)
        cosd = pc.tile([128, SSH], BF16)
        nc.sync.dma_start(out=cosd, in_=d_cos)
        sind = pc.tile([128, SSH], BF16)
        nc.sync.dma_start(out=sind, in_=d_sin)

        # =============== stage A: shard projections ===============
        with tc.tile_pool(name="pw", bufs=1) as pw, \
             tc.tile_pool(name="pA", bufs=1) as pa, \
             tc.tile_pool(name="pAq", bufs=3) as paq, \
             tc.tile_pool(name="pAs", bufs=3) as pas, \
             tc.tile_pool(name="ppA", bufs=3, space="PSUM") as ppa, \
             tc.tile_pool(name="ppSt", bufs=2, space="PSUM") as ppst, \
             tc.tile_pool(name="ppM", bufs=2, space="PSUM") as ppm:
            # ---- stage-A weights: per-k-slice tiles so matmuls start early ----
            xt = pw.tile([128, KT, SSH], BF16)
            # Early (eager) loads on the SP HW queue: only what the first
            # ~20us of compute needs. Everything else is loaded via gpsimd
            # SWDGE triggers placed AFTER the AllGather in program order, so
            # those transfers enter the global DMA FIFO behind the
            # collective staging instead of ahead of it.
            wkva = pw.tile([128, KT, KVL + RP], BF16)
            nc.sync.dma_start(out=xt, in_=d_xt)
            for half in range(2):
                hk = slice(half * KT // 2, (half + 1) * KT // 2)
                nc.sync.dma_start(out=wkva[:, hk, :], in_=d_wkva[:, hk, :])
            wqa = pw.tile([128, KT, QL], BF16)
            # gate wqa-colA behind wkva via a write-after-read dep: the
            # reader consumes both the colA region and the wkva tail, so the
            # colA DMA (a writer of that region) must wait for wkva.
            gate = pc.tile([1, 2], BF16, name="gate")
            nc.vector.tensor_tensor(gate[0:1, 0:1], wqa[0:1, 0, 0:1],
                                    wkva[0:1, KT - 1, 0:1], mybir.AluOpType.mult)
            nc.scalar.dma_start(out=wqa[:, :, 0:QL // 2], in_=d_wqa[:, :, 0:QL // 2])
            wqb = pw.tile([128, QLT, H * QD], BF16)
            wk = pg.tile([128, CT, HPC * NOPE], BF16)
            wv = pg.tile([128, CT, HPC * VD], BF16)
            wo = pg.tile([128, HPC, HID], BF16)
            msk = pg.tile([128, 4, SB], F32)

            # --- kv LoRA-A ---
            ckvu = pa.tile([128, CT, SSH], BF16)
            kpe = pa.tile([RP, SSH], BF16)
            p_st = ppst.tile([1, SSH], F32, tag="st", name="cstat")
            sqc = pa.tile([128, CT, SSH], BF16, name="sqc")
            for m in range(CT + 1):
                mw = 128 if m < CT else RP
                p_a = ppa.tile([128, SSH], F32, tag="a")
                for k in range(KT):
                    nc.tensor.matmul(p_a[:mw, :], wkva[:, k, m * 128:m * 128 + mw],
                                     xt[:, k, :], start=(k == 0), stop=(k == KT - 1))
                if m < CT:
                    nc.vector.tensor_copy(ckvu[:, m, :], p_a[:])
                    nc.vector.tensor_mul(sqc[:, m, :], ckvu[:, m, :], ckvu[:, m, :])
                else:
                    nc.vector.tensor_copy(kpe[:], p_a[:mw, :])
            for m in range(CT):
                nc.tensor.matmul(p_st[:], ones_c[:], sqc[:, m, :],
                                 start=(m == 0), stop=(m == CT - 1))
            rms_c = pa.tile([1, SSH], BF16)
            nc.scalar.activation(rms_c[:], p_st[:], Sqrt, scale=1.0 / KVL,
                                 bias=eps1[:])
            p_bc = ppm.tile([128, SSH], F32, tag="m")
            nc.tensor.matmul(p_bc[:], ones_r[:], rms_c[:], start=True, stop=True)
            invc = pa.tile([128, SSH], BF16)
            nc.vector.reciprocal(invc[:], p_bc[:])
            ckv = pa.tile([128, CT, SSH], BF16)
            for m in range(CT):
                nc.vector.tensor_mul(ckv[:, m, :], ckvu[:, m, :], invc[:])
            nc.gpsimd.dma_start(
                out=ag_in[0:KVL, :].rearrange("(t p) c -> p t c", p=128),
                in_=ckv[:])
            # --- k_pe rope (scale folded: none needed in bf16) ---
            p_rk = ppm.tile([128, SSH], F32, tag="m", name="rotk")
            nc.tensor.matmul(p_rk[:RP, :], rotq[0:RP, 0:RP], kpe[:],
                             start=True, stop=True)
            rk16 = pas.tile([RP, SSH], BF16, tag="rk")
            nc.vector.tensor_copy(rk16[:], p_rk[:RP, :])
            t1 = pas.tile([RP, SSH], BF16, tag="t1")
            nc.vector.tensor_mul(t1[:], kpe[:], cosd[0:RP, :])
            t2 = pas.tile([RP, SSH], BF16, tag="t2")
            nc.vector.tensor_mul(t2[:], rk16[:], sind[0:RP, :])
            kpd = pa.tile([RP, SSH], BF16)
            nc.vector.tensor_add(kpd[:], t1[:], t2[:])
            nc.gpsimd.dma_start(out=ag_in[KVL:KVL + RP, :], in_=kpd[:])
            # --- collective #1: AllGather latent+kpe ---
            nc.gpsimd.collective_compute(
                "AllGather", mybir.AluOpType.bypass, replica_groups=groups,
                ins=[ag_in[:].opt()], outs=[ag_out[:].opt()])
            # deferred bulk weight loads, chained with write-after-read
            # gates so each transfer enters the exclusive DMA FIFO after the
            # AllGather staging and after the previous weight transfer.
            agmark = pc.tile([1, 2], BF16, name="agmark")
            nc.gpsimd.dma_start(out=agmark[0:1, 0:2],
                                in_=ag_in[KVL + RP - 1:KVL + RP, 0:2])
            Mul = mybir.AluOpType.mult

            def gate_read(region, token):
                g = pas.tile([1, 1], BF16, tag="g8")
                nc.vector.tensor_tensor(g[:], region, token, Mul)

            gate_read(wqa[0:1, 0, QL - 1:QL], ckv[0:1, CT - 1, 0:1])
            nc.gpsimd.dma_start(out=wqa[:, :, QL // 2:QL],
                                in_=d_wqa[:, :, QL // 2:QL])
            gate_read(wqb[0:1, 0, H * QD - 1:H * QD], agmark[0:1, 0:1])
            nc.gpsimd.dma_start(out=wqb[:, :, H * NOPE:H * QD],
                                in_=d_wqb[:, :, H * NOPE:H * QD])
            for q4 in range(4):
                qs = slice(q4 * H * NOPE // 4, (q4 + 1) * H * NOPE // 4)
                gate_read(wqb[0:1, 0, q4 * H * NOPE // 4:q4 * H * NOPE // 4 + 1],
                          wqb[0:1, 0, H * QD - 1:H * QD])
                nc.gpsimd.dma_start(out=wqb[:, :, qs], in_=d_wqb[:, :, qs])
            for wtile, dsrc in ((wk, d_wk), (wv, d_wv), (wo, d_wo), (msk, d_msk)):
                gate_read(wtile[0:1, 0, 0:1], wqb[0:1, 0, H * NOPE - 1:H * NOPE])
                nc.gpsimd.dma_start(out=wtile, in_=dsrc)

            # --- q LoRA-A ---
            qlu = pa.tile([128, QLT, SSH], BF16)
            p_qst = ppst.tile([1, SSH], F32, tag="st", name="qstat")
            sqq = pa.tile([128, QLT, SSH], BF16, name="sqq")
            for k in range(QLT):
                p_a = ppa.tile([128, SSH], F32, tag="a")
                for kk in range(KT):
                    nc.tensor.matmul(p_a[:], wqa[:, kk, k * 128:(k + 1) * 128],
                                     xt[:, kk, :], start=(kk == 0), stop=(kk == KT - 1))
                nc.vector.tensor_copy(qlu[:, k, :], p_a[:])
                nc.vector.tensor_mul(sqq[:, k, :], qlu[:, k, :], qlu[:, k, :])
            for k in range(QLT):
                nc.tensor.matmul(p_qst[:], ones_c[:], sqq[:, k, :],
                                 start=(k == 0), stop=(k == QLT - 1))
            rms_q = pa.tile([1, SSH], BF16)
            nc.scalar.activation(rms_q[:], p_qst[:], Sqrt, scale=1.0 / QL,
                                 bias=eps1[:])
            p_bq = ppm.tile([128, SSH], F32, tag="m")
            nc.tensor.matmul(p_bq[:], ones_r[:], rms_q[:], start=True, stop=True)
            invq = pa.tile([128, SSH], F32)
            nc.vector.reciprocal(invq[:], p_bq[:])

            # --- q_b for all heads: rope tiles (16..23) first so the rope
            # chain and the AllToAll staging DMAs start as early as possible;
            # nope tiles follow in parity order (A2A#1's inputs first).
            q16 = pa.tile([128, H + NC, SSH], BF16, name="q16")

            def qb_group(mt):
                p_q = ppa.tile([128, SSH], F32, tag="a")
                for k in range(QLT):
                    nc.tensor.matmul(p_q[:], wqb[:, k, mt * 128:(mt + 1) * 128],
                                     qlu[:, k, :], start=(k == 0), stop=(k == QLT - 1))
                nc.vector.tensor_mul(q16[:, mt, :], p_q[:], invq[:])

            for mt in range(H, H + NC):
                qb_group(mt)
            # rope rotate-half + cos/sin (inputs ready; no PE stalls)
            for d in range(NC):
                p_rq = ppm.tile([128, SSH], F32, tag="m")
                nc.tensor.matmul(p_rq[:], rotq[:], q16[:, H + d, :],
                                 start=True, stop=True)
                rq16 = pas.tile([128, SSH], BF16, tag="rk", name="rq16")
                nc.vector.tensor_copy(rq16[:], p_rq[:])
                t1q = pas.tile([128, SSH], BF16, tag="t1")
                nc.vector.tensor_mul(t1q[:], q16[:, H + d, :], cosd[:])
                t2q = pas.tile([128, SSH], BF16, tag="t2")
                nc.vector.tensor_mul(t2q[:], rq16[:], sind[:])
                nc.vector.tensor_add(q16[:, H + d, :], t1q[:], t2q[:])
            for mt in range(0, H, 2):
                qb_group(mt)
            nc.gpsimd.dma_start(
                out=aa_in[0][:, 0:NOPE, :].rearrange("j p c -> p j c"),
                in_=q16[:, 0:H:2, :].rearrange("p j c -> p j c"))
            nc.gpsimd.dma_start(
                out=aa_in[0][:, NOPE:QD, :].rearrange("j p c -> p j c"),
                in_=q16[0:RP, H:H + NC, :])
            nc.gpsimd.collective_compute(
                "AllToAll", mybir.AluOpType.bypass, replica_groups=groups,
                ins=[aa_in[0][:].opt()], outs=[aa_out[0][:].opt()])
            for mt in range(1, H, 2):
                qb_group(mt)
            nc.gpsimd.dma_start(
                out=aa_in[1][:, 0:NOPE, :].rearrange("j p c -> p j c"),
                in_=q16[:, 1:H:2, :].rearrange("p j c -> p j c"))
            nc.gpsimd.dma_start(
                out=aa_in[1][:, NOPE:QD, :].rearrange("j p c -> p j c"),
                in_=q16[RP:128, H:H + NC, :])
            nc.gpsimd.collective_compute(
                "AllToAll", mybir.AluOpType.bypass, replica_groups=groups,
                ins=[aa_in[1][:].opt()], outs=[aa_out[1][:].opt()])

        # =============== stage B: head-local attention ===============
        with tc.tile_pool(name="pB", bufs=1) as pb, \
             tc.tile_pool(name="pBe", bufs=6) as pbe, \
             tc.tile_pool(name="pBo", bufs=3) as pbo, \
             tc.tile_pool(name="pBn", bufs=2) as pbn, \
             tc.tile_pool(name="ppS", bufs=3, space="PSUM") as pps, \
             tc.tile_pool(name="ppO", bufs=2, space="PSUM") as ppo, \
             tc.tile_pool(name="ppD", bufs=1, space="PSUM") as ppd, \
             tc.tile_pool(name="ppC", bufs=2, space="PSUM") as ppc:
            ckvg = pb.tile([128, CT, S], BF16)
            for t in range(CT):
                nc.gpsimd.dma_start(
                    out=ckvg[:, t, :].rearrange("p (j c) -> p j c", j=NC),
                    in_=ag_out[:, t * 128:(t + 1) * 128, :].rearrange(
                        "j p c -> p j c"))
            kpdg = pb.tile([RP, S], BF16)
            nc.gpsimd.dma_start(out=kpdg[:].rearrange("p (j c) -> p j c", j=NC),
                              in_=ag_out[:, KVL:KVL + RP, :].rearrange(
                                  "j p c -> p j c"))
            qt = [pb.tile([128, S], BF16, name=f"qt{h}") for h in range(HPC)]
            qpt = [pb.tile([RP, S], BF16, name=f"qpt{h}") for h in range(HPC)]

            def unpack_q(h):
                nc.gpsimd.dma_start(
                    out=qt[h][:].rearrange("p (j c) -> p j c", j=NC),
                    in_=aa_out[h][:, 0:NOPE, :].rearrange("j p c -> p j c"))
                nc.gpsimd.dma_start(
                    out=qpt[h][:].rearrange("p (j c) -> p j c", j=NC),
                    in_=aa_out[h][:, NOPE:QD, :].rearrange("j p c -> p j c"))

            # --- kv_b: kn per head, v (both heads) keys-on-partitions ---
            kn = [pb.tile([128, S], BF16, name=f"kn{h}") for h in range(HPC)]
            for h in range(HPC):
                for cb in range(S // SSH):
                    p_k = ppc.tile([128, SSH], F32, tag="c")
                    for t in range(CT):
                        nc.tensor.matmul(p_k[:], wk[:, t, h * NOPE:(h + 1) * NOPE],
                                         ckvg[:, t, cb * SSH:(cb + 1) * SSH],
                                         start=(t == 0), stop=(t == CT - 1))
                    nc.any.tensor_copy(kn[h][:, cb * SSH:(cb + 1) * SSH], p_k[:])
            vst = pb.tile([128, S // 128, HPC * VD], BF16)
            for sb in range(S // 128):
                p_v = ppc.tile([128, HPC * VD], F32, tag="c")
                for t in range(CT):
                    nc.tensor.matmul(p_v[:], ckvg[:, t, sb * 128:(sb + 1) * 128],
                                     wv[:, t, :], start=(t == 0), stop=(t == CT - 1))
                nc.any.tensor_copy(vst[:, sb, :], p_v[:])

            # --- attention: heads outer (matches AllToAll arrival).
            # Software-pipelined: AV/den for ik are issued after the scores
            # of ik+1 so the PE never stalls on the exp; the per-(qb,h)
            # normalization finisher is deferred into the next iteration's
            # matmul stream.
            ao = pb.tile([128, NSB, HPC, SB], BF16)
            pending = None

            def finisher(fin):
                h, qb, p_o, p_d = fin
                den = pbn.tile([1, SB], BF16, tag="den")
                nc.vector.tensor_copy(den[:], p_d[:])
                p_b = ppc.tile([128, SB], F32, tag="c", name="bcast")
                nc.tensor.matmul(p_b[:], ones_r[:], den[:], start=True, stop=True)
                rec = pbn.tile([128, SB], F32, tag="rec")
                nc.vector.reciprocal(rec[:], p_b[:])
                nc.vector.tensor_mul(ao[:, qb, h, :], p_o[:], rec[:])

            def oproj(qb):
                for st in range(SB // 128):
                    sc = slice(qb * SB + st * 128, qb * SB + (st + 1) * 128)
                    ot = pbo.tile([128, HID], BF16, tag="ot")
                    for nb in range(HID // SB):
                        ncols = bass.ts(nb, SB)
                        p_c = ppc.tile([128, SB], F32, tag="c")
                        for hh in range(HPC):
                            nc.tensor.matmul(
                                p_c[:], ao[:, qb, hh, st * 128:(st + 1) * 128],
                                wo[:, hh, ncols],
                                start=(hh == 0), stop=(hh == HPC - 1))
                        if (st + nb) % 2 == 0:
                            nc.vector.tensor_copy(ot[:, ncols], p_c[:])
                        else:
                            nc.scalar.activation(
                                ot[:, ncols], p_c[:],
                                mybir.ActivationFunctionType.Copy)
                    nc.sync.dma_start(out=d_out[sc, :], in_=ot[:])

            for h in range(HPC):
                unpack_q(h)
                for qb in range(NSB):
                    qcols = bass.ts(qb, SB)
                    nk = 4 * (qb + 1)
                    p_o = ppo.tile([128, SB], F32, tag="o")
                    p_d = ppd.tile([1, SB], F32, tag="d")
                    prev_e = None
                    for ik in range(nk):
                        kc = slice(ik * 128, (ik + 1) * 128)
                        p_s = pps.tile([128, SB], F32, tag="s")
                        nc.tensor.matmul(p_s[:], kn[h][:, kc], qt[h][:, qcols],
                                         start=True, stop=False)
                        nc.tensor.matmul(p_s[:], kpdg[:, kc], qpt[h][:, qcols],
                                         start=False, stop=True)
                        if ik == 1 and pending is not None:
                            fin, oqb = pending
                            finisher(fin)
                            pending = None
                            if oqb is not None:
                                oproj(oqb)
                        if prev_e is not None:
                            pik = ik - 1
                            nc.tensor.matmul(p_o[:], vst[:, pik, h * VD:(h + 1) * VD],
                                             prev_e[:], start=(pik == 0),
                                             stop=(pik == nk - 1))
                            nc.tensor.matmul(p_d[:], ones_c[:], prev_e[:],
                                             start=(pik == 0), stop=(pik == nk - 1))
                        r = ik - 4 * qb
                        if r >= 0:
                            nc.any.tensor_add(p_s[:], p_s[:], msk[:, r, :])
                        e = pbe.tile([128, SB], BF16, tag="e")
                        nc.scalar.activation(e[:], p_s[:], Exp, scale=SCALE)
                        prev_e = e
                    nc.tensor.matmul(p_o[:], vst[:, nk - 1, h * VD:(h + 1) * VD],
                                     prev_e[:], start=(nk == 1), stop=True)
                    nc.tensor.matmul(p_d[:], ones_c[:], prev_e[:],
                                     start=(nk == 1), stop=True)
                    pending = ((h, qb, p_o, p_d),
                               qb if h == HPC - 1 else None)
            fin, oqb = pending
            finisher(fin)
            if oqb is not None:
                oproj(oqb)


def _host_constants():
    inv_freq = 1.0 / (ROPE_THETA ** (np.arange(0, RP, 2, dtype=np.float32) / RP))
    t = np.arange(S, dtype=np.float32)
    freqs = np.outer(t, inv_freq)
    emb = np.concatenate([freqs, freqs], -1)          # [S, 64]
    cos, sin = np.cos(emb), np.sin(emb)
    cosd = np.concatenate([cos.T, cos.T], 0).astype(np.float32)   # [128, S]
    sind = np.concatenate([sin.T, sin.T], 0).astype(np.float32)

    # additive causal mask for diagonal 128-key blocks: [128, 4, 512]
    mska = np.zeros((128, 4, SB), np.float32)
    for r in range(4):
        for p in range(128):
            mska[p, r, :p + 128 * r] = NEG
    # rotate-half as matmul lhsT: same as baseline
    Q = np.zeros((RP, RP), np.float32)
    for i in range(RP // 2):
        Q[i, i + RP // 2] = -1.0
        Q[i + RP // 2, i] = 1.0
    P = np.zeros((128, 128), np.float32)
    P[:RP, :RP] = Q
    P[RP:, RP:] = Q
    rotq = P.T.copy()
    return cosd, sind, mska, rotq


def _tile3(w, kt):
    """[kt*128, F] -> [128, kt, F]"""
    return np.ascontiguousarray(
        w.reshape(kt, 128, w.shape[1]).transpose(1, 0, 2))


def kernel(hidden_states, w_q_a, q_a_weight, w_q_b, w_kv_a, kv_a_weight,
           w_kv_b, w_o):
    global LAST_RESULT
    if "nc" not in _CACHE:
        _CACHE["nc"] = _build_program()
    nc = _CACHE["nc"]

    x = np.asarray(hidden_states, np.float32)[0]       # [S, 2048]
    xt = np.ascontiguousarray(x.T)                     # [2048, S]
    wqa_t = np.asarray(w_q_a, np.float32).T            # [HID, QL]
    wkva_t = np.asarray(w_kv_a, np.float32).T          # [HID, KVL+RP]
    wqb_eff = np.asarray(w_q_b, np.float32) * np.asarray(q_a_weight, np.float32)[None, :]
    wkvb_eff = np.asarray(w_kv_b, np.float32) * np.asarray(kv_a_weight, np.float32)[None, :]
    won = np.asarray(w_o, np.float32)                  # [HID, H*VD]

    # q_b output feature permutation: nope head-major, then rope packed 2/tile
    perm = np.zeros(H * QD, np.int64)
    for h in range(H):
        perm[h * NOPE:(h + 1) * NOPE] = h * QD + np.arange(NOPE)
    base = H * NOPE
    for d in range(NC):
        for j in range(HPC):
            hh = 2 * d + j
            perm[base + d * 128 + j * RP: base + d * 128 + (j + 1) * RP] = \
                hh * QD + NOPE + np.arange(RP)
    wqb_p = wqb_eff[perm, :]                           # [3072, QL]

    cosd, sind, mska, rotq = _host_constants()

    wqa16 = _tile3(wqa_t, KT).astype(NPBF)
    wkva16 = _tile3(wkva_t, KT).astype(NPBF)
    wqb16 = _tile3(np.ascontiguousarray(wqb_p.T), QLT).astype(NPBF)
    rotq16 = rotq.astype(NPBF)

    shared = {"wqa16": wqa16, "wkva16": wkva16, "wqb16": wqb16,
              "maskadd": mska, "rotq16": rotq16}

    in_maps = []
    for c in range(NC):
        h0, h1 = HPC * c, HPC * c + 1
        wk_t = np.concatenate(
            [wkvb_eff[h * (NOPE + VD):h * (NOPE + VD) + NOPE] for h in (h0, h1)],
            0).T                                        # [KVL, 256]
        wv_t = np.concatenate(
            [wkvb_eff[h * (NOPE + VD) + NOPE:(h + 1) * (NOPE + VD)] for h in (h0, h1)],
            0).T                                        # [KVL, 256]
        wo_t = np.stack(
            [np.ascontiguousarray(won[:, h * VD:(h + 1) * VD].T) for h in (h0, h1)],
            1)                                          # [128, 2, HID]
        cols = slice(c * SSH, (c + 1) * SSH)
        im = dict(shared)
        im.update({
            "xt16": _tile3(np.ascontiguousarray(xt[:, cols]), KT).astype(NPBF),
            "wk16": _tile3(wk_t, CT).astype(NPBF),
            "wv16": _tile3(wv_t, CT).astype(NPBF),
            "wo16": np.ascontiguousarray(wo_t).astype(NPBF),
            "cosd": np.ascontiguousarray(cosd[:, cols]).astype(NPBF),
            "sind": np.ascontiguousarray(sind[:, cols]).astype(NPBF),
        })
        in_maps.append(im)

    res = run_bass_kernel_spmd(nc, in_maps, list(range(NC)))
    LAST_RESULT = res
    out = np.zeros((S, HID), np.float32)
    for c in range(NC):
        out += np.asarray(res.results[c]["out"]).astype(np.float32)
    return out.reshape(1, S, HID)


# revision 30
# speedup vs baseline: 1.8289x; 1.0135x over previous
"""MLA (DeepSeek-style multi-head latent attention) forward on 8 trn2 cores.

Layout v2: sequence-sharded LoRA-A + device collectives + bf16 matmuls.

Each core computes the LoRA-A projections (q_latent, compressed-kv latent,
k_pe) only for its 256-column sequence shard (8x less replicated work than
pure head-TP). The normalized kv latent + rope'd k_pe are AllGathered
(shared by every head); the per-head q vectors are redistributed with two
AllToAlls (one per head of each core's head pair) so attention runs fully
head-local: core c owns heads 2c, 2c+1 over the full sequence. kv_b expands
kn/v from the gathered latent per head; o_proj is input-split on heads and
the partial products are summed on the host (the unshard step).

All matmuls run in bf16 (1 PE cycle/row regardless of free-dim size, half
the DMA/communication bytes of fp32; final accuracy ~4e-3 vs the 2e-2
gate). Softmax runs over the key (partition) axis: exp on the scalar
engine, denominator via a ones-column matmul, broadcast of per-column
scalars via a K=1 matmul. RoPE rotate-half is a matmul against a constant
signed permutation. o_proj results are DMA'd directly from PSUM.
"""
import numpy as np
import ml_dtypes

import concourse.bass as bass
import concourse.tile as tile
from concourse import bacc, mybir
from concourse.bass_utils import run_bass_kernel_spmd

F32 = mybir.dt.float32
BF16 = mybir.dt.bfloat16
NPBF = ml_dtypes.bfloat16

HID = 2048
S = 2048
H = 16
QL = 1536
KVL = 512
NOPE = 128
RP = 64
VD = 128
QD = NOPE + RP              # 192
SCALE = QD ** -0.5
EPS = 1e-6
ROPE_THETA = 10000.0

NC = 8
HPC = 2                     # heads per core
SSH = S // NC               # 256-seq shard
KT = HID // 128             # 16
QLT = QL // 128             # 12
CT = KVL // 128             # 4
SB = 512                    # attention query block
NSB = S // SB               # 4
NEG = -30000.0

_CACHE = {}
LAST_RESULT = None


def _build_program():
    nc = bacc.Bacc("TRN2", target_bir_lowering=False, debug=False,
                   num_devices=NC)
    d_xt = nc.dram_tensor("xt16", [128, KT, SSH], BF16, kind="ExternalInput").ap()
    d_wqa = nc.dram_tensor("wqa16", [128, KT, QL], BF16, kind="ExternalInput").ap()
    d_wkva = nc.dram_tensor("wkva16", [128, KT, KVL + RP], BF16, kind="ExternalInput").ap()
    d_wqb = nc.dram_tensor("wqb16", [128, QLT, H * QD], BF16, kind="ExternalInput").ap()
    d_wk = nc.dram_tensor("wk16", [128, CT, HPC * NOPE], BF16, kind="ExternalInput").ap()
    d_wv = nc.dram_tensor("wv16", [128, CT, HPC * VD], BF16, kind="ExternalInput").ap()
    d_wo = nc.dram_tensor("wo16", [128, HPC, HID], BF16, kind="ExternalInput").ap()
    d_cos = nc.dram_tensor("cosd", [128, SSH], BF16, kind="ExternalInput").ap()
    d_sin = nc.dram_tensor("sind", [128, SSH], BF16, kind="ExternalInput").ap()
    d_msk = nc.dram_tensor("maskadd", [128, 4, SB], F32, kind="ExternalInput").ap()
    d_rotq = nc.dram_tensor("rotq16", [128, 128], BF16, kind="ExternalInput").ap()
    d_out = nc.dram_tensor("out", [S, HID], BF16, kind="ExternalOutput").ap()

    with tile.TileContext(nc) as tc:
        _mla(tc, d_xt, d_wqa, d_wkva, d_wqb, d_wk, d_wv, d_wo, d_cos, d_sin,
             d_msk, d_rotq, d_out)
    nc.compile()
    return nc


def _mla(tc, d_xt, d_wqa, d_wkva, d_wqb, d_wk, d_wv, d_wo, d_cos, d_sin,
         d_msk, d_rotq, d_out):
    nc = tc.nc
    Exp = mybir.ActivationFunctionType.Exp
    Sqrt = mybir.ActivationFunctionType.Sqrt
    groups = [list(range(NC))]

    with nc.allow_low_precision(reason="bf16 pipeline"), \
         tc.tile_pool(name="pdram", bufs=1, space="DRAM") as pdram, \
         tc.tile_pool(name="pconst", bufs=1) as pc, \
         tc.tile_pool(name="pglob", bufs=1) as pg:
        # ---- DRAM bounce buffers for collectives ----
        ag_in = pdram.tile([KVL + RP, SSH], BF16)
        ag_out = pdram.tile([NC, KVL + RP, SSH], BF16)
        aa_in = [pdram.tile([NC, QD, SSH], BF16, name=f"aain{i}") for i in range(HPC)]
        aa_out = [pdram.tile([NC, QD, SSH], BF16, name=f"aaout{i}") for i in range(HPC)]

        # ---- small constants ----
        ones_c = pc.tile([128, 1], BF16)
        nc.vector.memset(ones_c, 1.0)
        ones_r = pc.tile([1, 128], BF16)
        nc.vector.memset(ones_r, 1.0)
        eps1 = pc.tile([1, 1], F32)
        nc.vector.memset(eps1, EPS)
        rotq = pc.tile([128, 128], BF16)
        nc.sync.dma_start(out=rotq, in_=d_rotq)
        cosd = pc.tile([128, SSH], BF16)
        nc.sync.dma_start(out=cosd, in_=d_cos)
        sind = pc.tile([128, SSH], BF16)
        nc.sync.dma_start(out=sind, in_=d_sin)

        # =============== stage A: shard projections ===============
        with tc.tile_pool(name="pw", bufs=1) as pw, \
             tc.tile_pool(name="pA", bufs=1) as pa, \
             tc.tile_pool(name="pAq", bufs=3) as paq, \
             tc.tile_pool(name="pAs", bufs=3) as pas, \
             tc.tile_pool(name="ppA", bufs=3, space="PSUM") as ppa, \
             tc.tile_pool(name="ppSt", bufs=2, space="PSUM") as ppst, \
             tc.tile_pool(name="ppM", bufs=2, space="PSUM") as ppm:
            # ---- stage-A weights: per-k-slice tiles so matmuls start early ----
            xt = pw.tile([128, KT, SSH], BF16)
            # Early (eager) loads on the SP HW queue: only what the first
            # ~20us of compute needs. Everything else is loaded via gpsimd
            # SWDGE triggers placed AFTER the AllGather in program order, so
            # those transfers enter the global DMA FIFO behind the
            # collective staging instead of ahead of it.
            wkva = pw.tile([128, KT, KVL + RP], BF16)
            nc.sync.dma_start(out=xt, in_=d_xt)
            for half in range(2):
                hk = slice(half * KT // 2, (half + 1) * KT // 2)
                nc.sync.dma_start(out=wkva[:, hk, :], in_=d_wkva[:, hk, :])
            wqa = pw.tile([128, KT, QL], BF16)
            # gate wqa-colA behind wkva via a write-after-read dep: the
            # reader consumes both the colA region and the wkva tail, so the
            # colA DMA (a writer of that region) must wait for wkva.
            gate = pc.tile([1, 2], BF16, name="gate")
            nc.vector.tensor_tensor(gate[0:1, 0:1], wqa[0:1, 0, 0:1],
                                    wkva[0:1, KT - 1, 0:1], mybir.AluOpType.mult)
            nc.scalar.dma_start(out=wqa[:, :, 0:QL // 2], in_=d_wqa[:, :, 0:QL // 2])
            wqb = pw.tile([128, QLT, H * QD], BF16)
            wk = pg.tile([128, CT, HPC * NOPE], BF16)
            wv = pg.tile([128, CT, HPC * VD], BF16)
            wo = pg.tile([128, HPC, HID], BF16)
            msk = pg.tile([128, 4, SB], F32)

            # --- kv LoRA-A ---
            ckvu = pa.tile([128, CT, SSH], BF16)
            kpe = pa.tile([RP, SSH], BF16)
            p_st = ppst.tile([1, SSH], F32, tag="st", name="cstat")
            sqc = pa.tile([128, CT, SSH], BF16, name="sqc")
            for m in range(CT + 1):
                mw = 128 if m < CT else RP
                p_a = ppa.tile([128, SSH], F32, tag="a")
                for k in range(KT):
                    nc.tensor.matmul(p_a[:mw, :], wkva[:, k, m * 128:m * 128 + mw],
                                     xt[:, k, :], start=(k == 0), stop=(k == KT - 1))
                if m < CT:
                    nc.vector.tensor_copy(ckvu[:, m, :], p_a[:])
                    nc.vector.tensor_mul(sqc[:, m, :], ckvu[:, m, :], ckvu[:, m, :])
                else:
                    nc.vector.tensor_copy(kpe[:], p_a[:mw, :])
            for m in range(CT):
                nc.tensor.matmul(p_st[:], ones_c[:], sqc[:, m, :],
                                 start=(m == 0), stop=(m == CT - 1))
            rms_c = pa.tile([1, SSH], BF16)
            nc.scalar.activation(rms_c[:], p_st[:], Sqrt, scale=1.0 / KVL,
                                 bias=eps1[:])
            p_bc = ppm.tile([128, SSH], F32, tag="m")
            nc.tensor.matmul(p_bc[:], ones_r[:], rms_c[:], start=True, stop=True)
            invc = pa.tile([128, SSH], BF16)
            nc.vector.reciprocal(invc[:], p_bc[:])
            ckv = pa.tile([128, CT, SSH], BF16)
            for m in range(CT):
                nc.vector.tensor_mul(ckv[:, m, :], ckvu[:, m, :], invc[:])
            nc.gpsimd.dma_start(
                out=ag_in[0:KVL, :].rearrange("(t p) c -> p t c", p=128),
                in_=ckv[:])
            # --- k_pe rope (scale folded: none needed in bf16) ---
            p_rk = ppm.tile([128, SSH], F32, tag="m", name="rotk")
            nc.tensor.matmul(p_rk[:RP, :], rotq[0:RP, 0:RP], kpe[:],
                             start=True, stop=True)
            rk16 = pas.tile([RP, SSH], BF16, tag="rk")
            nc.vector.tensor_copy(rk16[:], p_rk[:RP, :])
            t1 = pas.tile([RP, SSH], BF16, tag="t1")
            nc.vector.tensor_mul(t1[:], kpe[:], cosd[0:RP, :])
            t2 = pas.tile([RP, SSH], BF16, tag="t2")
            nc.vector.tensor_mul(t2[:], rk16[:], sind[0:RP, :])
            kpd = pa.tile([RP, SSH], BF16)
            nc.vector.tensor_add(kpd[:], t1[:], t2[:])
            nc.gpsimd.dma_start(out=ag_in[KVL:KVL + RP, :], in_=kpd[:])
            # --- collective #1: AllGather latent+kpe ---
            nc.gpsimd.collective_compute(
                "AllGather", mybir.AluOpType.bypass, replica_groups=groups,
                ins=[ag_in[:].opt()], outs=[ag_out[:].opt()])
            # deferred bulk weight loads, chained with write-after-read
            # gates so each transfer enters the exclusive DMA FIFO after the
            # AllGather staging and after the previous weight transfer.
            agmark = pc.tile([1, 2], BF16, name="agmark")
            nc.gpsimd.dma_start(out=agmark[0:1, 0:2],
                                in_=ag_in[KVL + RP - 1:KVL + RP, 0:2])
            Mul = mybir.AluOpType.mult

            def gate_read(region, token):
                g = pas.tile([1, 1], BF16, tag="g8")
                nc.vector.tensor_tensor(g[:], region, token, Mul)

            gate_read(wqa[0:1, 0, QL - 1:QL], ckv[0:1, CT - 1, 0:1])
            nc.gpsimd.dma_start(out=wqa[:, :, QL // 2:QL],
                                in_=d_wqa[:, :, QL // 2:QL])
            gate_read(wqb[0:1, 0, H * QD - 1:H * QD], agmark[0:1, 0:1])
            nc.gpsimd.dma_start(out=wqb[:, :, H * NOPE:H * QD],
                                in_=d_wqb[:, :, H * NOPE:H * QD])
            for q4 in range(4):
                qs = slice(q4 * H * NOPE // 4, (q4 + 1) * H * NOPE // 4)
                gate_read(wqb[0:1, 0, q4 * H * NOPE // 4:q4 * H * NOPE // 4 + 1],
                          wqb[0:1, 0, H * QD - 1:H * QD])
                nc.gpsimd.dma_start(out=wqb[:, :, qs], in_=d_wqb[:, :, qs])
            for wtile, dsrc in ((wk, d_wk), (wv, d_wv), (wo, d_wo), (msk, d_msk)):
                gate_read(wtile[0:1, 0, 0:1], wqb[0:1, 0, H * NOPE - 1:H * NOPE])
                nc.gpsimd.dma_start(out=wtile, in_=dsrc)

            # --- q LoRA-A ---
            qlu = pa.tile([128, QLT, SSH], BF16)
            p_qst = ppst.tile([1, SSH], F32, tag="st", name="qstat")
            sqq = pa.tile([128, QLT, SSH], BF16, name="sqq")
            for k in range(QLT):
                p_a = ppa.tile([128, SSH], F32, tag="a")
                for kk in range(KT):
                    nc.tensor.matmul(p_a[:], wqa[:, kk, k * 128:(k + 1) * 128],
                                     xt[:, kk, :], start=(kk == 0), stop=(kk == KT - 1))
                nc.vector.tensor_copy(qlu[:, k, :], p_a[:])
                nc.vector.tensor_mul(sqq[:, k, :], qlu[:, k, :], qlu[:, k, :])
            for k in range(QLT):
                nc.tensor.matmul(p_qst[:], ones_c[:], sqq[:, k, :],
                                 start=(k == 0), stop=(k == QLT - 1))
            rms_q = pa.tile([1, SSH], BF16)
            nc.scalar.activation(rms_q[:], p_qst[:], Sqrt, scale=1.0 / QL,
                                 bias=eps1[:])
            p_bq = ppm.tile([128, SSH], F32, tag="m")
            nc.tensor.matmul(p_bq[:], ones_r[:], rms_q[:], start=True, stop=True)
            invq = pa.tile([128, SSH], F32)
            nc.vector.reciprocal(invq[:], p_bq[:])

            # --- q_b for all heads: rope tiles (16..23) first so the rope
            # chain and the AllToAll staging DMAs start as early as possible;
            # nope tiles follow in parity order (A2A#1's inputs first).
            q16 = pa.tile([128, H + NC, SSH], BF16, name="q16")

            def qb_group(mt):
                p_q = ppa.tile([128, SSH], F32, tag="a")
                for k in range(QLT):
                    nc.tensor.matmul(p_q[:], wqb[:, k, mt * 128:(mt + 1) * 128],
                                     qlu[:, k, :], start=(k == 0), stop=(k == QLT - 1))
                nc.vector.tensor_mul(q16[:, mt, :], p_q[:], invq[:])

            for mt in range(H, H + NC):
                qb_group(mt)
            # rope rotate-half + cos/sin (inputs ready; no PE stalls)
            for d in range(NC):
                p_rq = ppm.tile([128, SSH], F32, tag="m")
                nc.tensor.matmul(p_rq[:], rotq[:], q16[:, H + d, :],
                                 start=True, stop=True)
                rq16 = pas.tile([128, SSH], BF16, tag="rk", name="rq16")
                nc.vector.tensor_copy(rq16[:], p_rq[:])
                t1q = pas.tile([128, SSH], BF16, tag="t1")
                nc.vector.tensor_mul(t1q[:], q16[:, H + d, :], cosd[:])
                t2q = pas.tile([128, SSH], BF16, tag="t2")
                nc.vector.tensor_mul(t2q[:], rq16[:], sind[:])
                nc.vector.tensor_add(q16[:, H + d, :], t1q[:], t2q[:])
            for mt in range(0, H, 2):
                qb_group(mt)
            nc.gpsimd.dma_start(
                out=aa_in[0][:, 0:NOPE, :].rearrange("j p c -> p j c"),
                in_=q16[:, 0:H:2, :].rearrange("p j c -> p j c"))
            nc.gpsimd.dma_start(
                out=aa_in[0][:, NOPE:QD, :].rearrange("j p c -> p j c"),
                in_=q16[0:RP, H:H + NC, :])
            nc.gpsimd.collective_compute(
                "AllToAll", mybir.AluOpType.bypass, replica_groups=groups,
                ins=[aa_in[0][:].opt()], outs=[aa_out[0][:].opt()])
            for mt in range(1, H, 2):
                qb_group(mt)
            nc.gpsimd.dma_start(
                out=aa_in[1][:, 0:NOPE, :].rearrange("j p c -> p j c"),
                in_=q16[:, 1:H:2, :].rearrange("p j c -> p j c"))
            nc.gpsimd.dma_start(
                out=aa_in[1][:, NOPE:QD, :].rearrange("j p c -> p j c"),
                in_=q16[RP:128, H:H + NC, :])
            nc.gpsimd.collective_compute(
                "AllToAll", mybir.AluOpType.bypass, replica_groups=groups,
                ins=[aa_in[1][:].opt()], outs=[aa_out[1][:].opt()])

        # =============== stage B: head-local attention ===============
        with tc.tile_pool(name="pB", bufs=1) as pb, \
             tc.tile_pool(name="pBe", bufs=6) as pbe, \
             tc.tile_pool(name="pBo", bufs=3) as pbo, \
             tc.tile_pool(name="pBn", bufs=2) as pbn, \
             tc.tile_pool(name="ppS", bufs=3, space="PSUM") as pps, \
             tc.tile_pool(name="ppO", bufs=2, space="PSUM") as ppo, \
             tc.tile_pool(name="ppD", bufs=1, space="PSUM") as ppd, \
             tc.tile_pool(name="ppC", bufs=2, space="PSUM") as ppc:
            ckvg = pb.tile([128, CT, S], BF16)
            for t in range(CT):
                nc.gpsimd.dma_start(
                    out=ckvg[:, t, :].rearrange("p (j c) -> p j c", j=NC),
                    in_=ag_out[:, t * 128:(t + 1) * 128, :].rearrange(
                        "j p c -> p j c"))
            kpdg = pb.tile([RP, S], BF16)
            nc.gpsimd.dma_start(out=kpdg[:].rearrange("p (j c) -> p j c", j=NC),
                              in_=ag_out[:, KVL:KVL + RP, :].rearrange(
                                  "j p c -> p j c"))
            qt = [pb.tile([128, S], BF16, name=f"qt{h}") for h in range(HPC)]
            qpt = [pb.tile([RP, S], BF16, name=f"qpt{h}") for h in range(HPC)]

            def unpack_q(h):
                nc.gpsimd.dma_start(
                    out=qt[h][:].rearrange("p (j c) -> p j c", j=NC),
                    in_=aa_out[h][:, 0:NOPE, :].rearrange("j p c -> p j c"))
                nc.gpsimd.dma_start(
                    out=qpt[h][:].rearrange("p (j c) -> p j c", j=NC),
                    in_=aa_out[h][:, NOPE:QD, :].rearrange("j p c -> p j c"))

            # --- kv_b: kn per head, v (both heads) keys-on-partitions ---
            kn = [pb.tile([128, S], BF16, name=f"kn{h}") for h in range(HPC)]
            for h in range(HPC):
                for cb in range(S // SSH):
                    p_k = ppc.tile([128, SSH], F32, tag="c")
                    for t in range(CT):
                        nc.tensor.matmul(p_k[:], wk[:, t, h * NOPE:(h + 1) * NOPE],
                                         ckvg[:, t, cb * SSH:(cb + 1) * SSH],
                                         start=(t == 0), stop=(t == CT - 1))
                    nc.any.tensor_copy(kn[h][:, cb * SSH:(cb + 1) * SSH], p_k[:])
            vst = pb.tile([128, S // 128, HPC * VD], BF16)
            for sb in range(S // 128):
                p_v = ppc.tile([128, HPC * VD], F32, tag="c")
                for t in range(CT):
                    nc.tensor.matmul(p_v[:], ckvg[:, t, sb * 128:(sb + 1) * 128],
                                     wv[:, t, :], start=(t == 0), stop=(t == CT - 1))
                nc.any.tensor_copy(vst[:, sb, :], p_v[:])

            # --- attention: heads outer (matches AllToAll arrival).
            # Software-pipelined: AV/den for ik are issued after the scores
            # of ik+1 so the PE never stalls on the exp; the per-(qb,h)
            # normalization finisher is deferred into the next iteration's
            # matmul stream.
            ao = pb.tile([128, NSB, HPC, SB], BF16)
            pending = None

            def finisher(fin):
                h, qb, p_o, p_d = fin
                den = pbn.tile([1, SB], BF16, tag="den")
                nc.vector.tensor_copy(den[:], p_d[:])
                p_b = ppc.tile([128, SB], F32, tag="c", name="bcast")
                nc.tensor.matmul(p_b[:], ones_r[:], den[:], start=True, stop=True)
                rec = pbn.tile([128, SB], F32, tag="rec")
                nc.vector.reciprocal(rec[:], p_b[:])
                nc.vector.tensor_mul(ao[:, qb, h, :], p_o[:], rec[:])

            def oproj(qb):
                for st in range(SB // 128):
                    sc = slice(qb * SB + st * 128, qb * SB + (st + 1) * 128)
                    ot = pbo.tile([128, HID], BF16, tag="ot")
                    for nb in range(HID // SB):
                        ncols = bass.ts(nb, SB)
                        p_c = ppc.tile([128, SB], F32, tag="c")
                        for hh in range(HPC):
                            nc.tensor.matmul(
                                p_c[:], ao[:, qb, hh, st * 128:(st + 1) * 128],
                                wo[:, hh, ncols],
                                start=(hh == 0), stop=(hh == HPC - 1))
                        nc.vector.tensor_copy(ot[:, ncols], p_c[:])
                    nc.sync.dma_start(out=d_out[sc, :], in_=ot[:])

            for h in range(HPC):
                unpack_q(h)
                for qb in range(NSB):
                    qcols = bass.ts(qb, SB)
                    nk = 4 * (qb + 1)
                    p_o = ppo.tile([128, SB], F32, tag="o")
                    p_d = ppd.tile([1, SB], F32, tag="d")
                    ework = []

                    def av_den(pik, pe_):
                        nc.tensor.matmul(p_o[:], vst[:, pik, h * VD:(h + 1) * VD],
                                         pe_[:], start=(pik == 0),
                                         stop=(pik == nk - 1))
                        nc.tensor.matmul(p_d[:], ones_c[:], pe_[:],
                                         start=(pik == 0), stop=(pik == nk - 1))

                    for ik in range(nk):
                        kc = slice(ik * 128, (ik + 1) * 128)
                        p_s = pps.tile([128, SB], F32, tag="s")
                        nc.tensor.matmul(p_s[:], kn[h][:, kc], qt[h][:, qcols],
                                         start=True, stop=False)
                        nc.tensor.matmul(p_s[:], kpdg[:, kc], qpt[h][:, qcols],
                                         start=False, stop=True)
                        if ik == 2 and pending is not None:
                            fin, oqb = pending
                            finisher(fin)
                            pending = None
                            if oqb is not None:
                                oproj(oqb)
                        if len(ework) == 2:
                            av_den(*ework.pop(0))
                        r = ik - 4 * qb
                        if r >= 0:
                            nc.vector.tensor_add(p_s[:], p_s[:], msk[:, r, :])
                        e = pbe.tile([128, SB], BF16, tag="e")
                        nc.scalar.activation(e[:], p_s[:], Exp, scale=SCALE)
                        ework.append((ik, e))
                    for item in ework:
                        av_den(*item)
                    pending = ((h, qb, p_o, p_d),
                               qb if h == HPC - 1 else None)
            fin, oqb = pending
            finisher(fin)
            if oqb is not None:
                oproj(oqb)


def _host_constants():
    inv_freq = 1.0 / (ROPE_THETA ** (np.arange(0, RP, 2, dtype=np.float32) / RP))
    t = np.arange(S, dtype=np.float32)
    freqs = np.outer(t, inv_freq)
    emb = np.concatenate([freqs, freqs], -1)          # [S, 64]
    cos, sin = np.cos(emb), np.sin(emb)
    cosd = np.concatenate([cos.T, cos.T], 0).astype(np.float32)   # [128, S]
    sind = np.concatenate([sin.T, sin.T], 0).astype(np.float32)

    # additive causal mask for diagonal 128-key blocks: [128, 4, 512]
    mska = np.zeros((128, 4, SB), np.float32)
    for r in range(4):
        for p in range(128):
            mska[p, r, :p + 128 * r] = NEG
    # rotate-half as matmul lhsT: same as baseline
    Q = np.zeros((RP, RP), np.float32)
    for i in range(RP // 2):
        Q[i, i + RP // 2] = -1.0
        Q[i + RP // 2, i] = 1.0
    P = np.zeros((128, 128), np.float32)
    P[:RP, :RP] = Q
    P[RP:, RP:] = Q
    rotq = P.T.copy()
    return cosd, sind, mska, rotq


def _tile3(w, kt):
    """[kt*128, F] -> [128, kt, F]"""
    return np.ascontiguousarray(
        w.reshape(kt, 128, w.shape[1]).transpose(1, 0, 2))


def kernel(hidden_states, w_q_a, q_a_weight, w_q_b, w_kv_a, kv_a_weight,
           w_kv_b, w_o):
    global LAST_RESULT
    if "nc" not in _CACHE:
        _CACHE["nc"] = _build_program()
    nc = _CACHE["nc"]

    x = np.asarray(hidden_states, np.float32)[0]       # [S, 2048]
    xt = np.ascontiguousarray(x.T)                     # [2048, S]
    wqa_t = np.asarray(w_q_a, np.float32).T            # [HID, QL]
    wkva_t = np.asarray(w_kv_a, np.float32).T          # [HID, KVL+RP]
    wqb_eff = np.asarray(w_q_b, np.float32) * np.asarray(q_a_weight, np.float32)[None, :]
    wkvb_eff = np.asarray(w_kv_b, np.float32) * np.asarray(kv_a_weight, np.float32)[None, :]
    won = np.asarray(w_o, np.float32)                  # [HID, H*VD]

    # q_b output feature permutation: nope head-major, then rope packed 2/tile
    perm = np.zeros(H * QD, np.int64)
    for h in range(H):
        perm[h * NOPE:(h + 1) * NOPE] = h * QD + np.arange(NOPE)
    base = H * NOPE
    for d in range(NC):
        for j in range(HPC):
            hh = 2 * d + j
            perm[base + d * 128 + j * RP: base + d * 128 + (j + 1) * RP] = \
                hh * QD + NOPE + np.arange(RP)
    wqb_p = wqb_eff[perm, :]                           # [3072, QL]

    cosd, sind, mska, rotq = _host_constants()

    wqa16 = _tile3(wqa_t, KT).astype(NPBF)
    wkva16 = _tile3(wkva_t, KT).astype(NPBF)
    wqb16 = _tile3(np.ascontiguousarray(wqb_p.T), QLT).astype(NPBF)
    rotq16 = rotq.astype(NPBF)

    shared = {"wqa16": wqa16, "wkva16": wkva16, "wqb16": wqb16,
              "maskadd": mska, "rotq16": rotq16}

    in_maps = []
    for c in range(NC):
        h0, h1 = HPC * c, HPC * c + 1
        wk_t = np.concatenate(
            [wkvb_eff[h * (NOPE + VD):h * (NOPE + VD) + NOPE] for h in (h0, h1)],
            0).T                                        # [KVL, 256]
        wv_t = np.concatenate(
            [wkvb_eff[h * (NOPE + VD) + NOPE:(h + 1) * (NOPE + VD)] for h in (h0, h1)],
            0).T                                        # [KVL, 256]
        wo_t = np.stack(
            [np.ascontiguousarray(won[:, h * VD:(h + 1) * VD].T) for h in (h0, h1)],
            1)                                          # [128, 2, HID]
        cols = slice(c * SSH, (c + 1) * SSH)
        im = dict(shared)
        im.update({
            "xt16": _tile3(np.ascontiguousarray(xt[:, cols]), KT).astype(NPBF),
            "wk16": _tile3(wk_t, CT).astype(NPBF),
            "wv16": _tile3(wv_t, CT).astype(NPBF),
            "wo16": np.ascontiguousarray(wo_t).astype(NPBF),
            "cosd": np.ascontiguousarray(cosd[:, cols]).astype(NPBF),
            "sind": np.ascontiguousarray(sind[:, cols]).astype(NPBF),
        })
        in_maps.append(im)

    res = run_bass_kernel_spmd(nc, in_maps, list(range(NC)))
    LAST_RESULT = res
    out = np.zeros((S, HID), np.float32)
    for c in range(NC):
        out += np.asarray(res.results[c]["out"]).astype(np.float32)
    return out.reshape(1, S, HID)


# revision 32
# speedup vs baseline: 1.8604x; 1.0172x over previous
"""MLA (DeepSeek-style multi-head latent attention) forward on 8 trn2 cores.

Layout v2: sequence-sharded LoRA-A + device collectives + bf16 matmuls.

Each core computes the LoRA-A projections (q_latent, compressed-kv latent,
k_pe) only for its 256-column sequence shard (8x less replicated work than
pure head-TP). The normalized kv latent + rope'd k_pe are AllGathered
(shared by every head); the per-head q vectors are redistributed with two
AllToAlls (one per head of each core's head pair) so attention runs fully
head-local: core c owns heads 2c, 2c+1 over the full sequence. kv_b expands
kn/v from the gathered latent per head; o_proj is input-split on heads and
the partial products are summed on the host (the unshard step).

All matmuls run in bf16 (1 PE cycle/row regardless of free-dim size, half
the DMA/communication bytes of fp32; final accuracy ~4e-3 vs the 2e-2
gate). Softmax runs over the key (partition) axis: exp on the scalar
engine, denominator via a ones-column matmul, broadcast of per-column
scalars via a K=1 matmul. RoPE rotate-half is a matmul against a constant
signed permutation. o_proj results are DMA'd directly from PSUM.
"""
import numpy as np
import ml_dtypes

import concourse.bass as bass
import concourse.tile as tile
from concourse import bacc, mybir
from concourse.bass_utils import run_bass_kernel_spmd

F32 = mybir.dt.float32
BF16 = mybir.dt.bfloat16
NPBF = ml_dtypes.bfloat16

HID = 2048
S = 2048
H = 16
QL = 1536
KVL = 512
NOPE = 128
RP = 64
VD = 128
QD = NOPE + RP              # 192
SCALE = QD ** -0.5
EPS = 1e-6
ROPE_THETA = 10000.0

NC = 8
HPC = 2                     # heads per core
SSH = S // NC               # 256-seq shard
KT = HID // 128             # 16
QLT = QL // 128             # 12
CT = KVL // 128             # 4
SB = 512                    # attention query block
NSB = S // SB               # 4
NEG = -30000.0

_CACHE = {}
LAST_RESULT = None


def _build_program():
    nc = bacc.Bacc("TRN2", target_bir_lowering=False, debug=False,
                   num_devices=NC)
    d_xt = nc.dram_tensor("xt16", [128, KT, SSH], BF16, kind="ExternalInput").ap()
    d_wqa = nc.dram_tensor("wqa16", [128, KT, QL], BF16, kind="ExternalInput").ap()
    d_wkva = nc.dram_tensor("wkva16", [128, KT, KVL + RP], BF16, kind="ExternalInput").ap()
    d_wqb = nc.dram_tensor("wqb16", [128, QLT, H * QD], BF16, kind="ExternalInput").ap()
    d_wk = nc.dram_tensor("wk16", [128, CT, HPC * NOPE], BF16, kind="ExternalInput").ap()
    d_wv = nc.dram_tensor("wv16", [128, CT, HPC * VD], BF16, kind="ExternalInput").ap()
    d_wo = nc.dram_tensor("wo16", [128, HPC, HID], BF16, kind="ExternalInput").ap()
    d_cos = nc.dram_tensor("cosd", [128, SSH], BF16, kind="ExternalInput").ap()
    d_sin = nc.dram_tensor("sind", [128, SSH], BF16, kind="ExternalInput").ap()
    d_msk = nc.dram_tensor("maskadd", [128, 4, SB], F32, kind="ExternalInput").ap()
    d_rotq = nc.dram_tensor("rotq16", [128, 128], BF16, kind="ExternalInput").ap()
    d_out = nc.dram_tensor("out", [S, HID], BF16, kind="ExternalOutput").ap()

    with tile.TileContext(nc) as tc:
        _mla(tc, d_xt, d_wqa, d_wkva, d_wqb, d_wk, d_wv, d_wo, d_cos, d_sin,
             d_msk, d_rotq, d_out)
    nc.compile()
    return nc


def _mla(tc, d_xt, d_wqa, d_wkva, d_wqb, d_wk, d_wv, d_wo, d_cos, d_sin,
         d_msk, d_rotq, d_out):
    nc = tc.nc
    Exp = mybir.ActivationFunctionType.Exp
    Sqrt = mybir.ActivationFunctionType.Sqrt
    groups = [list(range(NC))]

    with nc.allow_low_precision(reason="bf16 pipeline"), \
         tc.tile_pool(name="pdram", bufs=1, space="DRAM") as pdram, \
         tc.tile_pool(name="pconst", bufs=1) as pc, \
         tc.tile_pool(name="pglob", bufs=1) as pg:
        # ---- DRAM bounce buffers for collectives ----
        ag_in = pdram.tile([KVL + RP, SSH], BF16)
        ag_out = pdram.tile([NC, KVL + RP, SSH], BF16)
        aa_in = [pdram.tile([NC, QD, SSH], BF16, name=f"aain{i}") for i in range(HPC)]
        aa_out = [pdram.tile([NC, QD, SSH], BF16, name=f"aaout{i}") for i in range(HPC)]

        # ---- small constants ----
        ones_c = pc.tile([128, 1], BF16)
        nc.vector.memset(ones_c, 1.0)
        ones_r = pc.tile([1, 128], BF16)
        nc.vector.memset(ones_r, 1.0)
        eps1 = pc.tile([1, 1], F32)
        nc.vector.memset(eps1, EPS)
        rotq = pc.tile([128, 128], BF16)
        nc.sync.dma_start(out=rotq, in_=d_rotq)
        cosd = pc.tile([128, SSH], BF16)
        nc.sync.dma_start(out=cosd, in_=d_cos)
        sind = pc.tile([128, SSH], BF16)
        nc.sync.dma_start(out=sind, in_=d_sin)

        # =============== stage A: shard projections ===============
        with tc.tile_pool(name="pw", bufs=1) as pw, \
             tc.tile_pool(name="pA", bufs=1) as pa, \
             tc.tile_pool(name="pAq", bufs=3) as paq, \
             tc.tile_pool(name="pAs", bufs=3) as pas, \
             tc.tile_pool(name="ppA", bufs=3, space="PSUM") as ppa, \
             tc.tile_pool(name="ppSt", bufs=2, space="PSUM") as ppst, \
             tc.tile_pool(name="ppM", bufs=2, space="PSUM") as ppm:
            # ---- stage-A weights: per-k-slice tiles so matmuls start early ----
            xt = pw.tile([128, KT, SSH], BF16)
            # Early (eager) loads on the SP HW queue: only what the first
            # ~20us of compute needs. Everything else is loaded via gpsimd
            # SWDGE triggers placed AFTER the AllGather in program order, so
            # those transfers enter the global DMA FIFO behind the
            # collective staging instead of ahead of it.
            wkva = pw.tile([128, KT, KVL + RP], BF16)
            nc.sync.dma_start(out=xt, in_=d_xt)
            for half in range(2):
                hk = slice(half * KT // 2, (half + 1) * KT // 2)
                nc.sync.dma_start(out=wkva[:, hk, :], in_=d_wkva[:, hk, :])
            wqa = pw.tile([128, KT, QL], BF16)
            # gate wqa-colA behind wkva via a write-after-read dep: the
            # reader consumes both the colA region and the wkva tail, so the
            # colA DMA (a writer of that region) must wait for wkva.
            gate = pc.tile([1, 2], BF16, name="gate")
            nc.vector.tensor_tensor(gate[0:1, 0:1], wqa[0:1, 0, 0:1],
                                    wkva[0:1, KT - 1, 0:1], mybir.AluOpType.mult)
            nc.scalar.dma_start(out=wqa[:, :, 0:QL // 2], in_=d_wqa[:, :, 0:QL // 2])
            wqb = pw.tile([128, QLT, H * QD], BF16)
            wk = pg.tile([128, CT, HPC * NOPE], BF16)
            wv = pg.tile([128, CT, HPC * VD], BF16)
            wo = pg.tile([128, HPC, HID], BF16)
            msk = pg.tile([128, 4, SB], F32)

            # --- kv LoRA-A ---
            ckvu = pa.tile([128, CT, SSH], BF16)
            kpe = pa.tile([RP, SSH], BF16)
            p_st = ppst.tile([1, SSH], F32, tag="st", name="cstat")
            sqc = pa.tile([128, CT, SSH], BF16, name="sqc")
            for m in range(CT + 1):
                mw = 128 if m < CT else RP
                p_a = ppa.tile([128, SSH], F32, tag="a")
                for k in range(KT):
                    nc.tensor.matmul(p_a[:mw, :], wkva[:, k, m * 128:m * 128 + mw],
                                     xt[:, k, :], start=(k == 0), stop=(k == KT - 1))
                if m < CT:
                    nc.vector.tensor_copy(ckvu[:, m, :], p_a[:])
                    nc.scalar.activation(sqc[:, m, :], p_a[:],
                                         mybir.ActivationFunctionType.Square)
                else:
                    nc.vector.tensor_copy(kpe[:], p_a[:mw, :])
            for m in range(CT):
                nc.tensor.matmul(p_st[:], ones_c[:], sqc[:, m, :],
                                 start=(m == 0), stop=(m == CT - 1))
            rms_c = pa.tile([1, SSH], BF16)
            nc.scalar.activation(rms_c[:], p_st[:], Sqrt, scale=1.0 / KVL,
                                 bias=eps1[:])
            p_bc = ppm.tile([128, SSH], F32, tag="m")
            nc.tensor.matmul(p_bc[:], ones_r[:], rms_c[:], start=True, stop=True)
            invc = pa.tile([128, SSH], BF16)
            nc.vector.reciprocal(invc[:], p_bc[:])
            ckv = pa.tile([128, CT, SSH], BF16)
            for m in range(CT):
                nc.vector.tensor_mul(ckv[:, m, :], ckvu[:, m, :], invc[:])
            nc.scalar.dma_start(
                out=ag_in[0:KVL, :].rearrange("(t p) c -> p t c", p=128),
                in_=ckv[:])
            # --- k_pe rope (scale folded: none needed in bf16) ---
            p_rk = ppm.tile([128, SSH], F32, tag="m", name="rotk")
            nc.tensor.matmul(p_rk[:RP, :], rotq[0:RP, 0:RP], kpe[:],
                             start=True, stop=True)
            rk16 = pas.tile([RP, SSH], BF16, tag="rk")
            nc.vector.tensor_copy(rk16[:], p_rk[:RP, :])
            t1 = pas.tile([RP, SSH], BF16, tag="t1")
            nc.vector.tensor_mul(t1[:], kpe[:], cosd[0:RP, :])
            t2 = pas.tile([RP, SSH], BF16, tag="t2")
            nc.vector.tensor_mul(t2[:], rk16[:], sind[0:RP, :])
            kpd = pa.tile([RP, SSH], BF16)
            nc.vector.tensor_add(kpd[:], t1[:], t2[:])
            nc.scalar.dma_start(out=ag_in[KVL:KVL + RP, :], in_=kpd[:])
            # --- collective #1: AllGather latent+kpe ---
            nc.gpsimd.collective_compute(
                "AllGather", mybir.AluOpType.bypass, replica_groups=groups,
                ins=[ag_in[:].opt()], outs=[ag_out[:].opt()])
            # deferred bulk weight loads, chained with write-after-read
            # gates so each transfer enters the exclusive DMA FIFO after the
            # AllGather staging and after the previous weight transfer.
            agmark = pc.tile([1, 2], BF16, name="agmark")
            nc.gpsimd.dma_start(out=agmark[0:1, 0:2],
                                in_=ag_in[KVL + RP - 1:KVL + RP, 0:2])
            Mul = mybir.AluOpType.mult

            def gate_read(region, token):
                g = pas.tile([1, 1], BF16, tag="g8")
                nc.vector.tensor_tensor(g[:], region, token, Mul)

            gate_read(wqa[0:1, 0, QL - 1:QL], ckv[0:1, CT - 1, 0:1])
            nc.gpsimd.dma_start(out=wqa[:, :, QL // 2:QL],
                                in_=d_wqa[:, :, QL // 2:QL])
            gate_read(wqb[0:1, 0, H * QD - 1:H * QD], agmark[0:1, 0:1])
            nc.gpsimd.dma_start(out=wqb[:, :, H * NOPE:H * QD],
                                in_=d_wqb[:, :, H * NOPE:H * QD])
            for q4 in range(4):
                qs = slice(q4 * H * NOPE // 4, (q4 + 1) * H * NOPE // 4)
                gate_read(wqb[0:1, 0, q4 * H * NOPE // 4:q4 * H * NOPE // 4 + 1],
                          wqb[0:1, 0, H * QD - 1:H * QD])
                nc.gpsimd.dma_start(out=wqb[:, :, qs], in_=d_wqb[:, :, qs])
            for wtile, dsrc in ((wk, d_wk), (wv, d_wv), (wo, d_wo), (msk, d_msk)):
                gate_read(wtile[0:1, 0, 0:1], wqb[0:1, 0, H * NOPE - 1:H * NOPE])
                nc.gpsimd.dma_start(out=wtile, in_=dsrc)

            # --- q LoRA-A ---
            qlu = pa.tile([128, QLT, SSH], BF16)
            p_qst = ppst.tile([1, SSH], F32, tag="st", name="qstat")
            sqq = pa.tile([128, QLT, SSH], BF16, name="sqq")
            for k in range(QLT):
                p_a = ppa.tile([128, SSH], F32, tag="a")
                for kk in range(KT):
                    nc.tensor.matmul(p_a[:], wqa[:, kk, k * 128:(k + 1) * 128],
                                     xt[:, kk, :], start=(kk == 0), stop=(kk == KT - 1))
                nc.vector.tensor_copy(qlu[:, k, :], p_a[:])
                nc.vector.tensor_mul(sqq[:, k, :], qlu[:, k, :], qlu[:, k, :])
            for k in range(QLT):
                nc.tensor.matmul(p_qst[:], ones_c[:], sqq[:, k, :],
                                 start=(k == 0), stop=(k == QLT - 1))
            rms_q = pa.tile([1, SSH], BF16)
            nc.scalar.activation(rms_q[:], p_qst[:], Sqrt, scale=1.0 / QL,
                                 bias=eps1[:])
            p_bq = ppm.tile([128, SSH], F32, tag="m")
            nc.tensor.matmul(p_bq[:], ones_r[:], rms_q[:], start=True, stop=True)
            invq = pa.tile([128, SSH], F32)
            nc.vector.reciprocal(invq[:], p_bq[:])

            # --- q_b for all heads: rope tiles (16..23) first so the rope
            # chain and the AllToAll staging DMAs start as early as possible;
            # nope tiles follow in parity order (A2A#1's inputs first).
            q16 = pa.tile([128, H + NC, SSH], BF16, name="q16")

            def qb_group(mt):
                p_q = ppa.tile([128, SSH], F32, tag="a")
                for k in range(QLT):
                    nc.tensor.matmul(p_q[:], wqb[:, k, mt * 128:(mt + 1) * 128],
                                     qlu[:, k, :], start=(k == 0), stop=(k == QLT - 1))
                nc.vector.tensor_mul(q16[:, mt, :], p_q[:], invq[:])

            for mt in range(H, H + NC):
                qb_group(mt)
            # rope rotate-half + cos/sin (inputs ready; no PE stalls)
            for d in range(NC):
                p_rq = ppm.tile([128, SSH], F32, tag="m")
                nc.tensor.matmul(p_rq[:], rotq[:], q16[:, H + d, :],
                                 start=True, stop=True)
                rq16 = pas.tile([128, SSH], BF16, tag="rk", name="rq16")
                nc.vector.tensor_copy(rq16[:], p_rq[:])
                t1q = pas.tile([128, SSH], BF16, tag="t1")
                nc.vector.tensor_mul(t1q[:], q16[:, H + d, :], cosd[:])
                t2q = pas.tile([128, SSH], BF16, tag="t2")
                nc.vector.tensor_mul(t2q[:], rq16[:], sind[:])
                nc.vector.tensor_add(q16[:, H + d, :], t1q[:], t2q[:])
            for mt in range(0, H, 2):
                qb_group(mt)
            nc.gpsimd.dma_start(
                out=aa_in[0][:, 0:NOPE, :].rearrange("j p c -> p j c"),
                in_=q16[:, 0:H:2, :].rearrange("p j c -> p j c"))
            nc.gpsimd.dma_start(
                out=aa_in[0][:, NOPE:QD, :].rearrange("j p c -> p j c"),
                in_=q16[0:RP, H:H + NC, :])
            nc.gpsimd.collective_compute(
                "AllToAll", mybir.AluOpType.bypass, replica_groups=groups,
                ins=[aa_in[0][:].opt()], outs=[aa_out[0][:].opt()])
            for mt in range(1, H, 2):
                qb_group(mt)
            nc.gpsimd.dma_start(
                out=aa_in[1][:, 0:NOPE, :].rearrange("j p c -> p j c"),
                in_=q16[:, 1:H:2, :].rearrange("p j c -> p j c"))
            nc.gpsimd.dma_start(
                out=aa_in[1][:, NOPE:QD, :].rearrange("j p c -> p j c"),
                in_=q16[RP:128, H:H + NC, :])
            nc.gpsimd.collective_compute(
                "AllToAll", mybir.AluOpType.bypass, replica_groups=groups,
                ins=[aa_in[1][:].opt()], outs=[aa_out[1][:].opt()])

        # =============== stage B: head-local attention ===============
        with tc.tile_pool(name="pB", bufs=1) as pb, \
             tc.tile_pool(name="pBe", bufs=6) as pbe, \
             tc.tile_pool(name="pBo", bufs=3) as pbo, \
             tc.tile_pool(name="pBn", bufs=2) as pbn, \
             tc.tile_pool(name="ppS", bufs=3, space="PSUM") as pps, \
             tc.tile_pool(name="ppO", bufs=2, space="PSUM") as ppo, \
             tc.tile_pool(name="ppD", bufs=1, space="PSUM") as ppd, \
             tc.tile_pool(name="ppC", bufs=2, space="PSUM") as ppc:
            ckvg = pb.tile([128, CT, S], BF16)
            for t in range(CT):
                nc.gpsimd.dma_start(
                    out=ckvg[:, t, :].rearrange("p (j c) -> p j c", j=NC),
                    in_=ag_out[:, t * 128:(t + 1) * 128, :].rearrange(
                        "j p c -> p j c"))
            kpdg = pb.tile([RP, S], BF16)
            nc.gpsimd.dma_start(out=kpdg[:].rearrange("p (j c) -> p j c", j=NC),
                              in_=ag_out[:, KVL:KVL + RP, :].rearrange(
                                  "j p c -> p j c"))
            qt = [pb.tile([128, S], BF16, name=f"qt{h}") for h in range(HPC)]
            qpt = [pb.tile([RP, S], BF16, name=f"qpt{h}") for h in range(HPC)]

            def unpack_q(h):
                nc.gpsimd.dma_start(
                    out=qt[h][:].rearrange("p (j c) -> p j c", j=NC),
                    in_=aa_out[h][:, 0:NOPE, :].rearrange("j p c -> p j c"))
                nc.gpsimd.dma_start(
                    out=qpt[h][:].rearrange("p (j c) -> p j c", j=NC),
                    in_=aa_out[h][:, NOPE:QD, :].rearrange("j p c -> p j c"))

            # --- kv_b: kn per head, v (both heads) keys-on-partitions ---
            kn = [pb.tile([128, S], BF16, name=f"kn{h}") for h in range(HPC)]
            for h in range(HPC):
                for cb in range(S // SSH):
                    p_k = ppc.tile([128, SSH], F32, tag="c")
                    for t in range(CT):
                        nc.tensor.matmul(p_k[:], wk[:, t, h * NOPE:(h + 1) * NOPE],
                                         ckvg[:, t, cb * SSH:(cb + 1) * SSH],
                                         start=(t == 0), stop=(t == CT - 1))
                    nc.any.tensor_copy(kn[h][:, cb * SSH:(cb + 1) * SSH], p_k[:])
            vst = pb.tile([128, S // 128, HPC * VD], BF16)
            for sb in range(S // 128):
                p_v = ppc.tile([128, HPC * VD], F32, tag="c")
                for t in range(CT):
                    nc.tensor.matmul(p_v[:], ckvg[:, t, sb * 128:(sb + 1) * 128],
                                     wv[:, t, :], start=(t == 0), stop=(t == CT - 1))
                nc.any.tensor_copy(vst[:, sb, :], p_v[:])

            # --- attention: heads outer (matches AllToAll arrival).
            # Software-pipelined: AV/den for ik are issued after the scores
            # of ik+1 so the PE never stalls on the exp; the per-(qb,h)
            # normalization finisher is deferred into the next iteration's
            # matmul stream.
            ao = pb.tile([128, NSB, HPC, SB], BF16)
            pending = None

            def finisher(fin):
                h, qb, p_o, p_d = fin
                den = pbn.tile([1, SB], BF16, tag="den")
                nc.vector.tensor_copy(den[:], p_d[:])
                p_b = ppc.tile([128, SB], F32, tag="c", name="bcast")
                nc.tensor.matmul(p_b[:], ones_r[:], den[:], start=True, stop=True)
                rec = pbn.tile([128, SB], F32, tag="rec")
                nc.vector.reciprocal(rec[:], p_b[:])
                nc.vector.tensor_mul(ao[:, qb, h, :], p_o[:], rec[:])

            def oproj(qb):
                for st in range(SB // 128):
                    sc = slice(qb * SB + st * 128, qb * SB + (st + 1) * 128)
                    ot = pbo.tile([128, HID], BF16, tag="ot")
                    for nb in range(HID // SB):
                        ncols = bass.ts(nb, SB)
                        p_c = ppc.tile([128, SB], F32, tag="c")
                        for hh in range(HPC):
                            nc.tensor.matmul(
                                p_c[:], ao[:, qb, hh, st * 128:(st + 1) * 128],
                                wo[:, hh, ncols],
                                start=(hh == 0), stop=(hh == HPC - 1))
                        nc.vector.tensor_copy(ot[:, ncols], p_c[:])
                    nc.sync.dma_start(out=d_out[sc, :], in_=ot[:])

            for h in range(HPC):
                unpack_q(h)
                for qb in range(NSB):
                    qcols = bass.ts(qb, SB)
                    nk = 4 * (qb + 1)
                    p_o = ppo.tile([128, SB], F32, tag="o")
                    p_d = ppd.tile([1, SB], F32, tag="d")
                    ework = []

                    def av_den(pik, pe_):
                        nc.tensor.matmul(p_o[:], vst[:, pik, h * VD:(h + 1) * VD],
                                         pe_[:], start=(pik == 0),
                                         stop=(pik == nk - 1))
                        nc.tensor.matmul(p_d[:], ones_c[:], pe_[:],
                                         start=(pik == 0), stop=(pik == nk - 1))

                    for ik in range(nk):
                        kc = slice(ik * 128, (ik + 1) * 128)
                        p_s = pps.tile([128, SB], F32, tag="s")
                        nc.tensor.matmul(p_s[:], kn[h][:, kc], qt[h][:, qcols],
                                         start=True, stop=False)
                        nc.tensor.matmul(p_s[:], kpdg[:, kc], qpt[h][:, qcols],
                                         start=False, stop=True)
                        if ik == 2 and pending is not None:
                            fin, oqb = pending
                            finisher(fin)
                            pending = None
                            if oqb is not None:
                                oproj(oqb)
                        if len(ework) == 2:
                            av_den(*ework.pop(0))
                        r = ik - 4 * qb
                        if r >= 0:
                            nc.vector.tensor_add(p_s[:], p_s[:], msk[:, r, :])
                        e = pbe.tile([128, SB], BF16, tag="e")
                        nc.scalar.activation(e[:], p_s[:], Exp, scale=SCALE)
                        ework.append((ik, e))
                    for item in ework:
                        av_den(*item)
                    pending = ((h, qb, p_o, p_d),
                               qb if h == HPC - 1 else None)
            fin, oqb = pending
            finisher(fin)
            if oqb is not None:
                oproj(oqb)


def _host_constants():
    inv_freq = 1.0 / (ROPE_THETA ** (np.arange(0, RP, 2, dtype=np.float32) / RP))
    t = np.arange(S, dtype=np.float32)
    freqs = np.outer(t, inv_freq)
    emb = np.concatenate([freqs, freqs], -1)          # [S, 64]
    cos, sin = np.cos(emb), np.sin(emb)
    cosd = np.concatenate([cos.T, cos.T], 0).astype(np.float32)   # [128, S]
    sind = np.concatenate([sin.T, sin.T], 0).astype(np.float32)

    # additive causal mask for diagonal 128-key blocks: [128, 4, 512]
    mska = np.zeros((128, 4, SB), np.float32)
    for r in range(4):
        for p in range(128):
            mska[p, r, :p + 128 * r] = NEG
    # rotate-half as matmul lhsT: same as baseline
    Q = np.zeros((RP, RP), np.float32)
    for i in range(RP // 2):
        Q[i, i + RP // 2] = -1.0
        Q[i + RP // 2, i] = 1.0
    P = np.zeros((128, 128), np.float32)
    P[:RP, :RP] = Q
    P[RP:, RP:] = Q
    rotq = P.T.copy()
    return cosd, sind, mska, rotq


def _tile3(w, kt):
    """[kt*128, F] -> [128, kt, F]"""
    return np.ascontiguousarray(
        w.reshape(kt, 128, w.shape[1]).transpose(1, 0, 2))


def kernel(hidden_states, w_q_a, q_a_weight, w_q_b, w_kv_a, kv_a_weight,
           w_kv_b, w_o):
    global LAST_RESULT
    if "nc" not in _CACHE:
        _CACHE["nc"] = _build_program()
    nc = _CACHE["nc"]

    x = np.asarray(hidden_states, np.float32)[0]       # [S, 2048]
    xt = np.ascontiguousarray(x.T)                     # [2048, S]
    wqa_t = np.asarray(w_q_a, np.float32).T            # [HID, QL]
    wkva_t = np.asarray(w_kv_a, np.float32).T          # [HID, KVL+RP]
    wqb_eff = np.asarray(w_q_b, np.float32) * np.asarray(q_a_weight, np.float32)[None, :]
    wkvb_eff = np.asarray(w_kv_b, np.float32) * np.asarray(kv_a_weight, np.float32)[None, :]
    won = np.asarray(w_o, np.float32)                  # [HID, H*VD]

    # q_b output feature permutation: nope head-major, then rope packed 2/tile
    perm = np.zeros(H * QD, np.int64)
    for h in range(H):
        perm[h * NOPE:(h + 1) * NOPE] = h * QD + np.arange(NOPE)
    base = H * NOPE
    for d in range(NC):
        for j in range(HPC):
            hh = 2 * d + j
            perm[base + d * 128 + j * RP: base + d * 128 + (j + 1) * RP] = \
                hh * QD + NOPE + np.arange(RP)
    wqb_p = wqb_eff[perm, :]                           # [3072, QL]

    cosd, sind, mska, rotq = _host_constants()

    wqa16 = _tile3(wqa_t, KT).astype(NPBF)
    wkva16 = _tile3(wkva_t, KT).astype(NPBF)
    wqb16 = _tile3(np.ascontiguousarray(wqb_p.T), QLT).astype(NPBF)
    rotq16 = rotq.astype(NPBF)

    shared = {"wqa16": wqa16, "wkva16": wkva16, "wqb16": wqb16,
              "maskadd": mska, "rotq16": rotq16}

    in_maps = []
    for c in range(NC):
        h0, h1 = HPC * c, HPC * c + 1
        wk_t = np.concatenate(
            [wkvb_eff[h * (NOPE + VD):h * (NOPE + VD) + NOPE] for h in (h0, h1)],
            0).T                                        # [KVL, 256]
        wv_t = np.concatenate(
            [wkvb_eff[h * (NOPE + VD) + NOPE:(h + 1) * (NOPE + VD)] for h in (h0, h1)],
            0).T                                        # [KVL, 256]
        wo_t = np.stack(
            [np.ascontiguousarray(won[:, h * VD:(h + 1) * VD].T) for h in (h0, h1)],
            1)                                          # [128, 2, HID]
        cols = slice(c * SSH, (c + 1) * SSH)
        im = dict(shared)
        im.update({
            "xt16": _tile3(np.ascontiguousarray(xt[:, cols]), KT).astype(NPBF),
            "wk16": _tile3(wk_t, CT).astype(NPBF),
            "wv16": _tile3(wv_t, CT).astype(NPBF),
            "wo16": np.ascontiguousarray(wo_t).astype(NPBF),
            "cosd": np.ascontiguousarray(cosd[:, cols]).astype(NPBF),
            "sind": np.ascontiguousarray(sind[:, cols]).astype(NPBF),
        })
        in_maps.append(im)

    res = run_bass_kernel_spmd(nc, in_maps, list(range(NC)))
    LAST_RESULT = res
    out = np.zeros((S, HID), np.float32)
    for c in range(NC):
        out += np.asarray(res.results[c]["out"]).astype(np.float32)
    return out.reshape(1, S, HID)


# revision 39
# speedup vs baseline: 1.8644x; 1.0022x over previous
"""MLA (DeepSeek-style multi-head latent attention) forward on 8 trn2 cores.

Layout v2: sequence-sharded LoRA-A + device collectives + bf16 matmuls.

Each core computes the LoRA-A projections (q_latent, compressed-kv latent,
k_pe) only for its 256-column sequence shard (8x less replicated work than
pure head-TP). The normalized kv latent + rope'd k_pe are AllGathered
(shared by every head); the per-head q vectors are redistributed with two
AllToAlls (one per head of each core's head pair) so attention runs fully
head-local: core c owns heads 2c, 2c+1 over the full sequence. kv_b expands
kn/v from the gathered latent per head; o_proj is input-split on heads and
the partial products are summed on the host (the unshard step).

All matmuls run in bf16 (1 PE cycle/row regardless of free-dim size, half
the DMA/communication bytes of fp32; final accuracy ~4e-3 vs the 2e-2
gate). Softmax runs over the key (partition) axis: exp on the scalar
engine, denominator via a ones-column matmul, broadcast of per-column
scalars via a K=1 matmul. RoPE rotate-half is a matmul against a constant
signed permutation. o_proj results are DMA'd directly from PSUM.
"""
import numpy as np
import ml_dtypes

import concourse.bass as bass
import concourse.tile as tile
from concourse import bacc, mybir
from concourse.bass_utils import run_bass_kernel_spmd

F32 = mybir.dt.float32
BF16 = mybir.dt.bfloat16
NPBF = ml_dtypes.bfloat16

HID = 2048
S = 2048
H = 16
QL = 1536
KVL = 512
NOPE = 128
RP = 64
VD = 128
QD = NOPE + RP              # 192
SCALE = QD ** -0.5
EPS = 1e-6
ROPE_THETA = 10000.0

NC = 8
HPC = 2                     # heads per core
SSH = S // NC               # 256-seq shard
KT = HID // 128             # 16
QLT = QL // 128             # 12
CT = KVL // 128             # 4
SB = 512                    # attention query block
NSB = S // SB               # 4
NEG = -30000.0

_CACHE = {}
LAST_RESULT = None


def _build_program():
    nc = bacc.Bacc("TRN2", target_bir_lowering=False, debug=False,
                   num_devices=NC)
    d_xt = nc.dram_tensor("xt16", [128, KT, SSH], BF16, kind="ExternalInput").ap()
    d_wqa = nc.dram_tensor("wqa16", [128, KT, QL], BF16, kind="ExternalInput").ap()
    d_wkva = nc.dram_tensor("wkva16", [128, KT, KVL + RP], BF16, kind="ExternalInput").ap()
    d_wqb = nc.dram_tensor("wqb16", [128, QLT, H * QD], BF16, kind="ExternalInput").ap()
    d_wk = nc.dram_tensor("wk16", [128, CT, HPC * NOPE], BF16, kind="ExternalInput").ap()
    d_wv = nc.dram_tensor("wv16", [128, CT, HPC * VD], BF16, kind="ExternalInput").ap()
    d_wo = nc.dram_tensor("wo16", [128, HPC, HID], BF16, kind="ExternalInput").ap()
    d_cos = nc.dram_tensor("cosd", [128, SSH], BF16, kind="ExternalInput").ap()
    d_sin = nc.dram_tensor("sind", [128, SSH], BF16, kind="ExternalInput").ap()
    d_msk = nc.dram_tensor("maskadd", [128, 4, SB], F32, kind="ExternalInput").ap()
    d_rotq = nc.dram_tensor("rotq16", [128, 128], BF16, kind="ExternalInput").ap()
    d_out = nc.dram_tensor("out", [S, HID], BF16, kind="ExternalOutput").ap()

    with tile.TileContext(nc) as tc:
        _mla(tc, d_xt, d_wqa, d_wkva, d_wqb, d_wk, d_wv, d_wo, d_cos, d_sin,
             d_msk, d_rotq, d_out)
    nc.compile()
    return nc


def _mla(tc, d_xt, d_wqa, d_wkva, d_wqb, d_wk, d_wv, d_wo, d_cos, d_sin,
         d_msk, d_rotq, d_out):
    nc = tc.nc
    Exp = mybir.ActivationFunctionType.Exp
    Sqrt = mybir.ActivationFunctionType.Sqrt
    groups = [list(range(NC))]

    with nc.allow_low_precision(reason="bf16 pipeline"), \
         tc.tile_pool(name="pdram", bufs=1, space="DRAM") as pdram, \
         tc.tile_pool(name="pconst", bufs=1) as pc, \
         tc.tile_pool(name="pglob", bufs=1) as pg:
        # ---- DRAM bounce buffers for collectives ----
        ag_in = pdram.tile([KVL + RP, SSH], BF16)
        ag_out = pdram.tile([NC, KVL + RP, SSH], BF16)
        aa_in = [pdram.tile([NC, QD, SSH], BF16, name=f"aain{i}") for i in range(HPC)]
        aa_out = [pdram.tile([NC, QD, SSH], BF16, name=f"aaout{i}") for i in range(HPC)]

        # ---- small constants ----
        ones_c = pc.tile([128, 1], BF16)
        nc.vector.memset(ones_c, 1.0)
        ones_r = pc.tile([1, 128], BF16)
        nc.vector.memset(ones_r, 1.0)
        eps1 = pc.tile([1, 1], F32)
        nc.vector.memset(eps1, EPS)
        warm = pc.tile([1, 2], F32, name="actwarm")
        nc.scalar.activation(warm[0:1, 0:1], eps1[:], Sqrt)
        nc.scalar.activation(warm[0:1, 1:2], eps1[:], Exp)
        rotq = pc.tile([128, 128], BF16)
        nc.sync.dma_start(out=rotq, in_=d_rotq)
        cosd = pc.tile([128, SSH], BF16)
        nc.sync.dma_start(out=cosd, in_=d_cos)
        sind = pc.tile([128, SSH], BF16)
        nc.sync.dma_start(out=sind, in_=d_sin)

        # =============== stage A: shard projections ===============
        with tc.tile_pool(name="pw", bufs=1) as pw, \
             tc.tile_pool(name="pA", bufs=1) as pa, \
             tc.tile_pool(name="pAq", bufs=3) as paq, \
             tc.tile_pool(name="pAs", bufs=3) as pas, \
             tc.tile_pool(name="ppA", bufs=3, space="PSUM") as ppa, \
             tc.tile_pool(name="ppSt", bufs=2, space="PSUM") as ppst, \
             tc.tile_pool(name="ppM", bufs=2, space="PSUM") as ppm:
            # ---- stage-A weights: per-k-slice tiles so matmuls start early ----
            xt = pw.tile([128, KT, SSH], BF16)
            # Early (eager) loads on the SP HW queue: only what the first
            # ~20us of compute needs. Everything else is loaded via gpsimd
            # SWDGE triggers placed AFTER the AllGather in program order, so
            # those transfers enter the global DMA FIFO behind the
            # collective staging instead of ahead of it.
            wkva = pw.tile([128, KT, KVL + RP], BF16)
            for half in range(2):
                hk = slice(half * KT // 2, (half + 1) * KT // 2)
                nc.sync.dma_start(out=xt[:, hk, :], in_=d_xt[:, hk, :])
                nc.sync.dma_start(out=wkva[:, hk, :], in_=d_wkva[:, hk, :])
            wqa = pw.tile([128, KT, QL], BF16)
            # gate wqa-colA behind wkva via a write-after-read dep: the
            # reader consumes both the colA region and the wkva tail, so the
            # colA DMA (a writer of that region) must wait for wkva.
            gate = pc.tile([1, 2], BF16, name="gate")
            nc.vector.tensor_tensor(gate[0:1, 0:1], wqa[0:1, 0, 0:1],
                                    wkva[0:1, KT - 1, 0:1], mybir.AluOpType.mult)
            nc.scalar.dma_start(out=wqa[:, :, 0:QL // 2], in_=d_wqa[:, :, 0:QL // 2])
            wqb = pw.tile([128, QLT, H * QD], BF16)
            wk = pg.tile([128, CT, HPC * NOPE], BF16)
            wv = pg.tile([128, CT, HPC * VD], BF16)
            wo = pg.tile([128, HPC, HID], BF16)
            msk = pg.tile([128, 4, SB], F32)

            # --- kv LoRA-A ---
            ckvu = pa.tile([128, CT, SSH], BF16)
            kpe = pa.tile([RP, SSH], BF16)
            p_st = ppst.tile([1, SSH], F32, tag="st", name="cstat")
            sqc = pa.tile([128, CT, SSH], BF16, name="sqc")
            for m in range(CT + 1):
                mw = 128 if m < CT else RP
                p_a = ppa.tile([128, SSH], F32, tag="a")
                for k in range(KT):
                    nc.tensor.matmul(p_a[:mw, :], wkva[:, k, m * 128:m * 128 + mw],
                                     xt[:, k, :], start=(k == 0), stop=(k == KT - 1))
                if m < CT:
                    nc.vector.tensor_copy(ckvu[:, m, :], p_a[:])
                    nc.vector.tensor_mul(sqc[:, m, :], ckvu[:, m, :], ckvu[:, m, :])
                else:
                    nc.vector.tensor_copy(kpe[:], p_a[:mw, :])
            for m in range(CT):
                nc.tensor.matmul(p_st[:], ones_c[:], sqc[:, m, :],
                                 start=(m == 0), stop=(m == CT - 1))
            rms_c = pa.tile([1, SSH], BF16)
            nc.scalar.activation(rms_c[:], p_st[:], Sqrt, scale=1.0 / KVL,
                                 bias=eps1[:])
            p_bc = ppm.tile([128, SSH], F32, tag="m")
            nc.tensor.matmul(p_bc[:], ones_r[:], rms_c[:], start=True, stop=True)
            invc = pa.tile([128, SSH], BF16)
            nc.vector.reciprocal(invc[:], p_bc[:])
            ckv = pa.tile([128, CT, SSH], BF16)
            for m in range(CT):
                nc.vector.tensor_mul(ckv[:, m, :], ckvu[:, m, :], invc[:])
            nc.scalar.dma_start(
                out=ag_in[0:KVL, :].rearrange("(t p) c -> p t c", p=128),
                in_=ckv[:])
            # --- k_pe rope (scale folded: none needed in bf16) ---
            p_rk = ppm.tile([128, SSH], F32, tag="m", name="rotk")
            nc.tensor.matmul(p_rk[:RP, :], rotq[0:RP, 0:RP], kpe[:],
                             start=True, stop=True)
            rk16 = pas.tile([RP, SSH], BF16, tag="rk")
            nc.vector.tensor_copy(rk16[:], p_rk[:RP, :])
            t1 = pas.tile([RP, SSH], BF16, tag="t1")
            nc.vector.tensor_mul(t1[:], kpe[:], cosd[0:RP, :])
            t2 = pas.tile([RP, SSH], BF16, tag="t2")
            nc.vector.tensor_mul(t2[:], rk16[:], sind[0:RP, :])
            kpd = pa.tile([RP, SSH], BF16)
            nc.vector.tensor_add(kpd[:], t1[:], t2[:])
            nc.scalar.dma_start(out=ag_in[KVL:KVL + RP, :], in_=kpd[:])
            # --- collective #1: AllGather latent+kpe ---
            nc.gpsimd.collective_compute(
                "AllGather", mybir.AluOpType.bypass, replica_groups=groups,
                ins=[ag_in[:].opt()], outs=[ag_out[:].opt()])
            # deferred bulk weight loads, chained with write-after-read
            # gates so each transfer enters the exclusive DMA FIFO after the
            # AllGather staging and after the previous weight transfer.
            agmark = pc.tile([1, 2], BF16, name="agmark")
            nc.gpsimd.dma_start(out=agmark[0:1, 0:2],
                                in_=ag_in[KVL + RP - 1:KVL + RP, 0:2])
            Mul = mybir.AluOpType.mult

            def gate_read(region, token):
                g = pas.tile([1, 1], BF16, tag="g8")
                nc.vector.tensor_tensor(g[:], region, token, Mul)

            gate_read(wqa[0:1, 0, QL - 1:QL], ckv[0:1, CT - 1, 0:1])
            nc.gpsimd.dma_start(out=wqa[:, :, QL // 2:QL],
                                in_=d_wqa[:, :, QL // 2:QL])
            gate_read(wqb[0:1, 0, H * QD - 1:H * QD], agmark[0:1, 0:1])
            nc.gpsimd.dma_start(out=wqb[:, :, H * NOPE:H * QD],
                                in_=d_wqb[:, :, H * NOPE:H * QD])
            for q4 in range(4):
                qs = slice(q4 * H * NOPE // 4, (q4 + 1) * H * NOPE // 4)
                gate_read(wqb[0:1, 0, q4 * H * NOPE // 4:q4 * H * NOPE // 4 + 1],
                          wqb[0:1, 0, H * QD - 1:H * QD])
                nc.gpsimd.dma_start(out=wqb[:, :, qs], in_=d_wqb[:, :, qs])
            for wtile, dsrc in ((wk, d_wk), (wv, d_wv), (wo, d_wo), (msk, d_msk)):
                gate_read(wtile[0:1, 0, 0:1], wqb[0:1, 0, H * NOPE - 1:H * NOPE])
                nc.gpsimd.dma_start(out=wtile, in_=dsrc)

            # --- q LoRA-A ---
            qlu = pa.tile([128, QLT, SSH], BF16)
            p_qst = ppst.tile([1, SSH], F32, tag="st", name="qstat")
            sqq = pa.tile([128, QLT, SSH], BF16, name="sqq")
            for k in range(QLT):
                p_a = ppa.tile([128, SSH], F32, tag="a")
                for kk in range(KT):
                    nc.tensor.matmul(p_a[:], wqa[:, kk, k * 128:(k + 1) * 128],
                                     xt[:, kk, :], start=(kk == 0), stop=(kk == KT - 1))
                nc.vector.tensor_copy(qlu[:, k, :], p_a[:])
                nc.vector.tensor_mul(sqq[:, k, :], qlu[:, k, :], qlu[:, k, :])
            for k in range(QLT):
                nc.tensor.matmul(p_qst[:], ones_c[:], sqq[:, k, :],
                                 start=(k == 0), stop=(k == QLT - 1))
            rms_q = pa.tile([1, SSH], BF16)
            nc.scalar.activation(rms_q[:], p_qst[:], Sqrt, scale=1.0 / QL,
                                 bias=eps1[:])
            p_bq = ppm.tile([128, SSH], F32, tag="m")
            nc.tensor.matmul(p_bq[:], ones_r[:], rms_q[:], start=True, stop=True)
            invq = pa.tile([128, SSH], F32)
            nc.vector.reciprocal(invq[:], p_bq[:])

            # --- q_b for all heads: rope tiles (16..23) first so the rope
            # chain and the AllToAll staging DMAs start as early as possible;
            # nope tiles follow in parity order (A2A#1's inputs first).
            q16 = pa.tile([128, H + NC, SSH], BF16, name="q16")

            def qb_group(mt):
                p_q = ppa.tile([128, SSH], F32, tag="a")
                for k in range(QLT):
                    nc.tensor.matmul(p_q[:], wqb[:, k, mt * 128:(mt + 1) * 128],
                                     qlu[:, k, :], start=(k == 0), stop=(k == QLT - 1))
                nc.vector.tensor_mul(q16[:, mt, :], p_q[:], invq[:])

            for mt in range(H, H + NC):
                qb_group(mt)
            # rope rotate-half + cos/sin (inputs ready; no PE stalls)
            for d in range(NC):
                p_rq = ppm.tile([128, SSH], F32, tag="m")
                nc.tensor.matmul(p_rq[:], rotq[:], q16[:, H + d, :],
                                 start=True, stop=True)
                rq16 = pas.tile([128, SSH], BF16, tag="rk", name="rq16")
                nc.vector.tensor_copy(rq16[:], p_rq[:])
                t1q = pas.tile([128, SSH], BF16, tag="t1")
                nc.vector.tensor_mul(t1q[:], q16[:, H + d, :], cosd[:])
                t2q = pas.tile([128, SSH], BF16, tag="t2")
                nc.vector.tensor_mul(t2q[:], rq16[:], sind[:])
                nc.vector.tensor_add(q16[:, H + d, :], t1q[:], t2q[:])
            for mt in range(0, H, 2):
                qb_group(mt)
            nc.gpsimd.dma_start(
                out=aa_in[0][:, 0:NOPE, :].rearrange("j p c -> p j c"),
                in_=q16[:, 0:H:2, :].rearrange("p j c -> p j c"))
            nc.gpsimd.dma_start(
                out=aa_in[0][:, NOPE:QD, :].rearrange("j p c -> p j c"),
                in_=q16[0:RP, H:H + NC, :])
            nc.gpsimd.collective_compute(
                "AllToAll", mybir.AluOpType.bypass, replica_groups=groups,
                ins=[aa_in[0][:].opt()], outs=[aa_out[0][:].opt()])
            for mt in range(1, H, 2):
                qb_group(mt)
            nc.gpsimd.dma_start(
                out=aa_in[1][:, 0:NOPE, :].rearrange("j p c -> p j c"),
                in_=q16[:, 1:H:2, :].rearrange("p j c -> p j c"))
            nc.gpsimd.dma_start(
                out=aa_in[1][:, NOPE:QD, :].rearrange("j p c -> p j c"),
                in_=q16[RP:128, H:H + NC, :])
            nc.gpsimd.collective_compute(
                "AllToAll", mybir.AluOpType.bypass, replica_groups=groups,
                ins=[aa_in[1][:].opt()], outs=[aa_out[1][:].opt()])

        # =============== stage B: head-local attention ===============
        with tc.tile_pool(name="pB", bufs=1) as pb, \
             tc.tile_pool(name="pBe", bufs=6) as pbe, \
             tc.tile_pool(name="pBo", bufs=3) as pbo, \
             tc.tile_pool(name="pBn", bufs=2) as pbn, \
             tc.tile_pool(name="ppS", bufs=3, space="PSUM") as pps, \
             tc.tile_pool(name="ppO", bufs=2, space="PSUM") as ppo, \
             tc.tile_pool(name="ppD", bufs=1, space="PSUM") as ppd, \
             tc.tile_pool(name="ppC", bufs=2, space="PSUM") as ppc:
            ckvg = pb.tile([128, CT, S], BF16)
            for t in range(CT):
                nc.gpsimd.dma_start(
                    out=ckvg[:, t, :].rearrange("p (j c) -> p j c", j=NC),
                    in_=ag_out[:, t * 128:(t + 1) * 128, :].rearrange(
                        "j p c -> p j c"))
            kpdg = pb.tile([RP, S], BF16)
            nc.gpsimd.dma_start(out=kpdg[:].rearrange("p (j c) -> p j c", j=NC),
                              in_=ag_out[:, KVL:KVL + RP, :].rearrange(
                                  "j p c -> p j c"))
            qt = [pb.tile([128, S], BF16, name=f"qt{h}") for h in range(HPC)]
            qpt = [pb.tile([RP, S], BF16, name=f"qpt{h}") for h in range(HPC)]

            def unpack_q(h):
                nc.gpsimd.dma_start(
                    out=qt[h][:].rearrange("p (j c) -> p j c", j=NC),
                    in_=aa_out[h][:, 0:NOPE, :].rearrange("j p c -> p j c"))
                nc.gpsimd.dma_start(
                    out=qpt[h][:].rearrange("p (j c) -> p j c", j=NC),
                    in_=aa_out[h][:, NOPE:QD, :].rearrange("j p c -> p j c"))

            # --- kv_b: kn per head, v (both heads) keys-on-partitions ---
            kn = [pb.tile([128, S], BF16, name=f"kn{h}") for h in range(HPC)]
            for h in range(HPC):
                for cb in range(S // SSH):
                    p_k = ppc.tile([128, SSH], F32, tag="c")
                    for t in range(CT):
                        nc.tensor.matmul(p_k[:], wk[:, t, h * NOPE:(h + 1) * NOPE],
                                         ckvg[:, t, cb * SSH:(cb + 1) * SSH],
                                         start=(t == 0), stop=(t == CT - 1))
                    nc.any.tensor_copy(kn[h][:, cb * SSH:(cb + 1) * SSH], p_k[:])
            vst = pb.tile([128, S // 128, HPC * VD], BF16)
            for sb in range(S // 128):
                p_v = ppc.tile([128, HPC * VD], F32, tag="c")
                for t in range(CT):
                    nc.tensor.matmul(p_v[:], ckvg[:, t, sb * 128:(sb + 1) * 128],
                                     wv[:, t, :], start=(t == 0), stop=(t == CT - 1))
                nc.any.tensor_copy(vst[:, sb, :], p_v[:])

            # --- attention: heads outer (matches AllToAll arrival).
            # Software-pipelined: AV/den for ik are issued after the scores
            # of ik+1 so the PE never stalls on the exp; the per-(qb,h)
            # normalization finisher is deferred into the next iteration's
            # matmul stream.
            ao = pb.tile([128, NSB, HPC, SB], BF16)
            pending = None

            def finisher(fin):
                h, qb, p_o, p_d = fin
                den = pbn.tile([1, SB], BF16, tag="den")
                nc.vector.tensor_copy(den[:], p_d[:])
                p_b = ppc.tile([128, SB], F32, tag="c", name="bcast")
                nc.tensor.matmul(p_b[:], ones_r[:], den[:], start=True, stop=True)
                rec = pbn.tile([128, SB], F32, tag="rec")
                nc.vector.reciprocal(rec[:], p_b[:])
                nc.vector.tensor_mul(ao[:, qb, h, :], p_o[:], rec[:])

            def oproj(qb):
                for st in range(SB // 128):
                    sc = slice(qb * SB + st * 128, qb * SB + (st + 1) * 128)
                    ot = pbo.tile([128, HID], BF16, tag="ot")
                    for nb in range(HID // SB):
                        ncols = bass.ts(nb, SB)
                        p_c = ppc.tile([128, SB], F32, tag="c")
                        for hh in range(HPC):
                            nc.tensor.matmul(
                                p_c[:], ao[:, qb, hh, st * 128:(st + 1) * 128],
                                wo[:, hh, ncols],
                                start=(hh == 0), stop=(hh == HPC - 1))
                        nc.vector.tensor_copy(ot[:, ncols], p_c[:])
                    nc.sync.dma_start(out=d_out[sc, :], in_=ot[:])

            for h in range(HPC):
                unpack_q(h)
                for qb in range(NSB):
                    qcols = bass.ts(qb, SB)
                    nk = 4 * (qb + 1)
                    p_o = ppo.tile([128, SB], F32, tag="o")
                    p_d = ppd.tile([1, SB], F32, tag="d")
                    ework = []

                    def av_den(pik, pe_):
                        nc.tensor.matmul(p_o[:], vst[:, pik, h * VD:(h + 1) * VD],
                                         pe_[:], start=(pik == 0),
                                         stop=(pik == nk - 1))
                        nc.tensor.matmul(p_d[:], ones_c[:], pe_[:],
                                         start=(pik == 0), stop=(pik == nk - 1))

                    for ik in range(nk):
                        kc = slice(ik * 128, (ik + 1) * 128)
                        p_s = pps.tile([128, SB], F32, tag="s")
                        nc.tensor.matmul(p_s[:], kn[h][:, kc], qt[h][:, qcols],
                                         start=True, stop=False)
                        nc.tensor.matmul(p_s[:], kpdg[:, kc], qpt[h][:, qcols],
                                         start=False, stop=True)
                        if ik == 2 and pending is not None:
                            fin, oqb = pending
                            finisher(fin)
                            pending = None
                            if oqb is not None:
                                oproj(oqb)
                        if len(ework) == 2:
                            av_den(*ework.pop(0))
                        r = ik - 4 * qb
                        if r >= 0:
                            nc.vector.tensor_add(p_s[:], p_s[:], msk[:, r, :])
                        e = pbe.tile([128, SB], BF16, tag="e")
                        nc.scalar.activation(e[:], p_s[:], Exp, scale=SCALE)
                        ework.append((ik, e))
                    for item in ework:
                        av_den(*item)
                    pending = ((h, qb, p_o, p_d),
                               qb if h == HPC - 1 else None)
            fin, oqb = pending
            finisher(fin)
            if oqb is not None:
                oproj(oqb)


def _host_constants():
    inv_freq = 1.0 / (ROPE_THETA ** (np.arange(0, RP, 2, dtype=np.float32) / RP))
    t = np.arange(S, dtype=np.float32)
    freqs = np.outer(t, inv_freq)
    emb = np.concatenate([freqs, freqs], -1)          # [S, 64]
    cos, sin = np.cos(emb), np.sin(emb)
    cosd = np.concatenate([cos.T, cos.T], 0).astype(np.float32)   # [128, S]
    sind = np.concatenate([sin.T, sin.T], 0).astype(np.float32)

    # additive causal mask for diagonal 128-key blocks: [128, 4, 512]
    mska = np.zeros((128, 4, SB), np.float32)
    for r in range(4):
        for p in range(128):
            mska[p, r, :p + 128 * r] = NEG
    # rotate-half as matmul lhsT: same as baseline
    Q = np.zeros((RP, RP), np.float32)
    for i in range(RP // 2):
        Q[i, i + RP // 2] = -1.0
        Q[i + RP // 2, i] = 1.0
    P = np.zeros((128, 128), np.float32)
    P[:RP, :RP] = Q
    P[RP:, RP:] = Q
    rotq = P.T.copy()
    return cosd, sind, mska, rotq


def _tile3(w, kt):
    """[kt*128, F] -> [128, kt, F]"""
    return np.ascontiguousarray(
        w.reshape(kt, 128, w.shape[1]).transpose(1, 0, 2))


def kernel(hidden_states, w_q_a, q_a_weight, w_q_b, w_kv_a, kv_a_weight,
           w_kv_b, w_o):
    global LAST_RESULT
    if "nc" not in _CACHE:
        _CACHE["nc"] = _build_program()
    nc = _CACHE["nc"]

    x = np.asarray(hidden_states, np.float32)[0]       # [S, 2048]
    xt = np.ascontiguousarray(x.T)                     # [2048, S]
    wqa_t = np.asarray(w_q_a, np.float32).T            # [HID, QL]
    wkva_t = np.asarray(w_kv_a, np.float32).T          # [HID, KVL+RP]
    wqb_eff = np.asarray(w_q_b, np.float32) * np.asarray(q_a_weight, np.float32)[None, :]
    wkvb_eff = np.asarray(w_kv_b, np.float32) * np.asarray(kv_a_weight, np.float32)[None, :]
    won = np.asarray(w_o, np.float32)                  # [HID, H*VD]

    # q_b output feature permutation: nope head-major, then rope packed 2/tile
    perm = np.zeros(H * QD, np.int64)
    for h in range(H):
        perm[h * NOPE:(h + 1) * NOPE] = h * QD + np.arange(NOPE)
    base = H * NOPE
    for d in range(NC):
        for j in range(HPC):
            hh = 2 * d + j
            perm[base + d * 128 + j * RP: base + d * 128 + (j + 1) * RP] = \
                hh * QD + NOPE + np.arange(RP)
    wqb_p = wqb_eff[perm, :]                           # [3072, QL]

    cosd, sind, mska, rotq = _host_constants()

    wqa16 = _tile3(wqa_t, KT).astype(NPBF)
    wkva16 = _tile3(wkva_t, KT).astype(NPBF)
    wqb16 = _tile3(np.ascontiguousarray(wqb_p.T), QLT).astype(NPBF)
    rotq16 = rotq.astype(NPBF)

    shared = {"wqa16": wqa16, "wkva16": wkva16, "wqb16": wqb16,
              "maskadd": mska, "rotq16": rotq16}

    in_maps = []
    for c in range(NC):
        h0, h1 = HPC * c, HPC * c + 1
        wk_t = np.concatenate(
            [wkvb_eff[h * (NOPE + VD):h * (NOPE + VD) + NOPE] for h in (h0, h1)],
            0).T                                        # [KVL, 256]
        wv_t = np.concatenate(
            [wkvb_eff[h * (NOPE + VD) + NOPE:(h + 1) * (NOPE + VD)] for h in (h0, h1)],
            0).T                                        # [KVL, 256]
        wo_t = np.stack(
            [np.ascontiguousarray(won[:, h * VD:(h + 1) * VD].T) for h in (h0, h1)],
            1)                                          # [128, 2, HID]
        cols = slice(c * SSH, (c + 1) * SSH)
        im = dict(shared)
        im.update({
            "xt16": _tile3(np.ascontiguousarray(xt[:, cols]), KT).astype(NPBF),
            "wk16": _tile3(wk_t, CT).astype(NPBF),
            "wv16": _tile3(wv_t, CT).astype(NPBF),
            "wo16": np.ascontiguousarray(wo_t).astype(NPBF),
            "cosd": np.ascontiguousarray(cosd[:, cols]).astype(NPBF),
            "sind": np.ascontiguousarray(sind[:, cols]).astype(NPBF),
        })
        in_maps.append(im)

    res = run_bass_kernel_spmd(nc, in_maps, list(range(NC)))
    LAST_RESULT = res
    out = np.zeros((S, HID), np.float32)
    for c in range(NC):
        out += np.asarray(res.results[c]["out"]).astype(np.float32)
    return out.reshape(1, S, HID)


# revision 41
# speedup vs baseline: 1.8649x; 1.0003x over previous
"""MLA (DeepSeek-style multi-head latent attention) forward on 8 trn2 cores.

Layout v2: sequence-sharded LoRA-A + device collectives + bf16 matmuls.

Each core computes the LoRA-A projections (q_latent, compressed-kv latent,
k_pe) only for its 256-column sequence shard (8x less replicated work than
pure head-TP). The normalized kv latent + rope'd k_pe are AllGathered
(shared by every head); the per-head q vectors are redistributed with two
AllToAlls (one per head of each core's head pair) so attention runs fully
head-local: core c owns heads 2c, 2c+1 over the full sequence. kv_b expands
kn/v from the gathered latent per head; o_proj is input-split on heads and
the partial products are summed on the host (the unshard step).

All matmuls run in bf16 (1 PE cycle/row regardless of free-dim size, half
the DMA/communication bytes of fp32; final accuracy ~4e-3 vs the 2e-2
gate). Softmax runs over the key (partition) axis: exp on the scalar
engine, denominator via a ones-column matmul, broadcast of per-column
scalars via a K=1 matmul. RoPE rotate-half is a matmul against a constant
signed permutation. o_proj results are DMA'd directly from PSUM.
"""
import numpy as np
import ml_dtypes

import concourse.bass as bass
import concourse.tile as tile
from concourse import bacc, mybir
from concourse.bass_utils import run_bass_kernel_spmd

F32 = mybir.dt.float32
BF16 = mybir.dt.bfloat16
NPBF = ml_dtypes.bfloat16

HID = 2048
S = 2048
H = 16
QL = 1536
KVL = 512
NOPE = 128
RP = 64
VD = 128
QD = NOPE + RP              # 192
SCALE = QD ** -0.5
EPS = 1e-6
ROPE_THETA = 10000.0

NC = 8
HPC = 2                     # heads per core
SSH = S // NC               # 256-seq shard
KT = HID // 128             # 16
QLT = QL // 128             # 12
CT = KVL // 128             # 4
SB = 512                    # attention query block
NSB = S // SB               # 4
NEG = -30000.0

_CACHE = {}
LAST_RESULT = None


def _build_program():
    nc = bacc.Bacc("TRN2", target_bir_lowering=False, debug=False,
                   num_devices=NC)
    d_xt = nc.dram_tensor("xt16", [128, KT, SSH], BF16, kind="ExternalInput").ap()
    d_wqa = nc.dram_tensor("wqa16", [128, KT, QL], BF16, kind="ExternalInput").ap()
    d_wkva = nc.dram_tensor("wkva16", [128, KT, KVL + RP], BF16, kind="ExternalInput").ap()
    d_wqb = nc.dram_tensor("wqb16", [128, QLT, H * QD], BF16, kind="ExternalInput").ap()
    d_wk = nc.dram_tensor("wk16", [128, CT, HPC * NOPE], BF16, kind="ExternalInput").ap()
    d_wv = nc.dram_tensor("wv16", [128, CT, HPC * VD], BF16, kind="ExternalInput").ap()
    d_wo = nc.dram_tensor("wo16", [128, HPC, HID], BF16, kind="ExternalInput").ap()
    d_cos = nc.dram_tensor("cosd", [128, SSH], BF16, kind="ExternalInput").ap()
    d_sin = nc.dram_tensor("sind", [128, SSH], BF16, kind="ExternalInput").ap()
    d_msk = nc.dram_tensor("maskadd", [128, 4, SB], F32, kind="ExternalInput").ap()
    d_rotq = nc.dram_tensor("rotq16", [128, 128], BF16, kind="ExternalInput").ap()
    d_out = nc.dram_tensor("out", [S, HID], BF16, kind="ExternalOutput").ap()

    with tile.TileContext(nc) as tc:
        _mla(tc, d_xt, d_wqa, d_wkva, d_wqb, d_wk, d_wv, d_wo, d_cos, d_sin,
             d_msk, d_rotq, d_out)
    nc.compile()
    return nc


def _mla(tc, d_xt, d_wqa, d_wkva, d_wqb, d_wk, d_wv, d_wo, d_cos, d_sin,
         d_msk, d_rotq, d_out):
    nc = tc.nc
    Exp = mybir.ActivationFunctionType.Exp
    Sqrt = mybir.ActivationFunctionType.Sqrt
    groups = [list(range(NC))]

    with nc.allow_low_precision(reason="bf16 pipeline"), \
         tc.tile_pool(name="pdram", bufs=1, space="DRAM") as pdram, \
         tc.tile_pool(name="pconst", bufs=1) as pc, \
         tc.tile_pool(name="pglob", bufs=1) as pg:
        # ---- DRAM bounce buffers for collectives ----
        ag_in = pdram.tile([KVL + RP, SSH], BF16)
        ag_out = pdram.tile([NC, KVL + RP, SSH], BF16)
        aa_in = [pdram.tile([NC, QD, SSH], BF16, name=f"aain{i}") for i in range(HPC)]
        aa_out = [pdram.tile([NC, QD, SSH], BF16, name=f"aaout{i}") for i in range(HPC)]

        # ---- small constants ----
        ones_c = pc.tile([128, 1], BF16)
        nc.vector.memset(ones_c, 1.0)
        ones_r = pc.tile([1, 128], BF16)
        nc.vector.memset(ones_r, 1.0)
        eps1 = pc.tile([1, 1], F32)
        nc.vector.memset(eps1, EPS)
        warm = pc.tile([1, 2], F32, name="actwarm")
        nc.scalar.activation(warm[0:1, 0:1], eps1[:], Sqrt)
        nc.scalar.activation(warm[0:1, 1:2], eps1[:], Exp)
        rotq = pc.tile([128, 128], BF16)
        nc.sync.dma_start(out=rotq, in_=d_rotq)
        cosd = pc.tile([128, SSH], BF16)
        nc.sync.dma_start(out=cosd, in_=d_cos)
        sind = pc.tile([128, SSH], BF16)
        nc.sync.dma_start(out=sind, in_=d_sin)

        # =============== stage A: shard projections ===============
        with tc.tile_pool(name="pw", bufs=1) as pw, \
             tc.tile_pool(name="pA", bufs=1) as pa, \
             tc.tile_pool(name="pAq", bufs=3) as paq, \
             tc.tile_pool(name="pAs", bufs=3) as pas, \
             tc.tile_pool(name="ppA", bufs=3, space="PSUM") as ppa, \
             tc.tile_pool(name="ppSt", bufs=2, space="PSUM") as ppst, \
             tc.tile_pool(name="ppM", bufs=2, space="PSUM") as ppm:
            # ---- stage-A weights: per-k-slice tiles so matmuls start early ----
            xt = pw.tile([128, KT, SSH], BF16)
            # Early (eager) loads on the SP HW queue: only what the first
            # ~20us of compute needs. Everything else is loaded via gpsimd
            # SWDGE triggers placed AFTER the AllGather in program order, so
            # those transfers enter the global DMA FIFO behind the
            # collective staging instead of ahead of it.
            wkva = pw.tile([128, KT, KVL + RP], BF16)
            for half in range(2):
                hk = slice(half * KT // 2, (half + 1) * KT // 2)
                nc.sync.dma_start(out=xt[:, hk, :], in_=d_xt[:, hk, :])
                nc.sync.dma_start(out=wkva[:, hk, :], in_=d_wkva[:, hk, :])
            wqa = pw.tile([128, KT, QL], BF16)
            # gate wqa-colA behind wkva via a write-after-read dep: the
            # reader consumes both the colA region and the wkva tail, so the
            # colA DMA (a writer of that region) must wait for wkva.
            gate = pc.tile([1, 2], BF16, name="gate")
            nc.vector.tensor_tensor(gate[0:1, 0:1], wqa[0:1, 0, 0:1],
                                    wkva[0:1, KT - 1, 0:1], mybir.AluOpType.mult)
            nc.scalar.dma_start(out=wqa[:, :, 0:QL // 2], in_=d_wqa[:, :, 0:QL // 2])
            wqb = pw.tile([128, QLT, H * QD], BF16)
            wk = pg.tile([128, CT, HPC * NOPE], BF16)
            wv = pg.tile([128, CT, HPC * VD], BF16)
            wo = pg.tile([128, HPC, HID], BF16)
            msk = pg.tile([128, 4, SB], F32)

            # --- kv LoRA-A ---
            ckvu = pa.tile([128, CT, SSH], BF16)
            kpe = pa.tile([RP, SSH], BF16)
            p_st = ppst.tile([1, SSH], F32, tag="st", name="cstat")
            sqc = pa.tile([128, CT, SSH], BF16, name="sqc")
            for m in range(CT + 1):
                mw = 128 if m < CT else RP
                p_a = ppa.tile([128, SSH], F32, tag="a")
                for k in range(KT):
                    nc.tensor.matmul(p_a[:mw, :], wkva[:, k, m * 128:m * 128 + mw],
                                     xt[:, k, :], start=(k == 0), stop=(k == KT - 1))
                if m < CT:
                    nc.vector.tensor_copy(ckvu[:, m, :], p_a[:])
                    nc.vector.tensor_mul(sqc[:, m, :], ckvu[:, m, :], ckvu[:, m, :])
                else:
                    nc.vector.tensor_copy(kpe[:], p_a[:mw, :])
            for m in range(CT):
                nc.tensor.matmul(p_st[:], ones_c[:], sqc[:, m, :],
                                 start=(m == 0), stop=(m == CT - 1))
            rms_c = pa.tile([1, SSH], BF16)
            nc.scalar.activation(rms_c[:], p_st[:], Sqrt, scale=1.0 / KVL,
                                 bias=eps1[:])
            p_bc = ppm.tile([128, SSH], F32, tag="m")
            nc.tensor.matmul(p_bc[:], ones_r[:], rms_c[:], start=True, stop=True)
            invc = pa.tile([128, SSH], BF16)
            nc.vector.reciprocal(invc[:], p_bc[:])
            ckv = pa.tile([128, CT, SSH], BF16)
            for m in range(CT):
                nc.vector.tensor_mul(ckv[:, m, :], ckvu[:, m, :], invc[:])
            nc.scalar.dma_start(
                out=ag_in[0:KVL, :].rearrange("(t p) c -> p t c", p=128),
                in_=ckv[:])
            # --- k_pe rope (scale folded: none needed in bf16) ---
            p_rk = ppm.tile([128, SSH], F32, tag="m", name="rotk")
            nc.tensor.matmul(p_rk[:RP, :], rotq[0:RP, 0:RP], kpe[:],
                             start=True, stop=True)
            rk16 = pas.tile([RP, SSH], BF16, tag="rk")
            nc.vector.tensor_copy(rk16[:], p_rk[:RP, :])
            t1 = pas.tile([RP, SSH], BF16, tag="t1")
            nc.vector.tensor_mul(t1[:], kpe[:], cosd[0:RP, :])
            t2 = pas.tile([RP, SSH], BF16, tag="t2")
            nc.vector.tensor_mul(t2[:], rk16[:], sind[0:RP, :])
            kpd = pa.tile([RP, SSH], BF16)
            nc.vector.tensor_add(kpd[:], t1[:], t2[:])
            nc.scalar.dma_start(out=ag_in[KVL:KVL + RP, :], in_=kpd[:])
            # --- collective #1: AllGather latent+kpe ---
            nc.gpsimd.collective_compute(
                "AllGather", mybir.AluOpType.bypass, replica_groups=groups,
                ins=[ag_in[:].opt()], outs=[ag_out[:].opt()])
            # deferred bulk weight loads, chained with write-after-read
            # gates so each transfer enters the exclusive DMA FIFO after the
            # AllGather staging and after the previous weight transfer.
            agmark = pc.tile([1, 2], BF16, name="agmark")
            nc.gpsimd.dma_start(out=agmark[0:1, 0:2],
                                in_=ag_in[KVL + RP - 1:KVL + RP, 0:2])
            Mul = mybir.AluOpType.mult

            def gate_read(region, token):
                g = pas.tile([1, 1], BF16, tag="g8")
                nc.vector.tensor_tensor(g[:], region, token, Mul)

            gate_read(wqa[0:1, 0, QL - 1:QL], ckv[0:1, CT - 1, 0:1])
            nc.gpsimd.dma_start(out=wqa[:, :, QL // 2:QL],
                                in_=d_wqa[:, :, QL // 2:QL])
            gate_read(wqb[0:1, 0, H * QD - 1:H * QD], agmark[0:1, 0:1])
            nc.gpsimd.dma_start(out=wqb[:, :, H * NOPE:H * QD],
                                in_=d_wqb[:, :, H * NOPE:H * QD])
            for q4 in range(4):
                qs = slice(q4 * H * NOPE // 4, (q4 + 1) * H * NOPE // 4)
                gate_read(wqb[0:1, 0, q4 * H * NOPE // 4:q4 * H * NOPE // 4 + 1],
                          wqb[0:1, 0, H * QD - 1:H * QD])
                nc.gpsimd.dma_start(out=wqb[:, :, qs], in_=d_wqb[:, :, qs])
            for wtile, dsrc in ((wk, d_wk), (wv, d_wv), (wo, d_wo), (msk, d_msk)):
                gate_read(wtile[0:1, 0, 0:1], wqb[0:1, 0, H * NOPE - 1:H * NOPE])
                nc.gpsimd.dma_start(out=wtile, in_=dsrc)

            # --- q LoRA-A ---
            qlu = pa.tile([128, QLT, SSH], BF16)
            p_qst = ppst.tile([1, SSH], F32, tag="st", name="qstat")
            sqq = pa.tile([128, QLT, SSH], BF16, name="sqq")
            for k in range(QLT):
                p_a = ppa.tile([128, SSH], F32, tag="a")
                for kk in range(KT):
                    nc.tensor.matmul(p_a[:], wqa[:, kk, k * 128:(k + 1) * 128],
                                     xt[:, kk, :], start=(kk == 0), stop=(kk == KT - 1))
                nc.vector.tensor_copy(qlu[:, k, :], p_a[:])
                nc.vector.tensor_mul(sqq[:, k, :], qlu[:, k, :], qlu[:, k, :])
            for k in range(QLT):
                nc.tensor.matmul(p_qst[:], ones_c[:], sqq[:, k, :],
                                 start=(k == 0), stop=(k == QLT - 1))
            rms_q = pa.tile([1, SSH], BF16)
            nc.scalar.activation(rms_q[:], p_qst[:], Sqrt, scale=1.0 / QL,
                                 bias=eps1[:])
            p_bq = ppm.tile([128, SSH], F32, tag="m")
            nc.tensor.matmul(p_bq[:], ones_r[:], rms_q[:], start=True, stop=True)
            invq = pa.tile([128, SSH], F32)
            nc.vector.reciprocal(invq[:], p_bq[:])

            # --- q_b for all heads: rope tiles (16..23) first so the rope
            # chain and the AllToAll staging DMAs start as early as possible;
            # nope tiles follow in parity order (A2A#1's inputs first).
            q16 = pa.tile([128, H + NC, SSH], BF16, name="q16")

            def qb_group(mt):
                p_q = ppa.tile([128, SSH], F32, tag="a")
                for k in range(QLT):
                    nc.tensor.matmul(p_q[:], wqb[:, k, mt * 128:(mt + 1) * 128],
                                     qlu[:, k, :], start=(k == 0), stop=(k == QLT - 1))
                nc.vector.tensor_mul(q16[:, mt, :], p_q[:], invq[:])

            for mt in range(H, H + NC):
                qb_group(mt)
            # rope rotate-half + cos/sin (inputs ready; no PE stalls)
            for d in range(NC):
                p_rq = ppm.tile([128, SSH], F32, tag="m")
                nc.tensor.matmul(p_rq[:], rotq[:], q16[:, H + d, :],
                                 start=True, stop=True)
                rq16 = pas.tile([128, SSH], BF16, tag="rk", name="rq16")
                nc.vector.tensor_copy(rq16[:], p_rq[:])
                t1q = pas.tile([128, SSH], BF16, tag="t1")
                nc.vector.tensor_mul(t1q[:], q16[:, H + d, :], cosd[:])
                t2q = pas.tile([128, SSH], BF16, tag="t2")
                nc.vector.tensor_mul(t2q[:], rq16[:], sind[:])
                nc.vector.tensor_add(q16[:, H + d, :], t1q[:], t2q[:])
            for mt in range(0, H, 2):
                qb_group(mt)
            nc.gpsimd.dma_start(
                out=aa_in[0][:, 0:NOPE, :].rearrange("j p c -> p j c"),
                in_=q16[:, 0:H:2, :].rearrange("p j c -> p j c"))
            nc.gpsimd.dma_start(
                out=aa_in[0][:, NOPE:QD, :].rearrange("j p c -> p j c"),
                in_=q16[0:RP, H:H + NC, :])
            nc.gpsimd.collective_compute(
                "AllToAll", mybir.AluOpType.bypass, replica_groups=groups,
                ins=[aa_in[0][:].opt()], outs=[aa_out[0][:].opt()])
            for mt in range(1, H, 2):
                qb_group(mt)
            nc.gpsimd.dma_start(
                out=aa_in[1][:, 0:NOPE, :].rearrange("j p c -> p j c"),
                in_=q16[:, 1:H:2, :].rearrange("p j c -> p j c"))
            nc.gpsimd.dma_start(
                out=aa_in[1][:, NOPE:QD, :].rearrange("j p c -> p j c"),
                in_=q16[RP:128, H:H + NC, :])
            nc.gpsimd.collective_compute(
                "AllToAll", mybir.AluOpType.bypass, replica_groups=groups,
                ins=[aa_in[1][:].opt()], outs=[aa_out[1][:].opt()])

        # =============== stage B: head-local attention ===============
        with tc.tile_pool(name="pB", bufs=1) as pb, \
             tc.tile_pool(name="pBe", bufs=10) as pbe, \
             tc.tile_pool(name="pBo", bufs=4) as pbo, \
             tc.tile_pool(name="pBn", bufs=4) as pbn, \
             tc.tile_pool(name="ppS", bufs=3, space="PSUM") as pps, \
             tc.tile_pool(name="ppO", bufs=2, space="PSUM") as ppo, \
             tc.tile_pool(name="ppD", bufs=1, space="PSUM") as ppd, \
             tc.tile_pool(name="ppC", bufs=2, space="PSUM") as ppc:
            ckvg = pb.tile([128, CT, S], BF16)
            for t in range(CT):
                nc.gpsimd.dma_start(
                    out=ckvg[:, t, :].rearrange("p (j c) -> p j c", j=NC),
                    in_=ag_out[:, t * 128:(t + 1) * 128, :].rearrange(
                        "j p c -> p j c"))
            kpdg = pb.tile([RP, S], BF16)
            nc.gpsimd.dma_start(out=kpdg[:].rearrange("p (j c) -> p j c", j=NC),
                              in_=ag_out[:, KVL:KVL + RP, :].rearrange(
                                  "j p c -> p j c"))
            qt = [pb.tile([128, S], BF16, name=f"qt{h}") for h in range(HPC)]
            qpt = [pb.tile([RP, S], BF16, name=f"qpt{h}") for h in range(HPC)]

            def unpack_q(h):
                nc.gpsimd.dma_start(
                    out=qt[h][:].rearrange("p (j c) -> p j c", j=NC),
                    in_=aa_out[h][:, 0:NOPE, :].rearrange("j p c -> p j c"))
                nc.gpsimd.dma_start(
                    out=qpt[h][:].rearrange("p (j c) -> p j c", j=NC),
                    in_=aa_out[h][:, NOPE:QD, :].rearrange("j p c -> p j c"))

            # --- kv_b: kn per head, v (both heads) keys-on-partitions ---
            kn = [pb.tile([128, S], BF16, name=f"kn{h}") for h in range(HPC)]
            for h in range(HPC):
                for cb in range(S // SSH):
                    p_k = ppc.tile([128, SSH], F32, tag="c")
                    for t in range(CT):
                        nc.tensor.matmul(p_k[:], wk[:, t, h * NOPE:(h + 1) * NOPE],
                                         ckvg[:, t, cb * SSH:(cb + 1) * SSH],
                                         start=(t == 0), stop=(t == CT - 1))
                    nc.any.tensor_copy(kn[h][:, cb * SSH:(cb + 1) * SSH], p_k[:])
            vst = pb.tile([128, S // 128, HPC * VD], BF16)
            for sb in range(S // 128):
                p_v = ppc.tile([128, HPC * VD], F32, tag="c")
                for t in range(CT):
                    nc.tensor.matmul(p_v[:], ckvg[:, t, sb * 128:(sb + 1) * 128],
                                     wv[:, t, :], start=(t == 0), stop=(t == CT - 1))
                nc.any.tensor_copy(vst[:, sb, :], p_v[:])

            # --- attention: heads outer (matches AllToAll arrival).
            # Software-pipelined: AV/den for ik are issued after the scores
            # of ik+1 so the PE never stalls on the exp; the per-(qb,h)
            # normalization finisher is deferred into the next iteration's
            # matmul stream.
            ao = pb.tile([128, NSB, HPC, SB], BF16)
            pending = None

            def finisher(fin):
                h, qb, p_o, p_d = fin
                den = pbn.tile([1, SB], BF16, tag="den")
                nc.vector.tensor_copy(den[:], p_d[:])
                p_b = ppc.tile([128, SB], F32, tag="c", name="bcast")
                nc.tensor.matmul(p_b[:], ones_r[:], den[:], start=True, stop=True)
                rec = pbn.tile([128, SB], F32, tag="rec")
                nc.vector.reciprocal(rec[:], p_b[:])
                nc.vector.tensor_mul(ao[:, qb, h, :], p_o[:], rec[:])

            def oproj(qb):
                for st in range(SB // 128):
                    sc = slice(qb * SB + st * 128, qb * SB + (st + 1) * 128)
                    ot = pbo.tile([128, HID], BF16, tag="ot")
                    for nb in range(HID // SB):
                        ncols = bass.ts(nb, SB)
                        p_c = ppc.tile([128, SB], F32, tag="c")
                        for hh in range(HPC):
                            nc.tensor.matmul(
                                p_c[:], ao[:, qb, hh, st * 128:(st + 1) * 128],
                                wo[:, hh, ncols],
                                start=(hh == 0), stop=(hh == HPC - 1))
                        nc.vector.tensor_copy(ot[:, ncols], p_c[:])
                    nc.sync.dma_start(out=d_out[sc, :], in_=ot[:])

            for h in range(HPC):
                unpack_q(h)
                for qb in range(NSB):
                    qcols = bass.ts(qb, SB)
                    nk = 4 * (qb + 1)
                    p_o = ppo.tile([128, SB], F32, tag="o")
                    p_d = ppd.tile([1, SB], F32, tag="d")
                    ework = []

                    def av_den(pik, pe_):
                        nc.tensor.matmul(p_o[:], vst[:, pik, h * VD:(h + 1) * VD],
                                         pe_[:], start=(pik == 0),
                                         stop=(pik == nk - 1))
                        nc.tensor.matmul(p_d[:], ones_c[:], pe_[:],
                                         start=(pik == 0), stop=(pik == nk - 1))

                    for ik in range(nk):
                        kc = slice(ik * 128, (ik + 1) * 128)
                        p_s = pps.tile([128, SB], F32, tag="s")
                        nc.tensor.matmul(p_s[:], kn[h][:, kc], qt[h][:, qcols],
                                         start=True, stop=False)
                        nc.tensor.matmul(p_s[:], kpdg[:, kc], qpt[h][:, qcols],
                                         start=False, stop=True)
                        if ik == 2 and pending is not None:
                            fin, oqb = pending
                            finisher(fin)
                            pending = None
                            if oqb is not None:
                                oproj(oqb)
                        if len(ework) == 2:
                            av_den(*ework.pop(0))
                        r = ik - 4 * qb
                        if r >= 0:
                            nc.vector.tensor_add(p_s[:], p_s[:], msk[:, r, :])
                        e = pbe.tile([128, SB], BF16, tag="e")
                        nc.scalar.activation(e[:], p_s[:], Exp, scale=SCALE)
                        ework.append((ik, e))
                    for item in ework:
                        av_den(*item)
                    pending = ((h, qb, p_o, p_d),
                               qb if h == HPC - 1 else None)
            fin, oqb = pending
            finisher(fin)
            if oqb is not None:
                oproj(oqb)


def _host_constants():
    inv_freq = 1.0 / (ROPE_THETA ** (np.arange(0, RP, 2, dtype=np.float32) / RP))
    t = np.arange(S, dtype=np.float32)
    freqs = np.outer(t, inv_freq)
    emb = np.concatenate([freqs, freqs], -1)          # [S, 64]
    cos, sin = np.cos(emb), np.sin(emb)
    cosd = np.concatenate([cos.T, cos.T], 0).astype(np.float32)   # [128, S]
    sind = np.concatenate([sin.T, sin.T], 0).astype(np.float32)

    # additive causal mask for diagonal 128-key blocks: [128, 4, 512]
    mska = np.zeros((128, 4, SB), np.float32)
    for r in range(4):
        for p in range(128):
            mska[p, r, :p + 128 * r] = NEG
    # rotate-half as matmul lhsT: same as baseline
    Q = np.zeros((RP, RP), np.float32)
    for i in range(RP // 2):
        Q[i, i + RP // 2] = -1.0
        Q[i + RP // 2, i] = 1.0
    P = np.zeros((128, 128), np.float32)
    P[:RP, :RP] = Q
    P[RP:, RP:] = Q
    rotq = P.T.copy()
    return cosd, sind, mska, rotq


def _tile3(w, kt):
    """[kt*128, F] -> [128, kt, F]"""
    return np.ascontiguousarray(
        w.reshape(kt, 128, w.shape[1]).transpose(1, 0, 2))


def kernel(hidden_states, w_q_a, q_a_weight, w_q_b, w_kv_a, kv_a_weight,
           w_kv_b, w_o):
    global LAST_RESULT
    if "nc" not in _CACHE:
        _CACHE["nc"] = _build_program()
    nc = _CACHE["nc"]

    x = np.asarray(hidden_states, np.float32)[0]       # [S, 2048]
    xt = np.ascontiguousarray(x.T)                     # [2048, S]
    wqa_t = np.asarray(w_q_a, np.float32).T            # [HID, QL]
    wkva_t = np.asarray(w_kv_a, np.float32).T          # [HID, KVL+RP]
    wqb_eff = np.asarray(w_q_b, np.float32) * np.asarray(q_a_weight, np.float32)[None, :]
    wkvb_eff = np.asarray(w_kv_b, np.float32) * np.asarray(kv_a_weight, np.float32)[None, :]
    won = np.asarray(w_o, np.float32)                  # [HID, H*VD]

    # q_b output feature permutation: nope head-major, then rope packed 2/tile
    perm = np.zeros(H * QD, np.int64)
    for h in range(H):
        perm[h * NOPE:(h + 1) * NOPE] = h * QD + np.arange(NOPE)
    base = H * NOPE
    for d in range(NC):
        for j in range(HPC):
            hh = 2 * d + j
            perm[base + d * 128 + j * RP: base + d * 128 + (j + 1) * RP] = \
                hh * QD + NOPE + np.arange(RP)
    wqb_p = wqb_eff[perm, :]                           # [3072, QL]

    cosd, sind, mska, rotq = _host_constants()

    wqa16 = _tile3(wqa_t, KT).astype(NPBF)
    wkva16 = _tile3(wkva_t, KT).astype(NPBF)
    wqb16 = _tile3(np.ascontiguousarray(wqb_p.T), QLT).astype(NPBF)
    rotq16 = rotq.astype(NPBF)

    shared = {"wqa16": wqa16, "wkva16": wkva16, "wqb16": wqb16,
              "maskadd": mska, "rotq16": rotq16}

    in_maps = []
    for c in range(NC):
        h0, h1 = HPC * c, HPC * c + 1
        wk_t = np.concatenate(
            [wkvb_eff[h * (NOPE + VD):h * (NOPE + VD) + NOPE] for h in (h0, h1)],
            0).T                                        # [KVL, 256]
        wv_t = np.concatenate(
            [wkvb_eff[h * (NOPE + VD) + NOPE:(h + 1) * (NOPE + VD)] for h in (h0, h1)],
            0).T                                        # [KVL, 256]
        wo_t = np.stack(
            [np.ascontiguousarray(won[:, h * VD:(h + 1) * VD].T) for h in (h0, h1)],
            1)                                          # [128, 2, HID]
        cols = slice(c * SSH, (c + 1) * SSH)
        im = dict(shared)
        im.update({
            "xt16": _tile3(np.ascontiguousarray(xt[:, cols]), KT).astype(NPBF),
            "wk16": _tile3(wk_t, CT).astype(NPBF),
            "wv16": _tile3(wv_t, CT).astype(NPBF),
            "wo16": np.ascontiguousarray(wo_t).astype(NPBF),
            "cosd": np.ascontiguousarray(cosd[:, cols]).astype(NPBF),
            "sind": np.ascontiguousarray(sind[:, cols]).astype(NPBF),
        })
        in_maps.append(im)

    res = run_bass_kernel_spmd(nc, in_maps, list(range(NC)))
    LAST_RESULT = res
    out = np.zeros((S, HID), np.float32)
    for c in range(NC):
        out += np.asarray(res.results[c]["out"]).astype(np.float32)
    return out.reshape(1, S, HID)
